# revision 4
# baseline (speedup 1.0000x reference)
"""DeltaRule memory scan kernel for Trainium2, 8 NeuronCores, data-parallel over batch.

Reference semantics (per batch element, H=512, L=2048):
    M_0 = 0  [H,H]
    for t in 0..L-2:   k = hidden[t]
        d = k.k + eps; delta = k - (M k)/d; M += outer(delta, k)
    out = (M @ hidden[L-1]) @ W.T + b

Implementation: chunked delta rule (UT transform), chunk C=128, float16.
Per chunk with keys K [C,H], r = 1/(rowsum(K^2)+eps):
    A  = strict_tril(diag(r) K K^T)            [C,C]
    T  = (I+A)^{-1} ~= (I-A)(I+A^2)(I+A^4)    (A nilpotent; exact through A^7)
    U  = K - diag(r) (K M^T + X Dl_prev)       [C,H]  (X-term: intra-pair cross)
    Dl = T U                                    [C,H]
    M^T += K_c0^T Dl_c0 + K_c1^T Dl_c1          (pair-accumulated in PSUM)
All matmuls f16 (1 cy/row); M^T state kept in f16 only (validated: relerr ~1.5e-3).
K^T and A^T produced by XBAR DMA transposes (no PE transposes in steady state).
Elementwise work spread across DVE / Act / Pool engines; 4 batch elements per
core with chunk phases interleaved for PE-queue continuity.
"""
import sys
import numpy as np
from contextlib import ExitStack

sys.path.insert(0, "/opt/trn_rl_repo")

import concourse.bass as bass
import concourse.mybir as mybir
from concourse import tile
from concourse.bass_utils import run_bass_kernel_spmd
from concourse.masks import make_identity, make_lower_triangular

B, L, H = 32, 2048, 512
NCORES = 8
BPC = B // NCORES          # 4 batch elements per core
C = 128                    # chunk size
T = L - 1                  # 2047 scan steps
NCHUNK = (T + C - 1) // C  # 16 (last chunk has 127 valid rows)
NLEV = 2                   # Neumann levels -> exact through A^7 (validated)
EPS = 1e-6
HB = H // 128              # 4 h-blocks

f32 = mybir.dt.float32
f16 = mybir.dt.float16

_cached = {}

# engine split for the 16 pair-end M updates (V=vector/DVE, P=pool)
_MUPD_ENG = "PVPPVPVPPVPPVPVP"


def _build_program(legalize=True):
    nc = bass.Bass(target_bir_lowering=False, debug=False)

    hidden_d = nc.declare_dram_parameter("hidden", [BPC, L, H], f32, isOutput=False)
    w_d = nc.declare_dram_parameter("W", [H, H], f32, isOutput=False)
    b_d = nc.declare_dram_parameter("bvec", [H], f32, isOutput=False)
    out_d = nc.declare_dram_parameter("out", [BPC, H], f32, isOutput=True)

    with tile.TileContext(nc) as tc, ExitStack() as ctx:
        consts = ctx.enter_context(tc.tile_pool(name="consts", bufs=1))
        wbuild = ctx.enter_context(tc.tile_pool(name="wbuild", bufs=2))
        mtpool = ctx.enter_context(tc.tile_pool(name="mt", bufs=1))
        kpool = ctx.enter_context(tc.tile_pool(name="k", bufs=12))
        k16pool = ctx.enter_context(tc.tile_pool(name="k16", bufs=14))
        chain = ctx.enter_context(tc.tile_pool(name="chain", bufs=6))
        upool = ctx.enter_context(tc.tile_pool(name="u", bufs=6))
        dlpool = ctx.enter_context(tc.tile_pool(name="dl", bufs=10))
        xpool = ctx.enter_context(tc.tile_pool(name="x", bufs=2))
        small = ctx.enter_context(tc.tile_pool(name="small", bufs=4))
        pslo = ctx.enter_context(tc.tile_pool(name="pslo", bufs=1, space="PSUM"))
        pshi = ctx.enter_context(tc.tile_pool(name="pshi", bufs=6, space="PSUM"))

        # ---- constants needed by the main loop ----
        ident_f = consts.tile([128, 128], f32, tag="identf")
        make_identity(nc, ident_f[:])
        ident_h = consts.tile([128, 128], f16, tag="identh")
        make_identity(nc, ident_h[:])
        identp = consts.tile([128, H], f16, tag="identp")
        for bi in range(BPC):
            nc.vector.tensor_copy(identp[:, bi * 128:(bi + 1) * 128], ident_h[:])
        smask = consts.tile([128, 128], f32, tag="smask")
        make_lower_triangular(nc, smask[:], val=1.0, diag=False)

        # persistent state M^T (f16 only), per (bi, jb)
        mt16 = [[mtpool.tile([128, H], f16, tag=f"mt{bi}_{jb}", name=f"mt{bi}_{jb}")
                 for jb in range(HB)] for bi in range(BPC)]

        G = {}

        def prep_load(c):
            t0 = c * C
            nrows = min(C, T - t0)
            st = {"k": [], "k16": [], "kt16": []}
            for bi in range(BPC):
                k_t = kpool.tile([128, H], f32, tag="K", name=f"k{c}_{bi}")
                if nrows < C:
                    nc.vector.memset(k_t[:], 0.0)
                    nc.sync.dma_start(k_t[:nrows, :], hidden_d[bi, t0:t0 + nrows, :])
                else:
                    nc.sync.dma_start(k_t[:], hidden_d[bi, t0:t0 + C, :])
                st["k"].append(k_t)
            for bi in range(BPC):
                k16 = k16pool.tile([128, H], f16, tag="k16", name=f"k16_{c}_{bi}")
                nc.gpsimd.tensor_copy(k16[:], st["k"][bi][:])
                st["k16"].append(k16)
            for bi in range(BPC):
                kt16 = k16pool.tile([128, H], f16, tag="kt16", name=f"kt16_{c}_{bi}")
                nc.sync.dma_start_transpose(
                    kt16[:].rearrange("p (f q) -> p f q", q=128), st["k16"][bi][:])
                st["kt16"].append(kt16)
            G[c] = st

        def prep_r(c):
            st = G[c]
            dall = small.tile([128, BPC], f32, tag="dall")
            for bi in range(BPC):
                scr = small.tile([128, H], f16, tag="scr")
                nc.scalar.activation(scr[:], st["k"][bi][:],
                                     mybir.ActivationFunctionType.Square,
                                     accum_out=dall[:, bi:bi + 1])
            rall = small.tile([128, BPC], f32, tag="rall")
            nrall = small.tile([128, BPC], f32, tag="nrall")
            nc.vector.tensor_scalar_add(dall[:], dall[:], EPS)
            nc.vector.reciprocal(rall[:], dall[:])
            nc.vector.tensor_scalar_mul(nrall[:], rall[:], -1.0)
            st["r"], st["nr"] = rall, nrall

        def aform(c):
            st = G[c]
            a_ps = pshi.tile([128, H], f32, tag="big")
            for bi in range(BPC):
                sl = slice(bi * 128, (bi + 1) * 128)
                for hb in range(HB):
                    hsl = slice(hb * 128, (hb + 1) * 128)
                    nc.tensor.matmul(a_ps[:, sl], st["kt16"][bi][:, hsl],
                                     st["kt16"][bi][:, hsl],
                                     start=(hb == 0), stop=(hb == HB - 1))
            a_all = chain.tile([128, H], f16, tag="ak")
            for bi in range(BPC):
                sl = slice(bi * 128, (bi + 1) * 128)
                nc.vector.scalar_tensor_tensor(a_all[:, sl], a_ps[:, sl],
                                               st["r"][:, bi:bi + 1], smask[:],
                                               mybir.AluOpType.mult,
                                               mybir.AluOpType.mult)
            at_all = chain.tile([128, H], f16, tag="atk")
            nc.sync.dma_start_transpose(
                at_all[:].rearrange("p (f q) -> p f q", q=128), a_all[:])
            g0 = chain.tile([128, H], f16, tag="g")
            nc.gpsimd.tensor_sub(g0[:], identp[:], at_all[:])
            st["ak"], st["atk"], st["g"] = a_all, at_all, g0

        def chain_sq(c, lev):
            # squaring half: A^(2^lev); its transpose via XBAR DMA (free-ish)
            st = G[c]
            ak, atk = st["ak"], st["atk"]
            sq1 = pshi.tile([128, H], f32, tag="big")
            for bi in range(BPC):
                sl = slice(bi * 128, (bi + 1) * 128)
                nc.tensor.matmul(sq1[:, sl], atk[:, sl], ak[:, sl], start=True, stop=True)
            ak2 = chain.tile([128, H], f16, tag="ak")
            nc.scalar.copy(ak2[:], sq1[:])
            if lev < NLEV:
                sq2 = pshi.tile([128, H], f32, tag="big")
                for bi in range(BPC):
                    sl = slice(bi * 128, (bi + 1) * 128)
                    nc.tensor.matmul(sq2[:, sl], ak[:, sl], atk[:, sl], start=True, stop=True)
                atk2 = chain.tile([128, H], f16, tag="atk")
                nc.scalar.copy(atk2[:], sq2[:])
            else:
                atk2 = None
            st["ak_n"], st["atk_n"] = ak2, atk2

        def chain_gps(c, lev):
            # accumulation half: g += (A^(2^lev))^T g
            st = G[c]
            ak2 = st["ak_n"]
            gps = pshi.tile([128, H], f32, tag="big")
            for bi in range(BPC):
                sl = slice(bi * 128, (bi + 1) * 128)
                nc.tensor.matmul(gps[:, sl], ak2[:, sl], st["g"][:, sl], start=True, stop=True)
            g_nxt = chain.tile([128, H], f16, tag="g")
            nc.vector.tensor_add(g_nxt[:], gps[:], st["g"][:])
            st["ak"], st["atk"], st["g"] = st["ak_n"], st["atk_n"], g_nxt

        def chain_level(c, lev):
            chain_sq(c, lev)
            chain_gps(c, lev)

        def xform(c):
            # X^T for pair (c-1, c): xts[:, bi-slice][a, b] = K_{c-1}[a]·K_c[b]
            st, stp = G[c], G[c - 1]
            xps = pshi.tile([128, H], f32, tag="big")
            for bi in range(BPC):
                sl = slice(bi * 128, (bi + 1) * 128)
                for hb in range(HB):
                    hsl = slice(hb * 128, (hb + 1) * 128)
                    nc.tensor.matmul(xps[:, sl], stp["kt16"][bi][:, hsl],
                                     st["kt16"][bi][:, hsl],
                                     start=(hb == 0), stop=(hb == HB - 1))
            xts = xpool.tile([128, H], f16, tag="x")
            nc.scalar.copy(xts[:], xps[:])
            st["x"] = xts

        def state_u(c):
            # chunks 0/1 update M individually (startup); pairs start at (2,3)
            st = G[c]
            cross = (c % 2 == 1 and c >= 3)
            st["u"] = []
            for bi in range(BPC):
                if c == 0:
                    st["u"].append(st["k16"][bi])
                    continue
                ups = pshi.tile([128, H], f32, tag="big")
                for hb in range(HB):
                    hsl = slice(hb * 128, (hb + 1) * 128)
                    nc.tensor.matmul(ups[:], st["kt16"][bi][:, hsl],
                                     mt16[bi][hb][:],
                                     start=(hb == 0),
                                     stop=(hb == HB - 1 and not cross))
                if cross:
                    sl = slice(bi * 128, (bi + 1) * 128)
                    nc.tensor.matmul(ups[:], st["x"][:, sl], G[c - 1]["dl"][bi][:],
                                     start=False, stop=True)
                # u = k16 + nr*ups: Act reads PSUM with per-partition scale,
                # Pool does the all-SBUF f16 add (Pool has no PSUM port)
                usc = upool.tile([128, H], f16, tag="usc")
                nc.scalar.activation(usc[:], ups[:], mybir.ActivationFunctionType.Copy,
                                     scale=st["nr"][:, bi:bi + 1])
                u_sb = upool.tile([128, H], f16, tag="u")
                nc.gpsimd.tensor_add(u_sb[:], usc[:], st["k16"][bi][:])
                st["u"].append(u_sb)

        def state_delta(c):
            st = G[c]
            st["dl"] = []
            for bi in range(BPC):
                sl = slice(bi * 128, (bi + 1) * 128)
                dps = pshi.tile([128, H], f32, tag="big")
                nc.tensor.matmul(dps[:], st["g"][:, sl], st["u"][bi][:], start=True, stop=True)
                dl = dlpool.tile([128, H], f16, tag="dl")
                if bi < 2:
                    nc.scalar.copy(dl[:], dps[:])
                else:
                    nc.vector.tensor_copy(dl[:], dps[:])
                st["dl"].append(dl)

        def mupd(c, bis):
            # pair-end update: M^T += K_{c-1}^T Dl_{c-1} + K_c^T Dl_c
            # (bi 2,3 run at the start of the NEXT iteration to even PE load;
            #  some adds bounce PSUM->SBUF via DMA so Pool can do the add)
            st, stp = G[c], G[c - 1]
            for bi in bis:
                for jb in range(HB):
                    jsl = slice(jb * 128, (jb + 1) * 128)
                    mps = pshi.tile([128, H], f32, tag="big")
                    nc.tensor.matmul(mps[:], stp["k16"][bi][:, jsl], stp["dl"][bi][:],
                                     start=True, stop=False)
                    nc.tensor.matmul(mps[:], st["k16"][bi][:, jsl], st["dl"][bi][:],
                                     start=False, stop=True)
                    nc.vector.tensor_add(mt16[bi][jb][:], mps[:], mt16[bi][jb][:])

        def mupd_single(c):
            # startup chunks 0 and 1: per-chunk M update (keeps PE busy early)
            st = G[c]
            for bi in range(BPC):
                for jb in range(HB):
                    jsl = slice(jb * 128, (jb + 1) * 128)
                    mps = pshi.tile([128, H], f32, tag="big")
                    nc.tensor.matmul(mps[:], st["k16"][bi][:, jsl], st["dl"][bi][:],
                                     start=True, stop=True)
                    if c == 0:
                        nc.scalar.copy(mt16[bi][jb][:], mps[:])
                    else:
                        nc.vector.tensor_add(mt16[bi][jb][:], mps[:], mt16[bi][jb][:])

        # ---- software-pipelined main loop ----
        # Issue order per iteration is tuned so each engine's in-order queue
        # services consumers before producers-for-later: PE never waits behind
        # unready work, Act chain copies aren't stuck behind squares, and the
        # k16->kt16 DMA for c+2 isn't stuck behind pair-end M adds.
        prep_load(0)
        prep_r(0)
        prep_load(1)
        prep_r(1)
        aform(0)
        for lev in range(1, NLEV + 1):
            chain_level(0, lev)
        for c in range(NCHUNK):
            nxt = c + 1 if c + 1 < NCHUNK else None
            nn = c + 2 if c + 2 < NCHUNK else None
            if c % 2 == 1 and c >= 3:
                xform(c)
            if nxt is not None:
                aform(nxt)
            state_u(c)
            if nxt is not None:
                chain_sq(nxt, 1)
            state_delta(c)
            if nxt is not None:
                chain_gps(nxt, 1)
                chain_sq(nxt, 2)
            if nn is not None:
                prep_load(nn)
            if c <= 1:
                mupd_single(c)
            elif c % 2 == 1:
                mupd(c, [0, 1, 2, 3])
            if nxt is not None:
                chain_gps(nxt, 2)
            if nn is not None:
                prep_r(nn)
            prev = c - 3
            if prev in G:
                del G[prev]

        # ---- late prologue: read_proj weights + query (overlap with scan) ----
        # wtALL[:, ib*512 + op*128 + o] = W^T[ib*128 + i', op*128 + o]
        wtall = consts.tile([128, HB * H], f16, tag="wtall")
        for op in range(HB):
            wsb = wbuild.tile([128, H], f32, tag="wsb")
            nc.sync.dma_start(wsb[:], w_d[op * 128:(op + 1) * 128, :])
            w16 = wbuild.tile([128, H], f16, tag="w16")
            nc.gpsimd.tensor_copy(w16[:], wsb[:])
            nc.sync.dma_start_transpose(
                wtall[:].rearrange("p (f o2 q) -> p f o2 q", f=HB, q=128)[:, :, op, :],
                w16[:])
        bias_all = consts.tile([BPC, H], f32, tag="biasall")
        for bi in range(BPC):
            nc.sync.dma_start(bias_all[bi:bi + 1, :], b_d[None, :])
        qs = []
        for bi in range(BPC):
            v4 = wbuild.tile([HB, 128], f32, tag="v4")
            nc.sync.dma_start(v4[:], hidden_d[bi, L - 1, :].rearrange("(f p) -> f p", p=128))
            tps = pslo.tile([128, HB], f32, tag="sm")
            nc.tensor.transpose(tps[:], v4[:], ident_f[:HB, :HB])
            q_t = consts.tile([128, HB], f16, tag=f"q{bi}", name=f"q{bi}")
            nc.scalar.copy(q_t[:], tps[:])
            qs.append(q_t)

        # ---- finale: ctx = M q (row form); out = ctx W^T + b ----
        # phase-major over bi so PE/Act/DVE overlap across batch elements;
        # out-proj packs all 4 bi into one lhsT per ib block (4 matmuls total)
        cpss, ctx_rows = [], []
        for bi in range(BPC):
            cps = pshi.tile([1, H], f32, tag="big")
            for jb in range(HB):
                nc.tensor.matmul(cps[:], qs[bi][:, jb:jb + 1], mt16[bi][jb][:],
                                 start=(jb == 0), stop=(jb == HB - 1))
            cpss.append(cps)
        for bi in range(BPC):
            ctx_row = small.tile([1, H], f16, tag=f"ctxrow{bi}")
            nc.scalar.copy(ctx_row[:], cpss[bi][:])
            ctx_rows.append(ctx_row)
        ctxT = small.tile([128, HB * BPC], f16, tag="ctxT")
        for bi in range(BPC):
            for ib in range(HB):
                tp2 = pslo.tile([128, 1], f16, tag="sm1")
                nc.tensor.transpose(tp2[:], ctx_rows[bi][:, ib * 128:(ib + 1) * 128],
                                    ident_h[:1, :1])
                nc.vector.tensor_copy(ctxT[:, ib * BPC + bi:ib * BPC + bi + 1], tp2[:])
        ops4 = pshi.tile([BPC, H], f32, tag="big")
        for ib in range(HB):
            nc.tensor.matmul(ops4[:], ctxT[:, ib * BPC:(ib + 1) * BPC],
                             wtall[:, ib * H:(ib + 1) * H],
                             start=(ib == 0), stop=(ib == HB - 1))
        out_all = small.tile([BPC, H], f32, tag="outall")
        nc.vector.tensor_add(out_all[:], ops4[:], bias_all[:])
        nc.sync.dma_start(out_d[:, :], out_all[:])

    if legalize:
        _legalize_waits(nc)
    return nc


def _legalize_waits(nc, max_waits=1):
    """This toolchain's walrus encodes at most one semaphore wait per
    instruction. Hoist extra waits onto standalone EventSemaphore
    instructions on the same engine queue, immediately before the owner."""
    import json as _json
    m = _json.loads(bytes(nc.to_json_bytes()))
    n_fix = 0
    for fn in m["functions"]:
        for blk in fn["blocks"]:
            out = []
            for ins in blk.get("instructions", []):
                si = ins.get("sync_info") or {}
                waits = si.get("on_wait") or []
                if len(waits) > max_waits and ins.get("opcode") != "EventSemaphore":
                    extra, keep = waits[:-max_waits], waits[-max_waits:]
                    for i, w in enumerate(extra):
                        out.append({
                            "name": f"{ins['name']}-w{i}",
                            "engine": ins["engine"],
                            "opcode": "EventSemaphore",
                            "ins": [], "outs": [],
                            "sync_info": {"on_wait": [w], "on_update": []},
                        })
                    si["on_wait"] = keep
                    ins["sync_info"] = si
                    n_fix += 1
                out.append(ins)
            blk["instructions"] = out
    nc.m = mybir.module_from_json_bytes(_json.dumps(m).encode())
    return n_fix


def kernel(hidden: np.ndarray, W: np.ndarray, b: np.ndarray) -> np.ndarray:
    if "nc" not in _cached:
        _cached["nc"] = _build_program()
    nc = _cached["nc"]

    hidden = np.ascontiguousarray(hidden, dtype=np.float32)
    W = np.ascontiguousarray(W, dtype=np.float32)
    b = np.ascontiguousarray(b, dtype=np.float32)

    in_maps = []
    for ci in range(NCORES):
        in_maps.append({
            "hidden": hidden[ci * BPC:(ci + 1) * BPC],
            "W": W,
            "bvec": b,
        })
    res = run_bass_kernel_spmd(nc, in_maps, core_ids=list(range(NCORES)))
    _cached["last_results"] = res
    out = np.concatenate([res.results[ci]["out"] for ci in range(NCORES)], axis=0)
    return out.astype(np.float32)


if __name__ == "__main__":
    rng = np.random.default_rng(0)
    h = rng.standard_normal((B, L, H), dtype=np.float32)
    w = rng.standard_normal((H, H), dtype=np.float32) * (1.0 / np.sqrt(H))
    bb = np.zeros((H,), np.float32)
    o = kernel(h, w, bb)
    print(o.shape, o.dtype)


# revision 6
# speedup vs baseline: 1.3569x; 1.3569x over previous
"""DeltaRule memory scan kernel for Trainium2, 8 NeuronCores, data-parallel over batch.

Reference semantics (per batch element, H=512, L=2048):
    M_0 = 0  [H,H]
    for t in 0..L-2:   k = hidden[t]
        d = k.k + eps; delta = k - (M k)/d; M += outer(delta, k)
    out = (M @ hidden[L-1]) @ W.T + b

Implementation: chunked delta rule (UT transform), chunk C=128, float16.
Per chunk with keys K [C,H], r = 1/(rowsum(K^2)+eps):
    A  = strict_tril(diag(r) K K^T)            [C,C]
    T  = (I+A)^{-1} ~= (I-A)(I+A^2)(I+A^4)    (A nilpotent; exact through A^7)
    U  = K - diag(r) (K M^T + X Dl_prev)       [C,H]  (X-term: intra-pair cross)
    Dl = T U                                    [C,H]
    M^T += K_c0^T Dl_c0 + K_c1^T Dl_c1          (pair-accumulated in PSUM)
All matmuls f16 (1 cy/row); M^T state kept in f16 only (validated: relerr ~1.5e-3).
K^T and A^T produced by XBAR DMA transposes (no PE transposes in steady state).
Elementwise work spread across DVE / Act / Pool engines; 4 batch elements per
core with chunk phases interleaved for PE-queue continuity.
"""
import sys
import numpy as np
from contextlib import ExitStack

sys.path.insert(0, "/opt/trn_rl_repo")

import concourse.bass as bass
import concourse.mybir as mybir
from concourse import tile
from concourse.bass_utils import run_bass_kernel_spmd
from concourse.masks import make_identity, make_lower_triangular

B, L, H = 32, 2048, 512
NCORES = 8
BPC = B // NCORES          # 4 batch elements per core
C = 128                    # chunk size
T = L - 1                  # 2047 scan steps
NCHUNK = (T + C - 1) // C  # 16 (last chunk has 127 valid rows)
NLEV = 2                   # Neumann levels -> exact through A^7 (validated)
EPS = 1e-6
HB = H // 128              # 4 h-blocks

f32 = mybir.dt.float32
f16 = mybir.dt.float16

_cached = {}

# engine split for the 16 pair-end M updates (V=vector/DVE, P=pool)
_MUPD_ENG = "PVPPVPVPPVPPVPVP"


def _build_program(legalize=True):
    nc = bass.Bass(target_bir_lowering=False, debug=False)

    hidden_d = nc.declare_dram_parameter("hidden", [BPC, L, H], f32, isOutput=False)
    w_d = nc.declare_dram_parameter("W", [H, H], f32, isOutput=False)
    b_d = nc.declare_dram_parameter("bvec", [H], f32, isOutput=False)
    out_d = nc.declare_dram_parameter("out", [BPC, H], f32, isOutput=True)

    with tile.TileContext(nc) as tc, ExitStack() as ctx:
        consts = ctx.enter_context(tc.tile_pool(name="consts", bufs=1))
        wbuild = ctx.enter_context(tc.tile_pool(name="wbuild", bufs=2))
        mtpool = ctx.enter_context(tc.tile_pool(name="mt", bufs=1))
        kpool = ctx.enter_context(tc.tile_pool(name="k", bufs=12))
        k16pool = ctx.enter_context(tc.tile_pool(name="k16", bufs=4))
        chain = ctx.enter_context(tc.tile_pool(name="chain", bufs=6))
        upool = ctx.enter_context(tc.tile_pool(name="u", bufs=6))
        dlpool = ctx.enter_context(tc.tile_pool(name="dl", bufs=10))
        xpool = ctx.enter_context(tc.tile_pool(name="x", bufs=2))
        small = ctx.enter_context(tc.tile_pool(name="small", bufs=4))
        pslo = ctx.enter_context(tc.tile_pool(name="pslo", bufs=1, space="PSUM"))
        pshi = ctx.enter_context(tc.tile_pool(name="pshi", bufs=6, space="PSUM"))

        # ---- constants needed by the main loop ----
        ident_f = consts.tile([128, 128], f32, tag="identf")
        make_identity(nc, ident_f[:])
        ident_h = consts.tile([128, 128], f16, tag="identh")
        make_identity(nc, ident_h[:])
        identp = consts.tile([128, H], f16, tag="identp")
        for bi in range(BPC):
            nc.vector.tensor_copy(identp[:, bi * 128:(bi + 1) * 128], ident_h[:])
        smask = consts.tile([128, 128], f32, tag="smask")
        make_lower_triangular(nc, smask[:], val=1.0, diag=False)

        # persistent state M^T (f16 only), per (bi, jb)
        mt16 = [[mtpool.tile([128, H], f16, tag=f"mt{bi}_{jb}", name=f"mt{bi}_{jb}")
                 for jb in range(HB)] for bi in range(BPC)]

        G = {}

        def prep_load(c):
            t0 = c * C
            nrows = min(C, T - t0)
            st = {"k": []}
            for bi in range(BPC):
                k_t = kpool.tile([128, H], f32, tag="K", name=f"k{c}_{bi}")
                if nrows < C:
                    nc.vector.memset(k_t[:], 0.0)
                    nc.sync.dma_start(k_t[:nrows, :], hidden_d[bi, t0:t0 + nrows, :])
                else:
                    nc.sync.dma_start(k_t[:], hidden_d[bi, t0:t0 + C, :])
                st["k"].append(k_t)
            # packed row-major f16 keys [128, (bi h)] (Pool: SBUF-only casts)
            k16a = k16pool.tile([128, BPC * H], f16, tag="k16", name=f"k16_{c}")
            for bi in range(BPC):
                nc.gpsimd.tensor_copy(k16a[:, bi * H:(bi + 1) * H], st["k"][bi][:])
            st["k16"] = k16a
            # K^T blocks via PE transposes of the f32 keys (XBAR DMA on the
            # single SP queue head-of-line-blocks the whole pipeline on hw)
            kt16a = k16pool.tile([128, BPC * H], f16, tag="kt16", name=f"kt16_{c}")
            for bi in range(BPC):
                ktps = pshi.tile([128, H], f32, tag="big")
                for hb in range(HB):
                    nc.tensor.transpose(ktps[:, hb * 128:(hb + 1) * 128],
                                        st["k"][bi][:, hb * 128:(hb + 1) * 128],
                                        ident_f[:])
                nc.scalar.copy(kt16a[:, bi * H:(bi + 1) * H], ktps[:])
            st["kt16"] = kt16a
            G[c] = st

        def prep_r(c):
            st = G[c]
            dall = small.tile([128, BPC], f32, tag="dall")
            for bi in range(BPC):
                scr = small.tile([128, H], f16, tag="scr")
                nc.scalar.activation(scr[:], st["k"][bi][:],
                                     mybir.ActivationFunctionType.Square,
                                     accum_out=dall[:, bi:bi + 1])
            rall = small.tile([128, BPC], f32, tag="rall")
            if c == NCHUNK - 1:
                # only the zero-padded final chunk needs the EPS guard
                nc.vector.tensor_scalar_add(dall[:], dall[:], EPS)
            nc.vector.reciprocal(rall[:], dall[:])
            st["r"] = rall

        def aform(c):
            st = G[c]
            kt = st["kt16"]
            a_ps = pshi.tile([128, H], f32, tag="big")
            for bi in range(BPC):
                sl = slice(bi * 128, (bi + 1) * 128)
                for hb in range(HB):
                    hsl = slice(bi * H + hb * 128, bi * H + (hb + 1) * 128)
                    nc.tensor.matmul(a_ps[:, sl], kt[:, hsl], kt[:, hsl],
                                     start=(hb == 0), stop=(hb == HB - 1))
            a_all = chain.tile([128, H], f16, tag="ak")
            for bi in range(BPC):
                sl = slice(bi * 128, (bi + 1) * 128)
                nc.vector.scalar_tensor_tensor(a_all[:, sl], a_ps[:, sl],
                                               st["r"][:, bi:bi + 1], smask[:],
                                               mybir.AluOpType.mult,
                                               mybir.AluOpType.mult)
            at_ps = pshi.tile([128, H], f16, tag="big")
            for bi in range(BPC):
                sl = slice(bi * 128, (bi + 1) * 128)
                nc.tensor.transpose(at_ps[:, sl], a_all[:, sl], ident_h[:])
            at_all = chain.tile([128, H], f16, tag="atk")
            nc.scalar.copy(at_all[:], at_ps[:])
            g0 = chain.tile([128, H], f16, tag="g")
            nc.vector.tensor_sub(g0[:], identp[:], at_all[:])
            st["ak"], st["atk"], st["g"] = a_all, at_all, g0

        def chain_sq(c, lev):
            # squaring half: A^(2^lev); its transpose via XBAR DMA (free-ish)
            st = G[c]
            ak, atk = st["ak"], st["atk"]
            sq1 = pshi.tile([128, H], f32, tag="big")
            for bi in range(BPC):
                sl = slice(bi * 128, (bi + 1) * 128)
                nc.tensor.matmul(sq1[:, sl], atk[:, sl], ak[:, sl], start=True, stop=True)
            ak2 = chain.tile([128, H], f16, tag="ak")
            nc.scalar.copy(ak2[:], sq1[:])
            if lev < NLEV:
                sq2 = pshi.tile([128, H], f32, tag="big")
                for bi in range(BPC):
                    sl = slice(bi * 128, (bi + 1) * 128)
                    nc.tensor.matmul(sq2[:, sl], ak[:, sl], atk[:, sl], start=True, stop=True)
                atk2 = chain.tile([128, H], f16, tag="atk")
                nc.scalar.copy(atk2[:], sq2[:])
            else:
                atk2 = None
            st["ak_n"], st["atk_n"] = ak2, atk2

        def chain_gps(c, lev):
            # accumulation half: g += (A^(2^lev))^T g
            st = G[c]
            ak2 = st["ak_n"]
            gps = pshi.tile([128, H], f32, tag="big")
            for bi in range(BPC):
                sl = slice(bi * 128, (bi + 1) * 128)
                nc.tensor.matmul(gps[:, sl], ak2[:, sl], st["g"][:, sl], start=True, stop=True)
            g_nxt = chain.tile([128, H], f16, tag="g")
            nc.vector.tensor_add(g_nxt[:], gps[:], st["g"][:])
            st["ak"], st["atk"], st["g"] = st["ak_n"], st["atk_n"], g_nxt

        def chain_level(c, lev):
            chain_sq(c, lev)
            chain_gps(c, lev)

        def xform(c):
            # X^T for pair (c-1, c): xts[:, bi-slice][a, b] = K_{c-1}[a]·K_c[b]
            st, stp = G[c], G[c - 1]
            xps = pshi.tile([128, H], f32, tag="big")
            for bi in range(BPC):
                sl = slice(bi * 128, (bi + 1) * 128)
                for hb in range(HB):
                    hsl = slice(bi * H + hb * 128, bi * H + (hb + 1) * 128)
                    nc.tensor.matmul(xps[:, sl], stp["kt16"][:, hsl],
                                     st["kt16"][:, hsl],
                                     start=(hb == 0), stop=(hb == HB - 1))
            # negated so the (negated-dl) cross product lands with + sign
            xts = xpool.tile([128, H], f16, tag="x")
            nc.scalar.activation(xts[:], xps[:], mybir.ActivationFunctionType.Copy,
                                 scale=-1.0)
            st["x"] = xts

        def state_u(c):
            # chunks 0/1 update M individually (startup); pairs start at (2,3)
            st = G[c]
            cross = (c % 2 == 1 and c >= 3)
            st["u"] = []
            for bi in range(BPC):
                if c == 0:
                    st["u"].append(st["k16"][:, bi * H:(bi + 1) * H])
                    continue
                ups = pshi.tile([128, H], f32, tag="big")
                for hb in range(HB):
                    hsl = slice(bi * H + hb * 128, bi * H + (hb + 1) * 128)
                    nc.tensor.matmul(ups[:], st["kt16"][:, hsl],
                                     mt16[bi][hb][:],
                                     start=(hb == 0),
                                     stop=(hb == HB - 1 and not cross))
                if cross:
                    sl = slice(bi * 128, (bi + 1) * 128)
                    nc.tensor.matmul(ups[:], st["x"][:, sl], G[c - 1]["dl"][bi][:],
                                     start=False, stop=True)
                # u_neg = r*ups - k (negated U; saves materializing -r)
                u_sb = upool.tile([128, H], f16, tag="u")
                nc.vector.scalar_tensor_tensor(u_sb[:], ups[:], st["r"][:, bi:bi + 1],
                                               st["k"][bi][:], mybir.AluOpType.mult,
                                               mybir.AluOpType.subtract)
                st["u"].append(u_sb[:])

        def state_delta(c):
            st = G[c]
            st["dl"] = []
            for bi in range(BPC):
                sl = slice(bi * 128, (bi + 1) * 128)
                dps = pshi.tile([128, H], f32, tag="big")
                nc.tensor.matmul(dps[:], st["g"][:, sl], st["u"][bi], start=True, stop=True)
                dl = dlpool.tile([128, H], f16, tag="dl")
                nc.scalar.copy(dl[:], dps[:])
                st["dl"].append(dl)

        def mupd(c, bis):
            # pair-end update: M^T += K_{c-1}^T Dl_{c-1} + K_c^T Dl_c
            # (bi 2,3 run at the start of the NEXT iteration to even PE load;
            #  some adds bounce PSUM->SBUF via DMA so Pool can do the add)
            st, stp = G[c], G[c - 1]
            for bi in bis:
                for jb in range(HB):
                    jsl = slice(bi * H + jb * 128, bi * H + (jb + 1) * 128)
                    mps = pshi.tile([128, H], f32, tag="big")
                    nc.tensor.matmul(mps[:], stp["k16"][:, jsl], stp["dl"][bi][:],
                                     start=True, stop=False)
                    nc.tensor.matmul(mps[:], st["k16"][:, jsl], st["dl"][bi][:],
                                     start=False, stop=True)
                    nc.vector.tensor_sub(mt16[bi][jb][:], mt16[bi][jb][:], mps[:])

        def mupd_single(c):
            # startup chunks 0 and 1: per-chunk M update (keeps PE busy early)
            st = G[c]
            for bi in range(BPC):
                for jb in range(HB):
                    jsl = slice(bi * H + jb * 128, bi * H + (jb + 1) * 128)
                    mps = pshi.tile([128, H], f32, tag="big")
                    nc.tensor.matmul(mps[:], st["k16"][:, jsl], st["dl"][bi][:],
                                     start=True, stop=True)
                    if c == 0:
                        nc.scalar.copy(mt16[bi][jb][:], mps[:])
                    else:
                        nc.vector.tensor_sub(mt16[bi][jb][:], mt16[bi][jb][:], mps[:])

        # ---- software-pipelined main loop ----
        # Issue order per iteration is tuned so each engine's in-order queue
        # services consumers before producers-for-later: PE never waits behind
        # unready work, Act chain copies aren't stuck behind squares, and the
        # k16->kt16 DMA for c+2 isn't stuck behind pair-end M adds.
        prep_load(0)
        prep_r(0)
        prep_load(1)
        prep_r(1)
        aform(0)
        for lev in range(1, NLEV + 1):
            chain_level(0, lev)
        for c in range(NCHUNK):
            nxt = c + 1 if c + 1 < NCHUNK else None
            nn = c + 2 if c + 2 < NCHUNK else None
            if c % 2 == 1 and c >= 3:
                xform(c)
            if nxt is not None:
                aform(nxt)
            state_u(c)
            if nxt is not None:
                chain_sq(nxt, 1)
            state_delta(c)
            if nxt is not None:
                chain_gps(nxt, 1)
                chain_sq(nxt, 2)
            if nn is not None:
                prep_load(nn)
            if c <= 1:
                mupd_single(c)
            elif c % 2 == 1:
                mupd(c, [0, 1, 2, 3])
            if nxt is not None:
                chain_gps(nxt, 2)
            if nn is not None:
                prep_r(nn)
            prev = c - 3
            if prev in G:
                del G[prev]

        # ---- late prologue: read_proj weights + query (overlap with scan) ----
        # wtALL[:, ib*512 + op*128 + o] = W^T[ib*128 + i', op*128 + o]
        wtall = consts.tile([128, HB * H], f16, tag="wtall")
        for op in range(HB):
            wsb = wbuild.tile([128, H], f32, tag="wsb")
            nc.sync.dma_start(wsb[:], w_d[op * 128:(op + 1) * 128, :])
            w16 = wbuild.tile([128, H], f16, tag="w16")
            nc.gpsimd.tensor_copy(w16[:], wsb[:])
            nc.sync.dma_start_transpose(
                wtall[:].rearrange("p (f o2 q) -> p f o2 q", f=HB, q=128)[:, :, op, :],
                w16[:])
        bias_all = consts.tile([BPC, H], f32, tag="biasall")
        for bi in range(BPC):
            nc.sync.dma_start(bias_all[bi:bi + 1, :], b_d[None, :])
        qs = []
        for bi in range(BPC):
            v4 = wbuild.tile([HB, 128], f32, tag="v4")
            nc.sync.dma_start(v4[:], hidden_d[bi, L - 1, :].rearrange("(f p) -> f p", p=128))
            tps = pslo.tile([128, HB], f32, tag="sm")
            nc.tensor.transpose(tps[:], v4[:], ident_f[:HB, :HB])
            q_t = consts.tile([128, HB], f16, tag=f"q{bi}", name=f"q{bi}")
            nc.scalar.copy(q_t[:], tps[:])
            qs.append(q_t)

        # ---- finale: ctx = M q (row form); out = ctx W^T + b ----
        # phase-major over bi so PE/Act/DVE overlap across batch elements;
        # out-proj packs all 4 bi into one lhsT per ib block (4 matmuls total)
        cpss, ctx_rows = [], []
        for bi in range(BPC):
            cps = pshi.tile([1, H], f32, tag="big")
            for jb in range(HB):
                nc.tensor.matmul(cps[:], qs[bi][:, jb:jb + 1], mt16[bi][jb][:],
                                 start=(jb == 0), stop=(jb == HB - 1))
            cpss.append(cps)
        for bi in range(BPC):
            ctx_row = small.tile([1, H], f16, tag=f"ctxrow{bi}")
            nc.scalar.copy(ctx_row[:], cpss[bi][:])
            ctx_rows.append(ctx_row)
        ctxT = small.tile([128, HB * BPC], f16, tag="ctxT")
        for bi in range(BPC):
            for ib in range(HB):
                tp2 = pslo.tile([128, 1], f16, tag="sm1")
                nc.tensor.transpose(tp2[:], ctx_rows[bi][:, ib * 128:(ib + 1) * 128],
                                    ident_h[:1, :1])
                nc.vector.tensor_copy(ctxT[:, ib * BPC + bi:ib * BPC + bi + 1], tp2[:])
        ops4 = pshi.tile([BPC, H], f32, tag="big")
        for ib in range(HB):
            nc.tensor.matmul(ops4[:], ctxT[:, ib * BPC:(ib + 1) * BPC],
                             wtall[:, ib * H:(ib + 1) * H],
                             start=(ib == 0), stop=(ib == HB - 1))
        out_all = small.tile([BPC, H], f32, tag="outall")
        nc.vector.tensor_add(out_all[:], ops4[:], bias_all[:])
        nc.sync.dma_start(out_d[:, :], out_all[:])

    if legalize:
        _legalize_waits(nc)
    return nc


def _legalize_waits(nc, max_waits=1):
    """This toolchain's walrus encodes at most one semaphore wait per
    instruction. Hoist extra waits onto standalone EventSemaphore
    instructions on the same engine queue, immediately before the owner."""
    import json as _json
    m = _json.loads(bytes(nc.to_json_bytes()))
    n_fix = 0
    for fn in m["functions"]:
        for blk in fn["blocks"]:
            out = []
            for ins in blk.get("instructions", []):
                si = ins.get("sync_info") or {}
                waits = si.get("on_wait") or []
                if len(waits) > max_waits and ins.get("opcode") != "EventSemaphore":
                    extra, keep = waits[:-max_waits], waits[-max_waits:]
                    for i, w in enumerate(extra):
                        out.append({
                            "name": f"{ins['name']}-w{i}",
                            "engine": ins["engine"],
                            "opcode": "EventSemaphore",
                            "ins": [], "outs": [],
                            "sync_info": {"on_wait": [w], "on_update": []},
                        })
                    si["on_wait"] = keep
                    ins["sync_info"] = si
                    n_fix += 1
                out.append(ins)
            blk["instructions"] = out
    nc.m = mybir.module_from_json_bytes(_json.dumps(m).encode())
    return n_fix


def kernel(hidden: np.ndarray, W: np.ndarray, b: np.ndarray) -> np.ndarray:
    if "nc" not in _cached:
        _cached["nc"] = _build_program()
    nc = _cached["nc"]

    hidden = np.ascontiguousarray(hidden, dtype=np.float32)
    W = np.ascontiguousarray(W, dtype=np.float32)
    b = np.ascontiguousarray(b, dtype=np.float32)

    in_maps = []
    for ci in range(NCORES):
        in_maps.append({
            "hidden": hidden[ci * BPC:(ci + 1) * BPC],
            "W": W,
            "bvec": b,
        })
    res = run_bass_kernel_spmd(nc, in_maps, core_ids=list(range(NCORES)))
    _cached["last_results"] = res
    out = np.concatenate([res.results[ci]["out"] for ci in range(NCORES)], axis=0)
    return out.astype(np.float32)


if __name__ == "__main__":
    rng = np.random.default_rng(0)
    h = rng.standard_normal((B, L, H), dtype=np.float32)
    w = rng.standard_normal((H, H), dtype=np.float32) * (1.0 / np.sqrt(H))
    bb = np.zeros((H,), np.float32)
    o = kernel(h, w, bb)
    print(o.shape, o.dtype)


# revision 7
# speedup vs baseline: 1.4181x; 1.0451x over previous
"""DeltaRule memory scan kernel for Trainium2, 8 NeuronCores, data-parallel over batch.

Reference semantics (per batch element, H=512, L=2048):
    M_0 = 0  [H,H]
    for t in 0..L-2:   k = hidden[t]
        d = k.k + eps; delta = k - (M k)/d; M += outer(delta, k)
    out = (M @ hidden[L-1]) @ W.T + b

Implementation: chunked delta rule (UT transform), chunk C=128, float16.
Per chunk with keys K [C,H], r = 1/(rowsum(K^2)+eps):
    A  = strict_tril(diag(r) K K^T)            [C,C]
    T  = (I+A)^{-1} ~= (I-A)(I+A^2)(I+A^4)    (A nilpotent; exact through A^7)
    U  = K - diag(r) (K M^T + X Dl_prev)       [C,H]  (X-term: intra-pair cross)
    Dl = T U                                    [C,H]
    M^T += K_c0^T Dl_c0 + K_c1^T Dl_c1          (pair-accumulated in PSUM)
All matmuls f16 (1 cy/row); M^T state kept in f16 only (validated: relerr ~1.5e-3).
K^T and A^T produced by XBAR DMA transposes (no PE transposes in steady state).
Elementwise work spread across DVE / Act / Pool engines; 4 batch elements per
core with chunk phases interleaved for PE-queue continuity.
"""
import sys
import numpy as np
from contextlib import ExitStack

sys.path.insert(0, "/opt/trn_rl_repo")

import concourse.bass as bass
import concourse.mybir as mybir
from concourse import tile
from concourse.bass_utils import run_bass_kernel_spmd
from concourse.masks import make_identity, make_lower_triangular

B, L, H = 32, 2048, 512
NCORES = 8
BPC = B // NCORES          # 4 batch elements per core
C = 128                    # chunk size
T = L - 1                  # 2047 scan steps
NCHUNK = (T + C - 1) // C  # 16 (last chunk has 127 valid rows)
NLEV = 2                   # Neumann levels -> exact through A^7 (validated)
EPS = 1e-6
HB = H // 128              # 4 h-blocks

f32 = mybir.dt.float32
f16 = mybir.dt.float16

_cached = {}

# engine split for the 16 pair-end M updates (V=vector/DVE, P=pool)
_MUPD_ENG = "PVPPVPVPPVPPVPVP"


def _build_program(legalize=True):
    nc = bass.Bass(target_bir_lowering=False, debug=False)

    hidden_d = nc.declare_dram_parameter("hidden", [BPC, L, H], f32, isOutput=False)
    w_d = nc.declare_dram_parameter("W", [H, H], f32, isOutput=False)
    b_d = nc.declare_dram_parameter("bvec", [H], f32, isOutput=False)
    out_d = nc.declare_dram_parameter("out", [BPC, H], f32, isOutput=True)

    with tile.TileContext(nc) as tc, ExitStack() as ctx:
        consts = ctx.enter_context(tc.tile_pool(name="consts", bufs=1))
        wbuild = ctx.enter_context(tc.tile_pool(name="wbuild", bufs=2))
        mtpool = ctx.enter_context(tc.tile_pool(name="mt", bufs=1))
        kpool = ctx.enter_context(tc.tile_pool(name="k", bufs=12))
        k16pool = ctx.enter_context(tc.tile_pool(name="k16", bufs=4))
        chain = ctx.enter_context(tc.tile_pool(name="chain", bufs=6))
        upool = ctx.enter_context(tc.tile_pool(name="u", bufs=6))
        dlpool = ctx.enter_context(tc.tile_pool(name="dl", bufs=10))
        xpool = ctx.enter_context(tc.tile_pool(name="x", bufs=2))
        small = ctx.enter_context(tc.tile_pool(name="small", bufs=4))
        pslo = ctx.enter_context(tc.tile_pool(name="pslo", bufs=1, space="PSUM"))
        pshi = ctx.enter_context(tc.tile_pool(name="pshi", bufs=6, space="PSUM"))

        # ---- constants needed by the main loop ----
        ident_f = consts.tile([128, 128], f32, tag="identf")
        make_identity(nc, ident_f[:])
        ident_h = consts.tile([128, 128], f16, tag="identh")
        make_identity(nc, ident_h[:])
        identp = consts.tile([128, H], f16, tag="identp")
        for bi in range(BPC):
            nc.vector.tensor_copy(identp[:, bi * 128:(bi + 1) * 128], ident_h[:])
        smask = consts.tile([128, 128], f32, tag="smask")
        make_lower_triangular(nc, smask[:], val=1.0, diag=False)

        # persistent state M^T (f16 only), per (bi, jb)
        mt16 = [[mtpool.tile([128, H], f16, tag=f"mt{bi}_{jb}", name=f"mt{bi}_{jb}")
                 for jb in range(HB)] for bi in range(BPC)]

        # pre-zeroed key tiles for the final (127-row) chunk: zeroing them in
        # the prologue keeps the memset out of the busy mid-scan DVE queue
        k15 = [consts.tile([128, H], f32, tag=f"k15_{bi}", name=f"k15_{bi}")
               for bi in range(BPC)]
        for bi in range(BPC):
            nc.vector.memset(k15[bi][:], 0.0)

        G = {}

        def prep_load(c):
            t0 = c * C
            nrows = min(C, T - t0)
            st = {"k": []}
            for bi in range(BPC):
                if nrows < C:
                    k_t = k15[bi]
                    nc.sync.dma_start(k_t[:nrows, :], hidden_d[bi, t0:t0 + nrows, :])
                else:
                    k_t = kpool.tile([128, H], f32, tag="K", name=f"k{c}_{bi}")
                    nc.sync.dma_start(k_t[:], hidden_d[bi, t0:t0 + C, :])
                st["k"].append(k_t)
            # packed row-major f16 keys [128, (bi h)] (Pool: SBUF-only casts)
            k16a = k16pool.tile([128, BPC * H], f16, tag="k16", name=f"k16_{c}")
            for bi in range(BPC):
                nc.gpsimd.tensor_copy(k16a[:, bi * H:(bi + 1) * H], st["k"][bi][:])
            st["k16"] = k16a
            G[c] = st

        def prep_tr(c):
            # K^T blocks via PE transposes of the f32 keys (XBAR DMA on the
            # single SP queue head-of-line-blocks the whole pipeline on hw)
            st = G[c]
            kt16a = k16pool.tile([128, BPC * H], f16, tag="kt16", name=f"kt16_{c}")
            for bi in range(BPC):
                ktps = pshi.tile([128, H], f32, tag="big")
                for hb in range(HB):
                    nc.tensor.transpose(ktps[:, hb * 128:(hb + 1) * 128],
                                        st["k"][bi][:, hb * 128:(hb + 1) * 128],
                                        ident_f[:])
                nc.scalar.copy(kt16a[:, bi * H:(bi + 1) * H], ktps[:])
            st["kt16"] = kt16a

        def prep_r(c):
            st = G[c]
            dall = small.tile([128, BPC], f32, tag="dall")
            for bi in range(BPC):
                scr = small.tile([128, H], f16, tag="scr")
                nc.scalar.activation(scr[:], st["k"][bi][:],
                                     mybir.ActivationFunctionType.Square,
                                     accum_out=dall[:, bi:bi + 1])
            rall = small.tile([128, BPC], f32, tag="rall")
            if c == NCHUNK - 1:
                # only the zero-padded final chunk needs the EPS guard
                nc.vector.tensor_scalar_add(dall[:], dall[:], EPS)
            nc.vector.reciprocal(rall[:], dall[:])
            st["r"] = rall

        def aform(c):
            st = G[c]
            kt = st["kt16"]
            a_ps = pshi.tile([128, H], f32, tag="big")
            for bi in range(BPC):
                sl = slice(bi * 128, (bi + 1) * 128)
                for hb in range(HB):
                    hsl = slice(bi * H + hb * 128, bi * H + (hb + 1) * 128)
                    nc.tensor.matmul(a_ps[:, sl], kt[:, hsl], kt[:, hsl],
                                     start=(hb == 0), stop=(hb == HB - 1))
            a_all = chain.tile([128, H], f16, tag="ak")
            for bi in range(BPC):
                sl = slice(bi * 128, (bi + 1) * 128)
                nc.vector.scalar_tensor_tensor(a_all[:, sl], a_ps[:, sl],
                                               st["r"][:, bi:bi + 1], smask[:],
                                               mybir.AluOpType.mult,
                                               mybir.AluOpType.mult)
            at_ps = pshi.tile([128, H], f16, tag="big")
            for bi in range(BPC):
                sl = slice(bi * 128, (bi + 1) * 128)
                nc.tensor.transpose(at_ps[:, sl], a_all[:, sl], ident_h[:])
            at_all = chain.tile([128, H], f16, tag="atk")
            nc.scalar.copy(at_all[:], at_ps[:])
            g0 = chain.tile([128, H], f16, tag="g")
            nc.vector.tensor_sub(g0[:], identp[:], at_all[:])
            st["ak"], st["atk"], st["g"] = a_all, at_all, g0

        def chain_sq(c, lev):
            # squaring half: A^(2^lev); its transpose via XBAR DMA (free-ish)
            st = G[c]
            ak, atk = st["ak"], st["atk"]
            sq1 = pshi.tile([128, H], f32, tag="big")
            for bi in range(BPC):
                sl = slice(bi * 128, (bi + 1) * 128)
                nc.tensor.matmul(sq1[:, sl], atk[:, sl], ak[:, sl], start=True, stop=True)
            ak2 = chain.tile([128, H], f16, tag="ak")
            nc.scalar.copy(ak2[:], sq1[:])
            if lev < NLEV:
                sq2 = pshi.tile([128, H], f32, tag="big")
                for bi in range(BPC):
                    sl = slice(bi * 128, (bi + 1) * 128)
                    nc.tensor.matmul(sq2[:, sl], ak[:, sl], atk[:, sl], start=True, stop=True)
                atk2 = chain.tile([128, H], f16, tag="atk")
                nc.scalar.copy(atk2[:], sq2[:])
            else:
                atk2 = None
            st["ak_n"], st["atk_n"] = ak2, atk2

        def chain_gps(c, lev):
            # accumulation half: g += (A^(2^lev))^T g
            st = G[c]
            ak2 = st["ak_n"]
            gps = pshi.tile([128, H], f32, tag="big")
            for bi in range(BPC):
                sl = slice(bi * 128, (bi + 1) * 128)
                nc.tensor.matmul(gps[:, sl], ak2[:, sl], st["g"][:, sl], start=True, stop=True)
            g_nxt = chain.tile([128, H], f16, tag="g")
            nc.vector.tensor_add(g_nxt[:], gps[:], st["g"][:])
            st["ak"], st["atk"], st["g"] = st["ak_n"], st["atk_n"], g_nxt

        def chain_level(c, lev):
            chain_sq(c, lev)
            chain_gps(c, lev)

        def xform(c):
            # X^T for pair (c-1, c): xts[:, bi-slice][a, b] = K_{c-1}[a]·K_c[b]
            st, stp = G[c], G[c - 1]
            xps = pshi.tile([128, H], f32, tag="big")
            for bi in range(BPC):
                sl = slice(bi * 128, (bi + 1) * 128)
                for hb in range(HB):
                    hsl = slice(bi * H + hb * 128, bi * H + (hb + 1) * 128)
                    nc.tensor.matmul(xps[:, sl], stp["kt16"][:, hsl],
                                     st["kt16"][:, hsl],
                                     start=(hb == 0), stop=(hb == HB - 1))
            # negated so the (negated-dl) cross product lands with + sign
            xts = xpool.tile([128, H], f16, tag="x")
            nc.scalar.activation(xts[:], xps[:], mybir.ActivationFunctionType.Copy,
                                 scale=-1.0)
            st["x"] = xts

        def state_u(c):
            # chunks 0/1 update M individually (startup); pairs start at (2,3)
            st = G[c]
            cross = (c % 2 == 1 and c >= 3)
            st["u"] = []
            for bi in range(BPC):
                if c == 0:
                    st["u"].append(st["k16"][:, bi * H:(bi + 1) * H])
                    continue
                ups = pshi.tile([128, H], f32, tag="big")
                for hb in range(HB):
                    hsl = slice(bi * H + hb * 128, bi * H + (hb + 1) * 128)
                    nc.tensor.matmul(ups[:], st["kt16"][:, hsl],
                                     mt16[bi][hb][:],
                                     start=(hb == 0),
                                     stop=(hb == HB - 1 and not cross))
                if cross:
                    sl = slice(bi * 128, (bi + 1) * 128)
                    nc.tensor.matmul(ups[:], st["x"][:, sl], G[c - 1]["dl"][bi][:],
                                     start=False, stop=True)
                # u_neg = r*ups - k (negated U; saves materializing -r)
                u_sb = upool.tile([128, H], f16, tag="u")
                nc.vector.scalar_tensor_tensor(u_sb[:], ups[:], st["r"][:, bi:bi + 1],
                                               st["k"][bi][:], mybir.AluOpType.mult,
                                               mybir.AluOpType.subtract)
                st["u"].append(u_sb[:])

        def state_delta(c):
            st = G[c]
            st["dl"] = []
            for bi in range(BPC):
                sl = slice(bi * 128, (bi + 1) * 128)
                dps = pshi.tile([128, H], f32, tag="big")
                nc.tensor.matmul(dps[:], st["g"][:, sl], st["u"][bi], start=True, stop=True)
                dl = dlpool.tile([128, H], f16, tag="dl")
                nc.scalar.copy(dl[:], dps[:])
                st["dl"].append(dl)

        def mupd(c, bis):
            # pair-end update: M^T += K_{c-1}^T Dl_{c-1} + K_c^T Dl_c
            # (bi 2,3 run at the start of the NEXT iteration to even PE load;
            #  some adds bounce PSUM->SBUF via DMA so Pool can do the add)
            st, stp = G[c], G[c - 1]
            for bi in bis:
                for jb in range(HB):
                    jsl = slice(bi * H + jb * 128, bi * H + (jb + 1) * 128)
                    mps = pshi.tile([128, H], f32, tag="big")
                    nc.tensor.matmul(mps[:], stp["k16"][:, jsl], stp["dl"][bi][:],
                                     start=True, stop=False)
                    nc.tensor.matmul(mps[:], st["k16"][:, jsl], st["dl"][bi][:],
                                     start=False, stop=True)
                    nc.vector.tensor_sub(mt16[bi][jb][:], mt16[bi][jb][:], mps[:])

        def mupd_single(c):
            # startup chunks 0 and 1: per-chunk M update (keeps PE busy early)
            st = G[c]
            for bi in range(BPC):
                for jb in range(HB):
                    jsl = slice(bi * H + jb * 128, bi * H + (jb + 1) * 128)
                    mps = pshi.tile([128, H], f32, tag="big")
                    nc.tensor.matmul(mps[:], st["k16"][:, jsl], st["dl"][bi][:],
                                     start=True, stop=True)
                    if c == 0:
                        nc.scalar.copy(mt16[bi][jb][:], mps[:])
                    else:
                        nc.vector.tensor_sub(mt16[bi][jb][:], mt16[bi][jb][:], mps[:])

        # ---- software-pipelined main loop ----
        # Issue order per iteration is tuned so each engine's in-order queue
        # services consumers before producers-for-later: PE never waits behind
        # unready work, Act chain copies aren't stuck behind squares, and the
        # k16->kt16 DMA for c+2 isn't stuck behind pair-end M adds.
        prep_load(0)
        prep_tr(0)
        prep_r(0)
        prep_load(1)
        prep_tr(1)
        prep_r(1)
        aform(0)
        for lev in range(1, NLEV + 1):
            chain_level(0, lev)
        for c in range(NCHUNK):
            nxt = c + 1 if c + 1 < NCHUNK else None
            nn = c + 2 if c + 2 < NCHUNK else None
            if c % 2 == 1 and c >= 3:
                xform(c)
            if nxt is not None:
                aform(nxt)
            state_u(c)
            if nxt is not None:
                chain_sq(nxt, 1)
            state_delta(c)
            if nxt is not None:
                chain_gps(nxt, 1)
                chain_sq(nxt, 2)
            if nn is not None:
                prep_load(nn)
            if c <= 1:
                mupd_single(c)
            elif c % 2 == 1:
                mupd(c, [0, 1, 2, 3])
            if nxt is not None:
                chain_gps(nxt, 2)
            if nn is not None:
                prep_tr(nn)
                prep_r(nn)
            prev = c - 3
            if prev in G:
                del G[prev]

        # ---- late prologue: read_proj weights + query (overlap with scan) ----
        # wtALL[:, ib*512 + op*128 + o] = W^T[ib*128 + i', op*128 + o]
        wtall = consts.tile([128, HB * H], f16, tag="wtall")
        for op in range(HB):
            wsb = wbuild.tile([128, H], f32, tag="wsb")
            nc.sync.dma_start(wsb[:], w_d[op * 128:(op + 1) * 128, :])
            w16 = wbuild.tile([128, H], f16, tag="w16")
            nc.gpsimd.tensor_copy(w16[:], wsb[:])
            nc.sync.dma_start_transpose(
                wtall[:].rearrange("p (f o2 q) -> p f o2 q", f=HB, q=128)[:, :, op, :],
                w16[:])
        bias_all = consts.tile([BPC, H], f32, tag="biasall")
        for bi in range(BPC):
            nc.sync.dma_start(bias_all[bi:bi + 1, :], b_d[None, :])
        qs = []
        for bi in range(BPC):
            v4 = wbuild.tile([HB, 128], f32, tag="v4")
            nc.sync.dma_start(v4[:], hidden_d[bi, L - 1, :].rearrange("(f p) -> f p", p=128))
            tps = pslo.tile([128, HB], f32, tag="sm")
            nc.tensor.transpose(tps[:], v4[:], ident_f[:HB, :HB])
            q_t = consts.tile([128, HB], f16, tag=f"q{bi}", name=f"q{bi}")
            nc.scalar.copy(q_t[:], tps[:])
            qs.append(q_t)

        # ---- finale: ctx = M q (row form); out = ctx W^T + b ----
        # phase-major over bi so PE/Act/DVE overlap across batch elements;
        # out-proj packs all 4 bi into one lhsT per ib block (4 matmuls total)
        cpss, ctx_rows = [], []
        for bi in range(BPC):
            cps = pshi.tile([1, H], f32, tag="big")
            for jb in range(HB):
                nc.tensor.matmul(cps[:], qs[bi][:, jb:jb + 1], mt16[bi][jb][:],
                                 start=(jb == 0), stop=(jb == HB - 1))
            cpss.append(cps)
        for bi in range(BPC):
            ctx_row = small.tile([1, H], f16, tag=f"ctxrow{bi}")
            nc.scalar.copy(ctx_row[:], cpss[bi][:])
            ctx_rows.append(ctx_row)
        ctxT = small.tile([128, HB * BPC], f16, tag="ctxT")
        for bi in range(BPC):
            for ib in range(HB):
                tp2 = pslo.tile([128, 1], f16, tag="sm1")
                nc.tensor.transpose(tp2[:], ctx_rows[bi][:, ib * 128:(ib + 1) * 128],
                                    ident_h[:1, :1])
                nc.vector.tensor_copy(ctxT[:, ib * BPC + bi:ib * BPC + bi + 1], tp2[:])
        ops4 = pshi.tile([BPC, H], f32, tag="big")
        for ib in range(HB):
            nc.tensor.matmul(ops4[:], ctxT[:, ib * BPC:(ib + 1) * BPC],
                             wtall[:, ib * H:(ib + 1) * H],
                             start=(ib == 0), stop=(ib == HB - 1))
        out_all = small.tile([BPC, H], f32, tag="outall")
        nc.vector.tensor_add(out_all[:], ops4[:], bias_all[:])
        nc.sync.dma_start(out_d[:, :], out_all[:])

    if legalize:
        _legalize_waits(nc)
    return nc


def _legalize_waits(nc, max_waits=1):
    """This toolchain's walrus encodes at most one semaphore wait per
    instruction. Hoist extra waits onto standalone EventSemaphore
    instructions on the same engine queue, immediately before the owner."""
    import json as _json
    m = _json.loads(bytes(nc.to_json_bytes()))
    n_fix = 0
    for fn in m["functions"]:
        for blk in fn["blocks"]:
            out = []
            for ins in blk.get("instructions", []):
                si = ins.get("sync_info") or {}
                waits = si.get("on_wait") or []
                if len(waits) > max_waits and ins.get("opcode") != "EventSemaphore":
                    extra, keep = waits[:-max_waits], waits[-max_waits:]
                    for i, w in enumerate(extra):
                        out.append({
                            "name": f"{ins['name']}-w{i}",
                            "engine": ins["engine"],
                            "opcode": "EventSemaphore",
                            "ins": [], "outs": [],
                            "sync_info": {"on_wait": [w], "on_update": []},
                        })
                    si["on_wait"] = keep
                    ins["sync_info"] = si
                    n_fix += 1
                out.append(ins)
            blk["instructions"] = out
    nc.m = mybir.module_from_json_bytes(_json.dumps(m).encode())
    return n_fix


def kernel(hidden: np.ndarray, W: np.ndarray, b: np.ndarray) -> np.ndarray:
    if "nc" not in _cached:
        _cached["nc"] = _build_program()
    nc = _cached["nc"]

    hidden = np.ascontiguousarray(hidden, dtype=np.float32)
    W = np.ascontiguousarray(W, dtype=np.float32)
    b = np.ascontiguousarray(b, dtype=np.float32)

    in_maps = []
    for ci in range(NCORES):
        in_maps.append({
            "hidden": hidden[ci * BPC:(ci + 1) * BPC],
            "W": W,
            "bvec": b,
        })
    res = run_bass_kernel_spmd(nc, in_maps, core_ids=list(range(NCORES)))
    _cached["last_results"] = res
    out = np.concatenate([res.results[ci]["out"] for ci in range(NCORES)], axis=0)
    return out.astype(np.float32)


if __name__ == "__main__":
    rng = np.random.default_rng(0)
    h = rng.standard_normal((B, L, H), dtype=np.float32)
    w = rng.standard_normal((H, H), dtype=np.float32) * (1.0 / np.sqrt(H))
    bb = np.zeros((H,), np.float32)
    o = kernel(h, w, bb)
    print(o.shape, o.dtype)


# revision 8
# speedup vs baseline: 1.4280x; 1.0070x over previous
"""DeltaRule memory scan kernel for Trainium2, 8 NeuronCores, data-parallel over batch.

Reference semantics (per batch element, H=512, L=2048):
    M_0 = 0  [H,H]
    for t in 0..L-2:   k = hidden[t]
        d = k.k + eps; delta = k - (M k)/d; M += outer(delta, k)
    out = (M @ hidden[L-1]) @ W.T + b

Implementation: chunked delta rule (UT transform), chunk C=128, float16.
Per chunk with keys K [C,H], r = 1/(rowsum(K^2)+eps):
    A  = strict_tril(diag(r) K K^T)            [C,C]
    T  = (I+A)^{-1} ~= (I-A)(I+A^2)(I+A^4)    (A nilpotent; exact through A^7)
    U  = K - diag(r) (K M^T + X Dl_prev)       [C,H]  (X-term: intra-pair cross)
    Dl = T U                                    [C,H]
    M^T += K_c0^T Dl_c0 + K_c1^T Dl_c1          (pair-accumulated in PSUM)
All matmuls f16 (1 cy/row); M^T state kept in f16 only (validated: relerr ~1.5e-3).
K^T and A^T produced by XBAR DMA transposes (no PE transposes in steady state).
Elementwise work spread across DVE / Act / Pool engines; 4 batch elements per
core with chunk phases interleaved for PE-queue continuity.
"""
import sys
import numpy as np
from contextlib import ExitStack

sys.path.insert(0, "/opt/trn_rl_repo")

import concourse.bass as bass
import concourse.mybir as mybir
from concourse import tile
from concourse.bass_utils import run_bass_kernel_spmd
from concourse.masks import make_identity, make_lower_triangular

B, L, H = 32, 2048, 512
NCORES = 8
BPC = B // NCORES          # 4 batch elements per core
C = 128                    # chunk size
T = L - 1                  # 2047 scan steps
NCHUNK = (T + C - 1) // C  # 16 (last chunk has 127 valid rows)
NLEV = 2                   # Neumann levels -> exact through A^7 (validated)
EPS = 1e-6
HB = H // 128              # 4 h-blocks

f32 = mybir.dt.float32
f16 = mybir.dt.float16

_cached = {}

# engine split for the 16 pair-end M updates (V=vector/DVE, P=pool)
_MUPD_ENG = "PVPPVPVPPVPPVPVP"


def _build_program(legalize=True):
    nc = bass.Bass(target_bir_lowering=False, debug=False)

    hidden_d = nc.declare_dram_parameter("hidden", [BPC, L, H], f32, isOutput=False)
    w_d = nc.declare_dram_parameter("W", [H, H], f32, isOutput=False)
    b_d = nc.declare_dram_parameter("bvec", [H], f32, isOutput=False)
    out_d = nc.declare_dram_parameter("out", [BPC, H], f32, isOutput=True)

    with tile.TileContext(nc) as tc, ExitStack() as ctx:
        consts = ctx.enter_context(tc.tile_pool(name="consts", bufs=1))
        wbuild = ctx.enter_context(tc.tile_pool(name="wbuild", bufs=2))
        mtpool = ctx.enter_context(tc.tile_pool(name="mt", bufs=1))
        kpool = ctx.enter_context(tc.tile_pool(name="k", bufs=12))
        k16pool = ctx.enter_context(tc.tile_pool(name="k16", bufs=4))
        chain = ctx.enter_context(tc.tile_pool(name="chain", bufs=6))
        upool = ctx.enter_context(tc.tile_pool(name="u", bufs=6))
        dlpool = ctx.enter_context(tc.tile_pool(name="dl", bufs=10))
        xpool = ctx.enter_context(tc.tile_pool(name="x", bufs=2))
        small = ctx.enter_context(tc.tile_pool(name="small", bufs=4))
        pslo = ctx.enter_context(tc.tile_pool(name="pslo", bufs=1, space="PSUM"))
        pshi = ctx.enter_context(tc.tile_pool(name="pshi", bufs=6, space="PSUM"))

        # ---- constants needed by the main loop ----
        ident_f = consts.tile([128, 128], f32, tag="identf")
        make_identity(nc, ident_f[:])
        ident_h = consts.tile([128, 128], f16, tag="identh")
        make_identity(nc, ident_h[:])
        identp = consts.tile([128, H], f16, tag="identp")
        for bi in range(BPC):
            nc.vector.tensor_copy(identp[:, bi * 128:(bi + 1) * 128], ident_h[:])
        smask = consts.tile([128, 128], f32, tag="smask")
        make_lower_triangular(nc, smask[:], val=1.0, diag=False)

        # persistent state M^T (f16 only), per (bi, jb)
        mt16 = [[mtpool.tile([128, H], f16, tag=f"mt{bi}_{jb}", name=f"mt{bi}_{jb}")
                 for jb in range(HB)] for bi in range(BPC)]

        # pre-zeroed key tiles for the final (127-row) chunk: zeroing them in
        # the prologue keeps the memset out of the busy mid-scan DVE queue
        k15 = [consts.tile([128, H], f32, tag=f"k15_{bi}", name=f"k15_{bi}")
               for bi in range(BPC)]
        for bi in range(BPC):
            nc.vector.memset(k15[bi][:], 0.0)

        G = {}

        def prep_load(c):
            t0 = c * C
            nrows = min(C, T - t0)
            st = {"k": []}
            for bi in range(BPC):
                if nrows < C:
                    k_t = k15[bi]
                    nc.sync.dma_start(k_t[:nrows, :], hidden_d[bi, t0:t0 + nrows, :])
                else:
                    k_t = kpool.tile([128, H], f32, tag="K", name=f"k{c}_{bi}")
                    nc.sync.dma_start(k_t[:], hidden_d[bi, t0:t0 + C, :])
                st["k"].append(k_t)
            # packed row-major f16 keys [128, (bi h)] (Pool: SBUF-only casts)
            k16a = k16pool.tile([128, BPC * H], f16, tag="k16", name=f"k16_{c}")
            for bi in range(BPC):
                nc.gpsimd.tensor_copy(k16a[:, bi * H:(bi + 1) * H], st["k"][bi][:])
            st["k16"] = k16a
            G[c] = st

        def prep_tr(c):
            # K^T blocks via PE transposes of the f32 keys (XBAR DMA on the
            # single SP queue head-of-line-blocks the whole pipeline on hw)
            st = G[c]
            kt16a = k16pool.tile([128, BPC * H], f16, tag="kt16", name=f"kt16_{c}")
            for bi in range(BPC):
                ktps = pshi.tile([128, H], f32, tag="big")
                for hb in range(HB):
                    nc.tensor.transpose(ktps[:, hb * 128:(hb + 1) * 128],
                                        st["k"][bi][:, hb * 128:(hb + 1) * 128],
                                        ident_f[:])
                nc.scalar.copy(kt16a[:, bi * H:(bi + 1) * H], ktps[:])
            st["kt16"] = kt16a

        def prep_r(c):
            st = G[c]
            dall = small.tile([128, BPC], f32, tag="dall")
            for bi in range(BPC):
                scr = small.tile([128, H], f16, tag="scr")
                nc.scalar.activation(scr[:], st["k"][bi][:],
                                     mybir.ActivationFunctionType.Square,
                                     accum_out=dall[:, bi:bi + 1])
            rall = small.tile([128, BPC], f32, tag="rall")
            if c == NCHUNK - 1:
                # only the zero-padded final chunk needs the EPS guard
                nc.vector.tensor_scalar_add(dall[:], dall[:], EPS)
            nc.vector.reciprocal(rall[:], dall[:])
            st["r"] = rall

        def aform(c):
            st = G[c]
            kt = st["kt16"]
            a_ps = pshi.tile([128, H], f32, tag="big")
            for bi in range(BPC):
                sl = slice(bi * 128, (bi + 1) * 128)
                for hb in range(HB):
                    hsl = slice(bi * H + hb * 128, bi * H + (hb + 1) * 128)
                    nc.tensor.matmul(a_ps[:, sl], kt[:, hsl], kt[:, hsl],
                                     start=(hb == 0), stop=(hb == HB - 1))
            a_all = chain.tile([128, H], f16, tag="ak")
            for bi in range(BPC):
                sl = slice(bi * 128, (bi + 1) * 128)
                nc.vector.scalar_tensor_tensor(a_all[:, sl], a_ps[:, sl],
                                               st["r"][:, bi:bi + 1], smask[:],
                                               mybir.AluOpType.mult,
                                               mybir.AluOpType.mult)
            at_ps = pshi.tile([128, H], f16, tag="big")
            for bi in range(BPC):
                sl = slice(bi * 128, (bi + 1) * 128)
                nc.tensor.transpose(at_ps[:, sl], a_all[:, sl], ident_h[:])
            at_all = chain.tile([128, H], f16, tag="atk")
            nc.scalar.copy(at_all[:], at_ps[:])
            g0 = chain.tile([128, H], f16, tag="g")
            nc.vector.tensor_sub(g0[:], identp[:], at_all[:])
            st["ak"], st["atk"], st["g"] = a_all, at_all, g0

        def chain_sq(c, lev):
            # squaring half: A^(2^lev); its transpose via XBAR DMA (free-ish)
            st = G[c]
            ak, atk = st["ak"], st["atk"]
            sq1 = pshi.tile([128, H], f32, tag="big")
            for bi in range(BPC):
                sl = slice(bi * 128, (bi + 1) * 128)
                nc.tensor.matmul(sq1[:, sl], atk[:, sl], ak[:, sl], start=True, stop=True)
            ak2 = chain.tile([128, H], f16, tag="ak")
            nc.scalar.copy(ak2[:], sq1[:])
            if lev < NLEV:
                sq2 = pshi.tile([128, H], f32, tag="big")
                for bi in range(BPC):
                    sl = slice(bi * 128, (bi + 1) * 128)
                    nc.tensor.matmul(sq2[:, sl], ak[:, sl], atk[:, sl], start=True, stop=True)
                atk2 = chain.tile([128, H], f16, tag="atk")
                nc.scalar.copy(atk2[:], sq2[:])
            else:
                atk2 = None
            st["ak_n"], st["atk_n"] = ak2, atk2

        def chain_gps(c, lev):
            # accumulation half: g += (A^(2^lev))^T g
            st = G[c]
            ak2 = st["ak_n"]
            gps = pshi.tile([128, H], f32, tag="big")
            for bi in range(BPC):
                sl = slice(bi * 128, (bi + 1) * 128)
                nc.tensor.matmul(gps[:, sl], ak2[:, sl], st["g"][:, sl], start=True, stop=True)
            g_nxt = chain.tile([128, H], f16, tag="g")
            nc.vector.tensor_add(g_nxt[:], gps[:], st["g"][:])
            st["ak"], st["atk"], st["g"] = st["ak_n"], st["atk_n"], g_nxt

        def chain_level(c, lev):
            chain_sq(c, lev)
            chain_gps(c, lev)

        def xform(c):
            # X^T for pair (c-1, c): xts[:, bi-slice][a, b] = K_{c-1}[a]·K_c[b]
            st, stp = G[c], G[c - 1]
            xps = pshi.tile([128, H], f32, tag="big")
            for bi in range(BPC):
                sl = slice(bi * 128, (bi + 1) * 128)
                for hb in range(HB):
                    hsl = slice(bi * H + hb * 128, bi * H + (hb + 1) * 128)
                    nc.tensor.matmul(xps[:, sl], stp["kt16"][:, hsl],
                                     st["kt16"][:, hsl],
                                     start=(hb == 0), stop=(hb == HB - 1))
            # negated so the (negated-dl) cross product lands with + sign
            xts = xpool.tile([128, H], f16, tag="x")
            nc.scalar.activation(xts[:], xps[:], mybir.ActivationFunctionType.Copy,
                                 scale=-1.0)
            st["x"] = xts

        def state_u(c):
            # chunks 0/1 update M individually (startup); pairs start at (2,3)
            st = G[c]
            cross = (c % 2 == 1 and c >= 3)
            st["u"] = []
            for bi in range(BPC):
                if c == 0:
                    st["u"].append(st["k16"][:, bi * H:(bi + 1) * H])
                    continue
                ups = pshi.tile([128, H], f32, tag="big")
                for hb in range(HB):
                    hsl = slice(bi * H + hb * 128, bi * H + (hb + 1) * 128)
                    nc.tensor.matmul(ups[:], st["kt16"][:, hsl],
                                     mt16[bi][hb][:],
                                     start=(hb == 0),
                                     stop=(hb == HB - 1 and not cross))
                if cross:
                    sl = slice(bi * 128, (bi + 1) * 128)
                    nc.tensor.matmul(ups[:], st["x"][:, sl], G[c - 1]["dl"][bi][:],
                                     start=False, stop=True)
                # u_neg = r*ups - k (negated U; saves materializing -r)
                u_sb = upool.tile([128, H], f16, tag="u")
                nc.vector.scalar_tensor_tensor(u_sb[:], ups[:], st["r"][:, bi:bi + 1],
                                               st["k"][bi][:], mybir.AluOpType.mult,
                                               mybir.AluOpType.subtract)
                st["u"].append(u_sb[:])

        def state_delta(c):
            st = G[c]
            st["dl"] = []
            for bi in range(BPC):
                sl = slice(bi * 128, (bi + 1) * 128)
                dps = pshi.tile([128, H], f32, tag="big")
                nc.tensor.matmul(dps[:], st["g"][:, sl], st["u"][bi], start=True, stop=True)
                dl = dlpool.tile([128, H], f16, tag="dl")
                nc.scalar.copy(dl[:], dps[:])
                st["dl"].append(dl)

        def mupd(c, bis):
            # pair-end update: M^T += K_{c-1}^T Dl_{c-1} + K_c^T Dl_c
            # (bi 2,3 run at the start of the NEXT iteration to even PE load;
            #  some adds bounce PSUM->SBUF via DMA so Pool can do the add)
            st, stp = G[c], G[c - 1]
            for bi in bis:
                for jb in range(HB):
                    jsl = slice(bi * H + jb * 128, bi * H + (jb + 1) * 128)
                    mps = pshi.tile([128, H], f32, tag="big")
                    nc.tensor.matmul(mps[:], stp["k16"][:, jsl], stp["dl"][bi][:],
                                     start=True, stop=False)
                    nc.tensor.matmul(mps[:], st["k16"][:, jsl], st["dl"][bi][:],
                                     start=False, stop=True)
                    nc.vector.tensor_sub(mt16[bi][jb][:], mt16[bi][jb][:], mps[:])

        def mupd_single(c):
            # startup chunks 0 and 1: per-chunk M update (keeps PE busy early)
            st = G[c]
            for bi in range(BPC):
                for jb in range(HB):
                    jsl = slice(bi * H + jb * 128, bi * H + (jb + 1) * 128)
                    mps = pshi.tile([128, H], f32, tag="big")
                    nc.tensor.matmul(mps[:], st["k16"][:, jsl], st["dl"][bi][:],
                                     start=True, stop=True)
                    if c == 0:
                        nc.scalar.copy(mt16[bi][jb][:], mps[:])
                    else:
                        nc.vector.tensor_sub(mt16[bi][jb][:], mt16[bi][jb][:], mps[:])

        # ---- software-pipelined main loop ----
        # Issue order per iteration is tuned so each engine's in-order queue
        # services consumers before producers-for-later: PE never waits behind
        # unready work, Act chain copies aren't stuck behind squares, and the
        # k16->kt16 DMA for c+2 isn't stuck behind pair-end M adds.
        prep_load(0)
        prep_tr(0)
        prep_r(0)
        prep_load(1)
        prep_tr(1)
        prep_r(1)
        aform(0)
        for lev in range(1, NLEV + 1):
            chain_level(0, lev)
        for c in range(NCHUNK):
            nxt = c + 1 if c + 1 < NCHUNK else None
            nn = c + 2 if c + 2 < NCHUNK else None
            if c % 2 == 1 and c >= 3:
                xform(c)
            if nxt is not None:
                aform(nxt)
            state_u(c)
            if nxt is not None:
                chain_sq(nxt, 1)
            state_delta(c)
            if nxt is not None:
                chain_gps(nxt, 1)
                chain_sq(nxt, 2)
            if nn is not None:
                prep_load(nn)
            if c <= 1:
                mupd_single(c)
            elif c % 2 == 1:
                mupd(c, [0, 1, 2, 3])
            if nxt is not None:
                chain_gps(nxt, 2)
            if nn is not None:
                prep_tr(nn)
                prep_r(nn)
            prev = c - 3
            if prev in G:
                del G[prev]

        # ---- late prologue: read_proj weights + query (overlap with scan) ----
        # wtALL[:, ib*512 + op*128 + o] = W^T[ib*128 + i', op*128 + o]
        # (PE transposes: XBAR DMA here would head-of-line-block the SP queue)
        wtall = consts.tile([128, HB * H], f16, tag="wtall")
        for op in range(HB):
            wsb = wbuild.tile([128, H], f32, tag="wsb")
            nc.sync.dma_start(wsb[:], w_d[op * 128:(op + 1) * 128, :])
            wps = pshi.tile([128, H], f32, tag="big")
            for ib in range(HB):
                nc.tensor.transpose(wps[:, ib * 128:(ib + 1) * 128],
                                    wsb[:, ib * 128:(ib + 1) * 128], ident_f[:])
            nc.scalar.copy(
                wtall[:].rearrange("p (f o2 q) -> p f o2 q", f=HB, q=128)[:, :, op, :],
                wps[:])
        bias_all = consts.tile([BPC, H], f32, tag="biasall")
        for bi in range(BPC):
            nc.sync.dma_start(bias_all[bi:bi + 1, :], b_d[None, :])
        qs = []
        for bi in range(BPC):
            v4 = wbuild.tile([HB, 128], f32, tag="v4")
            nc.sync.dma_start(v4[:], hidden_d[bi, L - 1, :].rearrange("(f p) -> f p", p=128))
            tps = pslo.tile([128, HB], f32, tag="sm")
            nc.tensor.transpose(tps[:], v4[:], ident_f[:HB, :HB])
            q_t = consts.tile([128, HB], f16, tag=f"q{bi}", name=f"q{bi}")
            nc.scalar.copy(q_t[:], tps[:])
            qs.append(q_t)

        # ---- finale: ctx = M q (row form); out = ctx W^T + b ----
        # phase-major over bi so PE/Act/DVE overlap across batch elements;
        # out-proj packs all 4 bi into one lhsT per ib block (4 matmuls total)
        cpss, ctx_rows = [], []
        for bi in range(BPC):
            cps = pshi.tile([1, H], f32, tag="big")
            for jb in range(HB):
                nc.tensor.matmul(cps[:], qs[bi][:, jb:jb + 1], mt16[bi][jb][:],
                                 start=(jb == 0), stop=(jb == HB - 1))
            cpss.append(cps)
        for bi in range(BPC):
            ctx_row = small.tile([1, H], f16, tag=f"ctxrow{bi}")
            nc.scalar.copy(ctx_row[:], cpss[bi][:])
            ctx_rows.append(ctx_row)
        ctxT = small.tile([128, HB * BPC], f16, tag="ctxT")
        for bi in range(BPC):
            for ib in range(HB):
                tp2 = pslo.tile([128, 1], f16, tag="sm1")
                nc.tensor.transpose(tp2[:], ctx_rows[bi][:, ib * 128:(ib + 1) * 128],
                                    ident_h[:1, :1])
                nc.vector.tensor_copy(ctxT[:, ib * BPC + bi:ib * BPC + bi + 1], tp2[:])
        ops4 = pshi.tile([BPC, H], f32, tag="big")
        for ib in range(HB):
            nc.tensor.matmul(ops4[:], ctxT[:, ib * BPC:(ib + 1) * BPC],
                             wtall[:, ib * H:(ib + 1) * H],
                             start=(ib == 0), stop=(ib == HB - 1))
        out_all = small.tile([BPC, H], f32, tag="outall")
        nc.vector.tensor_add(out_all[:], ops4[:], bias_all[:])
        nc.sync.dma_start(out_d[:, :], out_all[:])

    if legalize:
        _legalize_waits(nc)
    return nc


def _legalize_waits(nc, max_waits=1):
    """This toolchain's walrus encodes at most one semaphore wait per
    instruction. Hoist extra waits onto standalone EventSemaphore
    instructions on the same engine queue, immediately before the owner."""
    import json as _json
    m = _json.loads(bytes(nc.to_json_bytes()))
    n_fix = 0
    for fn in m["functions"]:
        for blk in fn["blocks"]:
            out = []
            for ins in blk.get("instructions", []):
                si = ins.get("sync_info") or {}
                waits = si.get("on_wait") or []
                if len(waits) > max_waits and ins.get("opcode") != "EventSemaphore":
                    extra, keep = waits[:-max_waits], waits[-max_waits:]
                    for i, w in enumerate(extra):
                        out.append({
                            "name": f"{ins['name']}-w{i}",
                            "engine": ins["engine"],
                            "opcode": "EventSemaphore",
                            "ins": [], "outs": [],
                            "sync_info": {"on_wait": [w], "on_update": []},
                        })
                    si["on_wait"] = keep
                    ins["sync_info"] = si
                    n_fix += 1
                out.append(ins)
            blk["instructions"] = out
    nc.m = mybir.module_from_json_bytes(_json.dumps(m).encode())
    return n_fix


def kernel(hidden: np.ndarray, W: np.ndarray, b: np.ndarray) -> np.ndarray:
    if "nc" not in _cached:
        _cached["nc"] = _build_program()
    nc = _cached["nc"]

    hidden = np.ascontiguousarray(hidden, dtype=np.float32)
    W = np.ascontiguousarray(W, dtype=np.float32)
    b = np.ascontiguousarray(b, dtype=np.float32)

    in_maps = []
    for ci in range(NCORES):
        in_maps.append({
            "hidden": hidden[ci * BPC:(ci + 1) * BPC],
            "W": W,
            "bvec": b,
        })
    res = run_bass_kernel_spmd(nc, in_maps, core_ids=list(range(NCORES)))
    _cached["last_results"] = res
    out = np.concatenate([res.results[ci]["out"] for ci in range(NCORES)], axis=0)
    return out.astype(np.float32)


if __name__ == "__main__":
    rng = np.random.default_rng(0)
    h = rng.standard_normal((B, L, H), dtype=np.float32)
    w = rng.standard_normal((H, H), dtype=np.float32) * (1.0 / np.sqrt(H))
    bb = np.zeros((H,), np.float32)
    o = kernel(h, w, bb)
    print(o.shape, o.dtype)


# revision 9
# speedup vs baseline: 1.5311x; 1.0722x over previous
"""DeltaRule memory scan kernel for Trainium2, 8 NeuronCores, data-parallel over batch.

Reference semantics (per batch element, H=512, L=2048):
    M_0 = 0  [H,H]
    for t in 0..L-2:   k = hidden[t]
        d = k.k + eps; delta = k - (M k)/d; M += outer(delta, k)
    out = (M @ hidden[L-1]) @ W.T + b

Implementation: chunked delta rule (UT transform), chunk C=128, float16.
Per chunk with keys K [C,H], r = 1/(rowsum(K^2)+eps):
    A  = strict_tril(diag(r) K K^T)            [C,C]
    T  = (I+A)^{-1} ~= (I-A)(I+A^2)(I+A^4)    (A nilpotent; exact through A^7)
    U  = K - diag(r) (K M^T + X Dl_prev)       [C,H]  (X-term: intra-pair cross)
    Dl = T U                                    [C,H]
    M^T += K_c0^T Dl_c0 + K_c1^T Dl_c1          (pair-accumulated in PSUM)
All matmuls f16 (1 cy/row); M^T state kept in f16 only (validated: relerr ~1.5e-3).
K^T and A^T produced by XBAR DMA transposes (no PE transposes in steady state).
Elementwise work spread across DVE / Act / Pool engines; 4 batch elements per
core with chunk phases interleaved for PE-queue continuity.
"""
import sys
import numpy as np
from contextlib import ExitStack

sys.path.insert(0, "/opt/trn_rl_repo")

import concourse.bass as bass
import concourse.mybir as mybir
from concourse import tile
from concourse.bass_utils import run_bass_kernel_spmd
from concourse.masks import make_identity, make_lower_triangular

B, L, H = 32, 2048, 512
NCORES = 8
BPC = B // NCORES          # 4 batch elements per core
C = 128                    # chunk size
T = L - 1                  # 2047 scan steps
NCHUNK = (T + C - 1) // C  # 16 (last chunk has 127 valid rows)
NLEV = 2                   # Neumann levels -> exact through A^7 (validated)
EPS = 1e-6
HB = H // 128              # 4 h-blocks

f32 = mybir.dt.float32
f16 = mybir.dt.float16

_cached = {}

# engine split for the 16 pair-end M updates (V=vector/DVE, P=pool)
_MUPD_ENG = "PVPPVPVPPVPPVPVP"


def _build_program(legalize=True):
    nc = bass.Bass(target_bir_lowering=False, debug=False)

    hidden_d = nc.declare_dram_parameter("hidden", [BPC, L, H], f32, isOutput=False)
    w_d = nc.declare_dram_parameter("W", [H, H], f32, isOutput=False)
    b_d = nc.declare_dram_parameter("bvec", [H], f32, isOutput=False)
    out_d = nc.declare_dram_parameter("out", [BPC, H], f32, isOutput=True)

    with tile.TileContext(nc) as tc, ExitStack() as ctx:
        consts = ctx.enter_context(tc.tile_pool(name="consts", bufs=1))
        wbuild = ctx.enter_context(tc.tile_pool(name="wbuild", bufs=2))
        mtpool = ctx.enter_context(tc.tile_pool(name="mt", bufs=1))
        kpool = ctx.enter_context(tc.tile_pool(name="k", bufs=12))
        k16pool = ctx.enter_context(tc.tile_pool(name="k16", bufs=4))
        chain = ctx.enter_context(tc.tile_pool(name="chain", bufs=6))
        upool = ctx.enter_context(tc.tile_pool(name="u", bufs=6))
        dlpool = ctx.enter_context(tc.tile_pool(name="dl", bufs=10))
        xpool = ctx.enter_context(tc.tile_pool(name="x", bufs=2))
        small = ctx.enter_context(tc.tile_pool(name="small", bufs=4))
        pslo = ctx.enter_context(tc.tile_pool(name="pslo", bufs=1, space="PSUM"))
        pshi = ctx.enter_context(tc.tile_pool(name="pshi", bufs=6, space="PSUM"))

        # ---- constants needed by the main loop ----
        ident_f = consts.tile([128, 128], f32, tag="identf")
        make_identity(nc, ident_f[:])
        ident_h = consts.tile([128, 128], f16, tag="identh")
        make_identity(nc, ident_h[:])
        identp = consts.tile([128, H], f16, tag="identp")
        for bi in range(BPC):
            nc.vector.tensor_copy(identp[:, bi * 128:(bi + 1) * 128], ident_h[:])
        smask = consts.tile([128, 128], f32, tag="smask")
        make_lower_triangular(nc, smask[:], val=1.0, diag=False)

        # persistent state M^T (f16 only), per (bi, jb)
        mt16 = [[mtpool.tile([128, H], f16, tag=f"mt{bi}_{jb}", name=f"mt{bi}_{jb}")
                 for jb in range(HB)] for bi in range(BPC)]

        # pre-zeroed key tiles for the final (127-row) chunk: zeroing them in
        # the prologue keeps the memset out of the busy mid-scan DVE queue
        k15 = [consts.tile([128, H], f32, tag=f"k15_{bi}", name=f"k15_{bi}")
               for bi in range(BPC)]
        for bi in range(BPC):
            nc.vector.memset(k15[bi][:], 0.0)

        G = {}

        def prep_load(c):
            t0 = c * C
            nrows = min(C, T - t0)
            st = {"k": []}
            for bi in range(BPC):
                if nrows < C:
                    k_t = k15[bi]
                    nc.sync.dma_start(k_t[:nrows, :], hidden_d[bi, t0:t0 + nrows, :])
                else:
                    k_t = kpool.tile([128, H], f32, tag="K", name=f"k{c}_{bi}")
                    nc.sync.dma_start(k_t[:], hidden_d[bi, t0:t0 + C, :])
                st["k"].append(k_t)
            # packed row-major f16 keys [128, (bi h)] (Pool: SBUF-only casts)
            k16a = k16pool.tile([128, BPC * H], f16, tag="k16", name=f"k16_{c}")
            for bi in range(BPC):
                nc.gpsimd.tensor_copy(k16a[:, bi * H:(bi + 1) * H], st["k"][bi][:])
            st["k16"] = k16a
            G[c] = st

        def prep_tr(c):
            # K^T blocks via PE transposes of the f32 keys (XBAR DMA on the
            # single SP queue head-of-line-blocks the whole pipeline on hw)
            st = G[c]
            kt16a = k16pool.tile([128, BPC * H], f16, tag="kt16", name=f"kt16_{c}")
            for bi in range(BPC):
                ktps = pshi.tile([128, H], f32, tag="big")
                for hb in range(HB):
                    nc.tensor.transpose(ktps[:, hb * 128:(hb + 1) * 128],
                                        st["k"][bi][:, hb * 128:(hb + 1) * 128],
                                        ident_f[:])
                nc.scalar.copy(kt16a[:, bi * H:(bi + 1) * H], ktps[:])
            st["kt16"] = kt16a

        def prep_r(c):
            st = G[c]
            dall = small.tile([128, BPC], f32, tag="dall")
            for bi in range(BPC):
                scr = small.tile([128, H], f16, tag="scr")
                nc.scalar.activation(scr[:], st["k"][bi][:],
                                     mybir.ActivationFunctionType.Square,
                                     accum_out=dall[:, bi:bi + 1])
            rall = small.tile([128, BPC], f32, tag="rall")
            if c == NCHUNK - 1:
                # only the zero-padded final chunk needs the EPS guard
                nc.vector.tensor_scalar_add(dall[:], dall[:], EPS)
            nc.vector.reciprocal(rall[:], dall[:])
            st["r"] = rall

        def aform(c):
            st = G[c]
            kt = st["kt16"]
            a_ps = pshi.tile([128, H], f32, tag="big")
            for bi in range(BPC):
                sl = slice(bi * 128, (bi + 1) * 128)
                for hb in range(HB):
                    hsl = slice(bi * H + hb * 128, bi * H + (hb + 1) * 128)
                    nc.tensor.matmul(a_ps[:, sl], kt[:, hsl], kt[:, hsl],
                                     start=(hb == 0), stop=(hb == HB - 1))
            a_all = chain.tile([128, H], f16, tag="ak")
            for bi in range(BPC):
                sl = slice(bi * 128, (bi + 1) * 128)
                nc.vector.scalar_tensor_tensor(a_all[:, sl], a_ps[:, sl],
                                               st["r"][:, bi:bi + 1], smask[:],
                                               mybir.AluOpType.mult,
                                               mybir.AluOpType.mult)
            at_ps = pshi.tile([128, H], f16, tag="big")
            for bi in range(BPC):
                sl = slice(bi * 128, (bi + 1) * 128)
                nc.tensor.transpose(at_ps[:, sl], a_all[:, sl], ident_h[:])
            at_all = chain.tile([128, H], f16, tag="atk")
            nc.scalar.copy(at_all[:], at_ps[:])
            g0 = chain.tile([128, H], f16, tag="g")
            nc.vector.tensor_sub(g0[:], identp[:], at_all[:])
            st["ak"], st["atk"], st["g"] = a_all, at_all, g0

        def chain_sq(c, lev):
            # squaring half: A^(2^lev); its transpose via XBAR DMA (free-ish)
            st = G[c]
            ak, atk = st["ak"], st["atk"]
            sq1 = pshi.tile([128, H], f32, tag="big")
            for bi in range(BPC):
                sl = slice(bi * 128, (bi + 1) * 128)
                nc.tensor.matmul(sq1[:, sl], atk[:, sl], ak[:, sl], start=True, stop=True)
            ak2 = chain.tile([128, H], f16, tag="ak")
            nc.scalar.copy(ak2[:], sq1[:])
            if lev < NLEV:
                sq2 = pshi.tile([128, H], f32, tag="big")
                for bi in range(BPC):
                    sl = slice(bi * 128, (bi + 1) * 128)
                    nc.tensor.matmul(sq2[:, sl], ak[:, sl], atk[:, sl], start=True, stop=True)
                atk2 = chain.tile([128, H], f16, tag="atk")
                nc.scalar.copy(atk2[:], sq2[:])
            else:
                atk2 = None
            st["ak_n"], st["atk_n"] = ak2, atk2

        def chain_gps(c, lev):
            # accumulation half: g += (A^(2^lev))^T g
            st = G[c]
            ak2 = st["ak_n"]
            gps = pshi.tile([128, H], f32, tag="big")
            for bi in range(BPC):
                sl = slice(bi * 128, (bi + 1) * 128)
                nc.tensor.matmul(gps[:, sl], ak2[:, sl], st["g"][:, sl], start=True, stop=True)
            g_nxt = chain.tile([128, H], f16, tag="g")
            nc.vector.tensor_add(g_nxt[:], gps[:], st["g"][:])
            st["ak"], st["atk"], st["g"] = st["ak_n"], st["atk_n"], g_nxt

        def chain_level(c, lev):
            chain_sq(c, lev)
            chain_gps(c, lev)

        def xform(c):
            # X^T for pair (c-1, c): xts[:, bi-slice][a, b] = K_{c-1}[a]·K_c[b]
            st, stp = G[c], G[c - 1]
            xps = pshi.tile([128, H], f32, tag="big")
            for bi in range(BPC):
                sl = slice(bi * 128, (bi + 1) * 128)
                for hb in range(HB):
                    hsl = slice(bi * H + hb * 128, bi * H + (hb + 1) * 128)
                    nc.tensor.matmul(xps[:, sl], stp["kt16"][:, hsl],
                                     st["kt16"][:, hsl],
                                     start=(hb == 0), stop=(hb == HB - 1))
            # negated so the (negated-dl) cross product lands with + sign
            xts = xpool.tile([128, H], f16, tag="x")
            nc.scalar.activation(xts[:], xps[:], mybir.ActivationFunctionType.Copy,
                                 scale=-1.0)
            st["x"] = xts

        def state_u(c):
            # chunks 0/1 update M individually (startup); pairs start at (2,3)
            st = G[c]
            cross = (c % 2 == 1 and c >= 3)
            st["u"] = []
            for bi in range(BPC):
                if c == 0:
                    st["u"].append(st["k16"][:, bi * H:(bi + 1) * H])
                    continue
                ups = pshi.tile([128, H], f32, tag="big")
                for hb in range(HB):
                    hsl = slice(bi * H + hb * 128, bi * H + (hb + 1) * 128)
                    nc.tensor.matmul(ups[:], st["kt16"][:, hsl],
                                     mt16[bi][hb][:],
                                     start=(hb == 0),
                                     stop=(hb == HB - 1 and not cross))
                if cross:
                    sl = slice(bi * 128, (bi + 1) * 128)
                    nc.tensor.matmul(ups[:], st["x"][:, sl], G[c - 1]["dl"][bi][:],
                                     start=False, stop=True)
                # u_neg = r*ups - k (negated U; saves materializing -r)
                u_sb = upool.tile([128, H], f16, tag="u")
                nc.vector.scalar_tensor_tensor(u_sb[:], ups[:], st["r"][:, bi:bi + 1],
                                               st["k"][bi][:], mybir.AluOpType.mult,
                                               mybir.AluOpType.subtract)
                st["u"].append(u_sb[:])

        def state_delta(c):
            st = G[c]
            st["dl"] = []
            for bi in range(BPC):
                sl = slice(bi * 128, (bi + 1) * 128)
                dps = pshi.tile([128, H], f32, tag="big")
                nc.tensor.matmul(dps[:], st["g"][:, sl], st["u"][bi], start=True, stop=True)
                dl = dlpool.tile([128, H], f16, tag="dl")
                nc.scalar.copy(dl[:], dps[:])
                st["dl"].append(dl)

        def mupd(c, bis):
            # pair-end update: M^T += K_{c-1}^T Dl_{c-1} + K_c^T Dl_c
            # (bi 2,3 run at the start of the NEXT iteration to even PE load;
            #  some adds bounce PSUM->SBUF via DMA so Pool can do the add)
            st, stp = G[c], G[c - 1]
            for bi in bis:
                for jb in range(HB):
                    jsl = slice(bi * H + jb * 128, bi * H + (jb + 1) * 128)
                    mps = pshi.tile([128, H], f32, tag="big")
                    nc.tensor.matmul(mps[:], stp["k16"][:, jsl], stp["dl"][bi][:],
                                     start=True, stop=False)
                    nc.tensor.matmul(mps[:], st["k16"][:, jsl], st["dl"][bi][:],
                                     start=False, stop=True)
                    nc.vector.tensor_sub(mt16[bi][jb][:], mt16[bi][jb][:], mps[:])

        def mupd_single(c):
            # startup chunks 0 and 1: per-chunk M update (keeps PE busy early)
            st = G[c]
            for bi in range(BPC):
                for jb in range(HB):
                    jsl = slice(bi * H + jb * 128, bi * H + (jb + 1) * 128)
                    mps = pshi.tile([128, H], f32, tag="big")
                    nc.tensor.matmul(mps[:], st["k16"][:, jsl], st["dl"][bi][:],
                                     start=True, stop=True)
                    if c == 0:
                        nc.scalar.copy(mt16[bi][jb][:], mps[:])
                    else:
                        nc.vector.tensor_sub(mt16[bi][jb][:], mt16[bi][jb][:], mps[:])

        # ---- early prologue: read_proj weights + query + bias ----
        # (issued first: one of these DMAs expands to a long fine-grained
        # descriptor burst; at program start it overlaps the pipeline fill)
        # wtALL[:, ib*512 + op*128 + o] = W^T[ib*128 + i', op*128 + o]
        wtall = consts.tile([128, HB * H], f16, tag="wtall")
        for op in range(HB):
            wsb = wbuild.tile([128, H], f32, tag="wsb")
            nc.sync.dma_start(wsb[:], w_d[op * 128:(op + 1) * 128, :])
            wps = pshi.tile([128, H], f32, tag="big")
            for ib in range(HB):
                nc.tensor.transpose(wps[:, ib * 128:(ib + 1) * 128],
                                    wsb[:, ib * 128:(ib + 1) * 128], ident_f[:])
            nc.scalar.copy(
                wtall[:].rearrange("p (f o2 q) -> p f o2 q", f=HB, q=128)[:, :, op, :],
                wps[:])
        bias_all = consts.tile([BPC, H], f32, tag="biasall")
        for bi in range(BPC):
            nc.sync.dma_start(bias_all[bi:bi + 1, :], b_d[None, :])
        qs = []
        for bi in range(BPC):
            v4 = wbuild.tile([HB, 128], f32, tag="v4")
            nc.sync.dma_start(v4[:], hidden_d[bi, L - 1, :].rearrange("(f p) -> f p", p=128))
            tps = pslo.tile([128, HB], f32, tag="sm")
            nc.tensor.transpose(tps[:], v4[:], ident_f[:HB, :HB])
            q_t = consts.tile([128, HB], f16, tag=f"q{bi}", name=f"q{bi}")
            nc.scalar.copy(q_t[:], tps[:])
            qs.append(q_t)

        # ---- software-pipelined main loop ----
        # Issue order per iteration is tuned so each engine's in-order queue
        # services consumers before producers-for-later: PE never waits behind
        # unready work, Act chain copies aren't stuck behind squares, and the
        # k16->kt16 DMA for c+2 isn't stuck behind pair-end M adds.
        prep_load(0)
        prep_tr(0)
        prep_r(0)
        prep_load(1)
        prep_tr(1)
        prep_r(1)
        aform(0)
        for lev in range(1, NLEV + 1):
            chain_level(0, lev)
        for c in range(NCHUNK):
            nxt = c + 1 if c + 1 < NCHUNK else None
            nn = c + 2 if c + 2 < NCHUNK else None
            if c % 2 == 1 and c >= 3:
                xform(c)
            if nxt is not None:
                aform(nxt)
            state_u(c)
            if nxt is not None:
                chain_sq(nxt, 1)
            state_delta(c)
            if nxt is not None:
                chain_gps(nxt, 1)
                chain_sq(nxt, 2)
            if nn is not None:
                prep_load(nn)
            if c <= 1:
                mupd_single(c)
            elif c % 2 == 1:
                mupd(c, [0, 1, 2, 3])
            if nxt is not None:
                chain_gps(nxt, 2)
            if nn is not None:
                prep_tr(nn)
                prep_r(nn)
            prev = c - 3
            if prev in G:
                del G[prev]

        # ---- finale: ctx = M q (row form); out = ctx W^T + b ----
        # phase-major over bi so PE/Act/DVE overlap across batch elements;
        # out-proj packs all 4 bi into one lhsT per ib block (4 matmuls total)
        cpss, ctx_rows = [], []
        for bi in range(BPC):
            cps = pshi.tile([1, H], f32, tag="big")
            for jb in range(HB):
                nc.tensor.matmul(cps[:], qs[bi][:, jb:jb + 1], mt16[bi][jb][:],
                                 start=(jb == 0), stop=(jb == HB - 1))
            cpss.append(cps)
        for bi in range(BPC):
            ctx_row = small.tile([1, H], f16, tag=f"ctxrow{bi}")
            nc.scalar.copy(ctx_row[:], cpss[bi][:])
            ctx_rows.append(ctx_row)
        ctxT = small.tile([128, HB * BPC], f16, tag="ctxT")
        for bi in range(BPC):
            for ib in range(HB):
                tp2 = pslo.tile([128, 1], f16, tag="sm1")
                nc.tensor.transpose(tp2[:], ctx_rows[bi][:, ib * 128:(ib + 1) * 128],
                                    ident_h[:1, :1])
                nc.vector.tensor_copy(ctxT[:, ib * BPC + bi:ib * BPC + bi + 1], tp2[:])
        ops4 = pshi.tile([BPC, H], f32, tag="big")
        for ib in range(HB):
            nc.tensor.matmul(ops4[:], ctxT[:, ib * BPC:(ib + 1) * BPC],
                             wtall[:, ib * H:(ib + 1) * H],
                             start=(ib == 0), stop=(ib == HB - 1))
        out_all = small.tile([BPC, H], f32, tag="outall")
        nc.vector.tensor_add(out_all[:], ops4[:], bias_all[:])
        nc.sync.dma_start(out_d[:, :], out_all[:])

    if legalize:
        _legalize_waits(nc)
    return nc


def _legalize_waits(nc, max_waits=1):
    """This toolchain's walrus encodes at most one semaphore wait per
    instruction. Hoist extra waits onto standalone EventSemaphore
    instructions on the same engine queue, immediately before the owner."""
    import json as _json
    m = _json.loads(bytes(nc.to_json_bytes()))
    n_fix = 0
    for fn in m["functions"]:
        for blk in fn["blocks"]:
            out = []
            for ins in blk.get("instructions", []):
                si = ins.get("sync_info") or {}
                waits = si.get("on_wait") or []
                if len(waits) > max_waits and ins.get("opcode") != "EventSemaphore":
                    extra, keep = waits[:-max_waits], waits[-max_waits:]
                    for i, w in enumerate(extra):
                        out.append({
                            "name": f"{ins['name']}-w{i}",
                            "engine": ins["engine"],
                            "opcode": "EventSemaphore",
                            "ins": [], "outs": [],
                            "sync_info": {"on_wait": [w], "on_update": []},
                        })
                    si["on_wait"] = keep
                    ins["sync_info"] = si
                    n_fix += 1
                out.append(ins)
            blk["instructions"] = out
    nc.m = mybir.module_from_json_bytes(_json.dumps(m).encode())
    return n_fix


def kernel(hidden: np.ndarray, W: np.ndarray, b: np.ndarray) -> np.ndarray:
    if "nc" not in _cached:
        _cached["nc"] = _build_program()
    nc = _cached["nc"]

    hidden = np.ascontiguousarray(hidden, dtype=np.float32)
    W = np.ascontiguousarray(W, dtype=np.float32)
    b = np.ascontiguousarray(b, dtype=np.float32)

    in_maps = []
    for ci in range(NCORES):
        in_maps.append({
            "hidden": hidden[ci * BPC:(ci + 1) * BPC],
            "W": W,
            "bvec": b,
        })
    res = run_bass_kernel_spmd(nc, in_maps, core_ids=list(range(NCORES)))
    _cached["last_results"] = res
    out = np.concatenate([res.results[ci]["out"] for ci in range(NCORES)], axis=0)
    return out.astype(np.float32)


if __name__ == "__main__":
    rng = np.random.default_rng(0)
    h = rng.standard_normal((B, L, H), dtype=np.float32)
    w = rng.standard_normal((H, H), dtype=np.float32) * (1.0 / np.sqrt(H))
    bb = np.zeros((H,), np.float32)
    o = kernel(h, w, bb)
    print(o.shape, o.dtype)


# revision 10
# speedup vs baseline: 1.5329x; 1.0012x over previous
"""DeltaRule memory scan kernel for Trainium2, 8 NeuronCores, data-parallel over batch.

Reference semantics (per batch element, H=512, L=2048):
    M_0 = 0  [H,H]
    for t in 0..L-2:   k = hidden[t]
        d = k.k + eps; delta = k - (M k)/d; M += outer(delta, k)
    out = (M @ hidden[L-1]) @ W.T + b

Implementation: chunked delta rule (UT transform), chunk C=128, float16.
Per chunk with keys K [C,H], r = 1/(rowsum(K^2)+eps):
    A  = strict_tril(diag(r) K K^T)            [C,C]
    T  = (I+A)^{-1} ~= (I-A)(I+A^2)(I+A^4)    (A nilpotent; exact through A^7)
    U  = K - diag(r) (K M^T + X Dl_prev)       [C,H]  (X-term: intra-pair cross)
    Dl = T U                                    [C,H]
    M^T += K_c0^T Dl_c0 + K_c1^T Dl_c1          (pair-accumulated in PSUM)
All matmuls f16 (1 cy/row); M^T state kept in f16 only (validated: relerr ~1.5e-3).
K^T and A^T produced by XBAR DMA transposes (no PE transposes in steady state).
Elementwise work spread across DVE / Act / Pool engines; 4 batch elements per
core with chunk phases interleaved for PE-queue continuity.
"""
import sys
import numpy as np
from contextlib import ExitStack

sys.path.insert(0, "/opt/trn_rl_repo")

import concourse.bass as bass
import concourse.mybir as mybir
from concourse import tile
from concourse.bass_utils import run_bass_kernel_spmd
from concourse.masks import make_identity, make_lower_triangular

B, L, H = 32, 2048, 512
NCORES = 8
BPC = B // NCORES          # 4 batch elements per core
C = 128                    # chunk size
T = L - 1                  # 2047 scan steps
NCHUNK = (T + C - 1) // C  # 16 (last chunk has 127 valid rows)
NLEV = 2                   # Neumann levels -> exact through A^7 (validated)
EPS = 1e-6
HB = H // 128              # 4 h-blocks

f32 = mybir.dt.float32
f16 = mybir.dt.float16

_cached = {}

# engine split for the 16 pair-end M updates (V=vector/DVE, P=pool)
_MUPD_ENG = "PVPPVPVPPVPPVPVP"


def _build_program(legalize=True):
    nc = bass.Bass(target_bir_lowering=False, debug=False)

    hidden_d = nc.declare_dram_parameter("hidden", [BPC, L, H], f32, isOutput=False)
    w_d = nc.declare_dram_parameter("W", [H, H], f32, isOutput=False)
    b_d = nc.declare_dram_parameter("bvec", [H], f32, isOutput=False)
    out_d = nc.declare_dram_parameter("out", [BPC, H], f32, isOutput=True)

    with tile.TileContext(nc) as tc, ExitStack() as ctx:
        consts = ctx.enter_context(tc.tile_pool(name="consts", bufs=1))
        wbuild = ctx.enter_context(tc.tile_pool(name="wbuild", bufs=2))
        mtpool = ctx.enter_context(tc.tile_pool(name="mt", bufs=1))
        kpool = ctx.enter_context(tc.tile_pool(name="k", bufs=16))
        k16pool = ctx.enter_context(tc.tile_pool(name="k16", bufs=5))
        chain = ctx.enter_context(tc.tile_pool(name="chain", bufs=6))
        upool = ctx.enter_context(tc.tile_pool(name="u", bufs=6))
        dlpool = ctx.enter_context(tc.tile_pool(name="dl", bufs=10))
        xpool = ctx.enter_context(tc.tile_pool(name="x", bufs=2))
        small = ctx.enter_context(tc.tile_pool(name="small", bufs=4))
        pslo = ctx.enter_context(tc.tile_pool(name="pslo", bufs=1, space="PSUM"))
        pshi = ctx.enter_context(tc.tile_pool(name="pshi", bufs=6, space="PSUM"))

        # ---- constants needed by the main loop ----
        ident_f = consts.tile([128, 128], f32, tag="identf")
        make_identity(nc, ident_f[:])
        ident_h = consts.tile([128, 128], f16, tag="identh")
        make_identity(nc, ident_h[:])
        identp = consts.tile([128, H], f16, tag="identp")
        for bi in range(BPC):
            nc.vector.tensor_copy(identp[:, bi * 128:(bi + 1) * 128], ident_h[:])
        smask = consts.tile([128, 128], f32, tag="smask")
        make_lower_triangular(nc, smask[:], val=1.0, diag=False)

        # persistent state M^T (f16 only), per (bi, jb)
        mt16 = [[mtpool.tile([128, H], f16, tag=f"mt{bi}_{jb}", name=f"mt{bi}_{jb}")
                 for jb in range(HB)] for bi in range(BPC)]

        # pre-zeroed key tiles for the final (127-row) chunk: zeroing them in
        # the prologue keeps the memset out of the busy mid-scan DVE queue
        k15 = [consts.tile([128, H], f32, tag=f"k15_{bi}", name=f"k15_{bi}")
               for bi in range(BPC)]
        for bi in range(BPC):
            nc.vector.memset(k15[bi][:], 0.0)

        G = {}

        def prep_load(c):
            t0 = c * C
            nrows = min(C, T - t0)
            st = {"k": []}
            for bi in range(BPC):
                if nrows < C:
                    k_t = k15[bi]
                    nc.sync.dma_start(k_t[:nrows, :], hidden_d[bi, t0:t0 + nrows, :])
                else:
                    k_t = kpool.tile([128, H], f32, tag="K", name=f"k{c}_{bi}")
                    nc.sync.dma_start(k_t[:], hidden_d[bi, t0:t0 + C, :])
                st["k"].append(k_t)
            # packed row-major f16 keys [128, (bi h)] (Pool: SBUF-only casts)
            k16a = k16pool.tile([128, BPC * H], f16, tag="k16", name=f"k16_{c}")
            for bi in range(BPC):
                nc.gpsimd.tensor_copy(k16a[:, bi * H:(bi + 1) * H], st["k"][bi][:])
            st["k16"] = k16a
            G[c] = st

        def prep_tr(c):
            # K^T blocks via f16 PE transposes of the packed keys (XBAR DMA on
            # the single SP queue head-of-line-blocks the whole pipeline on hw)
            st = G[c]
            kt16a = k16pool.tile([128, BPC * H], f16, tag="kt16", name=f"kt16_{c}")
            for bi in range(BPC):
                ktps = pshi.tile([128, H], f16, tag="big")
                for hb in range(HB):
                    hsl = slice(bi * H + hb * 128, bi * H + (hb + 1) * 128)
                    nc.tensor.transpose(ktps[:, hb * 128:(hb + 1) * 128],
                                        st["k16"][:, hsl], ident_h[:])
                nc.scalar.copy(kt16a[:, bi * H:(bi + 1) * H], ktps[:])
            st["kt16"] = kt16a

        def prep_r(c):
            st = G[c]
            dall = small.tile([128, BPC], f32, tag="dall")
            for bi in range(BPC):
                scr = small.tile([128, H], f16, tag="scr")
                nc.scalar.activation(scr[:], st["k"][bi][:],
                                     mybir.ActivationFunctionType.Square,
                                     accum_out=dall[:, bi:bi + 1])
            rall = small.tile([128, BPC], f32, tag="rall")
            if c == NCHUNK - 1:
                # only the zero-padded final chunk needs the EPS guard
                nc.vector.tensor_scalar_add(dall[:], dall[:], EPS)
            nc.vector.reciprocal(rall[:], dall[:])
            st["r"] = rall

        def aform(c):
            st = G[c]
            kt = st["kt16"]
            a_ps = pshi.tile([128, H], f32, tag="big")
            for bi in range(BPC):
                sl = slice(bi * 128, (bi + 1) * 128)
                for hb in range(HB):
                    hsl = slice(bi * H + hb * 128, bi * H + (hb + 1) * 128)
                    nc.tensor.matmul(a_ps[:, sl], kt[:, hsl], kt[:, hsl],
                                     start=(hb == 0), stop=(hb == HB - 1))
            a_all = chain.tile([128, H], f16, tag="ak")
            for bi in range(BPC):
                sl = slice(bi * 128, (bi + 1) * 128)
                nc.vector.scalar_tensor_tensor(a_all[:, sl], a_ps[:, sl],
                                               st["r"][:, bi:bi + 1], smask[:],
                                               mybir.AluOpType.mult,
                                               mybir.AluOpType.mult)
            at_ps = pshi.tile([128, H], f16, tag="big")
            for bi in range(BPC):
                sl = slice(bi * 128, (bi + 1) * 128)
                nc.tensor.transpose(at_ps[:, sl], a_all[:, sl], ident_h[:])
            at_all = chain.tile([128, H], f16, tag="atk")
            nc.scalar.copy(at_all[:], at_ps[:])
            g0 = chain.tile([128, H], f16, tag="g")
            nc.vector.tensor_sub(g0[:], identp[:], at_all[:])
            st["ak"], st["atk"], st["g"] = a_all, at_all, g0

        def chain_sq(c, lev):
            # squaring half: A^(2^lev); its transpose via XBAR DMA (free-ish)
            st = G[c]
            ak, atk = st["ak"], st["atk"]
            sq1 = pshi.tile([128, H], f32, tag="big")
            for bi in range(BPC):
                sl = slice(bi * 128, (bi + 1) * 128)
                nc.tensor.matmul(sq1[:, sl], atk[:, sl], ak[:, sl], start=True, stop=True)
            ak2 = chain.tile([128, H], f16, tag="ak")
            nc.scalar.copy(ak2[:], sq1[:])
            if lev < NLEV:
                sq2 = pshi.tile([128, H], f32, tag="big")
                for bi in range(BPC):
                    sl = slice(bi * 128, (bi + 1) * 128)
                    nc.tensor.matmul(sq2[:, sl], ak[:, sl], atk[:, sl], start=True, stop=True)
                atk2 = chain.tile([128, H], f16, tag="atk")
                nc.scalar.copy(atk2[:], sq2[:])
            else:
                atk2 = None
            st["ak_n"], st["atk_n"] = ak2, atk2

        def chain_gps(c, lev):
            # accumulation half: g += (A^(2^lev))^T g
            st = G[c]
            ak2 = st["ak_n"]
            gps = pshi.tile([128, H], f32, tag="big")
            for bi in range(BPC):
                sl = slice(bi * 128, (bi + 1) * 128)
                nc.tensor.matmul(gps[:, sl], ak2[:, sl], st["g"][:, sl], start=True, stop=True)
            g_nxt = chain.tile([128, H], f16, tag="g")
            nc.vector.tensor_add(g_nxt[:], gps[:], st["g"][:])
            st["ak"], st["atk"], st["g"] = st["ak_n"], st["atk_n"], g_nxt

        def chain_level(c, lev):
            chain_sq(c, lev)
            chain_gps(c, lev)

        def xform(c):
            # X^T for pair (c-1, c): xts[:, bi-slice][a, b] = K_{c-1}[a]·K_c[b]
            st, stp = G[c], G[c - 1]
            xps = pshi.tile([128, H], f32, tag="big")
            for bi in range(BPC):
                sl = slice(bi * 128, (bi + 1) * 128)
                for hb in range(HB):
                    hsl = slice(bi * H + hb * 128, bi * H + (hb + 1) * 128)
                    nc.tensor.matmul(xps[:, sl], stp["kt16"][:, hsl],
                                     st["kt16"][:, hsl],
                                     start=(hb == 0), stop=(hb == HB - 1))
            # negated so the (negated-dl) cross product lands with + sign
            xts = xpool.tile([128, H], f16, tag="x")
            nc.scalar.activation(xts[:], xps[:], mybir.ActivationFunctionType.Copy,
                                 scale=-1.0)
            st["x"] = xts

        def state_u(c):
            # chunks 0/1 update M individually (startup); pairs start at (2,3)
            st = G[c]
            cross = (c % 2 == 1 and c >= 3)
            st["u"] = []
            for bi in range(BPC):
                if c == 0:
                    st["u"].append(st["k16"][:, bi * H:(bi + 1) * H])
                    continue
                ups = pshi.tile([128, H], f32, tag="big")
                for hb in range(HB):
                    hsl = slice(bi * H + hb * 128, bi * H + (hb + 1) * 128)
                    nc.tensor.matmul(ups[:], st["kt16"][:, hsl],
                                     mt16[bi][hb][:],
                                     start=(hb == 0),
                                     stop=(hb == HB - 1 and not cross))
                if cross:
                    sl = slice(bi * 128, (bi + 1) * 128)
                    nc.tensor.matmul(ups[:], st["x"][:, sl], G[c - 1]["dl"][bi][:],
                                     start=False, stop=True)
                # u_neg = r*ups - k (negated U; saves materializing -r)
                u_sb = upool.tile([128, H], f16, tag="u")
                nc.vector.scalar_tensor_tensor(u_sb[:], ups[:], st["r"][:, bi:bi + 1],
                                               st["k"][bi][:], mybir.AluOpType.mult,
                                               mybir.AluOpType.subtract)
                st["u"].append(u_sb[:])

        def state_delta(c):
            st = G[c]
            st["dl"] = []
            for bi in range(BPC):
                sl = slice(bi * 128, (bi + 1) * 128)
                dps = pshi.tile([128, H], f32, tag="big")
                nc.tensor.matmul(dps[:], st["g"][:, sl], st["u"][bi], start=True, stop=True)
                dl = dlpool.tile([128, H], f16, tag="dl")
                nc.scalar.copy(dl[:], dps[:])
                st["dl"].append(dl)

        def mupd(c, bis):
            # pair-end update: M^T += K_{c-1}^T Dl_{c-1} + K_c^T Dl_c
            # (bi 2,3 run at the start of the NEXT iteration to even PE load;
            #  some adds bounce PSUM->SBUF via DMA so Pool can do the add)
            st, stp = G[c], G[c - 1]
            for bi in bis:
                for jb in range(HB):
                    jsl = slice(bi * H + jb * 128, bi * H + (jb + 1) * 128)
                    mps = pshi.tile([128, H], f32, tag="big")
                    nc.tensor.matmul(mps[:], stp["k16"][:, jsl], stp["dl"][bi][:],
                                     start=True, stop=False)
                    nc.tensor.matmul(mps[:], st["k16"][:, jsl], st["dl"][bi][:],
                                     start=False, stop=True)
                    nc.vector.tensor_sub(mt16[bi][jb][:], mt16[bi][jb][:], mps[:])

        def mupd_single(c):
            # startup chunks 0 and 1: per-chunk M update (keeps PE busy early)
            st = G[c]
            for bi in range(BPC):
                for jb in range(HB):
                    jsl = slice(bi * H + jb * 128, bi * H + (jb + 1) * 128)
                    mps = pshi.tile([128, H], f32, tag="big")
                    nc.tensor.matmul(mps[:], st["k16"][:, jsl], st["dl"][bi][:],
                                     start=True, stop=True)
                    if c == 0:
                        nc.scalar.copy(mt16[bi][jb][:], mps[:])
                    else:
                        nc.vector.tensor_sub(mt16[bi][jb][:], mt16[bi][jb][:], mps[:])

        # ---- early prologue: read_proj weights + query + bias ----
        # (issued first: one of these DMAs expands to a long fine-grained
        # descriptor burst; at program start it overlaps the pipeline fill)
        # wtALL[:, ib*512 + op*128 + o] = W^T[ib*128 + i', op*128 + o]
        wtall = consts.tile([128, HB * H], f16, tag="wtall")
        for op in range(HB):
            wsb = wbuild.tile([128, H], f32, tag="wsb")
            nc.sync.dma_start(wsb[:], w_d[op * 128:(op + 1) * 128, :])
            wps = pshi.tile([128, H], f32, tag="big")
            for ib in range(HB):
                nc.tensor.transpose(wps[:, ib * 128:(ib + 1) * 128],
                                    wsb[:, ib * 128:(ib + 1) * 128], ident_f[:])
            nc.scalar.copy(
                wtall[:].rearrange("p (f o2 q) -> p f o2 q", f=HB, q=128)[:, :, op, :],
                wps[:])
        bias_all = consts.tile([BPC, H], f32, tag="biasall")
        for bi in range(BPC):
            nc.sync.dma_start(bias_all[bi:bi + 1, :], b_d[None, :])
        qs = []
        for bi in range(BPC):
            v4 = wbuild.tile([HB, 128], f32, tag="v4")
            nc.sync.dma_start(v4[:], hidden_d[bi, L - 1, :].rearrange("(f p) -> f p", p=128))
            tps = pslo.tile([128, HB], f32, tag="sm")
            nc.tensor.transpose(tps[:], v4[:], ident_f[:HB, :HB])
            q_t = consts.tile([128, HB], f16, tag=f"q{bi}", name=f"q{bi}")
            nc.scalar.copy(q_t[:], tps[:])
            qs.append(q_t)

        # ---- software-pipelined main loop ----
        # Issue order per iteration is tuned so each engine's in-order queue
        # services consumers before producers-for-later: PE never waits behind
        # unready work, Act chain copies aren't stuck behind squares, and the
        # k16->kt16 DMA for c+2 isn't stuck behind pair-end M adds.
        prep_load(0)
        prep_load(1)
        prep_load(2)
        prep_tr(0)
        prep_r(0)
        prep_tr(1)
        prep_r(1)
        aform(0)
        for lev in range(1, NLEV + 1):
            chain_level(0, lev)
        aform(1)
        for c in range(NCHUNK):
            nxt = c + 1 if c + 1 < NCHUNK else None
            nn = c + 2 if c + 2 < NCHUNK else None
            if c % 2 == 1 and c >= 3:
                xform(c)
            state_u(c)
            if nxt is not None:
                chain_sq(nxt, 1)
            state_delta(c)
            if nxt is not None:
                chain_gps(nxt, 1)
                chain_sq(nxt, 2)
            if c + 3 < NCHUNK:
                prep_load(c + 3)
            if c <= 1:
                mupd_single(c)
            elif c % 2 == 1:
                mupd(c, [0, 1, 2, 3])
            if nxt is not None:
                chain_gps(nxt, 2)
            if nn is not None:
                prep_tr(nn)
                prep_r(nn)
                aform(nn)
            prev = c - 3
            if prev in G:
                del G[prev]

        # ---- finale: ctx = M q (row form); out = ctx W^T + b ----
        # phase-major over bi so PE/Act/DVE overlap across batch elements;
        # out-proj packs all 4 bi into one lhsT per ib block (4 matmuls total)
        cpss, ctx_rows = [], []
        for bi in range(BPC):
            cps = pshi.tile([1, H], f32, tag="big")
            for jb in range(HB):
                nc.tensor.matmul(cps[:], qs[bi][:, jb:jb + 1], mt16[bi][jb][:],
                                 start=(jb == 0), stop=(jb == HB - 1))
            cpss.append(cps)
        for bi in range(BPC):
            ctx_row = small.tile([1, H], f16, tag=f"ctxrow{bi}")
            nc.scalar.copy(ctx_row[:], cpss[bi][:])
            ctx_rows.append(ctx_row)
        ctxT = small.tile([128, HB * BPC], f16, tag="ctxT")
        for bi in range(BPC):
            for ib in range(HB):
                tp2 = pslo.tile([128, 1], f16, tag="sm1")
                nc.tensor.transpose(tp2[:], ctx_rows[bi][:, ib * 128:(ib + 1) * 128],
                                    ident_h[:1, :1])
                nc.vector.tensor_copy(ctxT[:, ib * BPC + bi:ib * BPC + bi + 1], tp2[:])
        ops4 = pshi.tile([BPC, H], f32, tag="big")
        for ib in range(HB):
            nc.tensor.matmul(ops4[:], ctxT[:, ib * BPC:(ib + 1) * BPC],
                             wtall[:, ib * H:(ib + 1) * H],
                             start=(ib == 0), stop=(ib == HB - 1))
        out_all = small.tile([BPC, H], f32, tag="outall")
        nc.vector.tensor_add(out_all[:], ops4[:], bias_all[:])
        nc.sync.dma_start(out_d[:, :], out_all[:])

    if legalize:
        _legalize_waits(nc)
    return nc


def _legalize_waits(nc, max_waits=1):
    """This toolchain's walrus encodes at most one semaphore wait per
    instruction. Hoist extra waits onto standalone EventSemaphore
    instructions on the same engine queue, immediately before the owner."""
    import json as _json
    m = _json.loads(bytes(nc.to_json_bytes()))
    n_fix = 0
    for fn in m["functions"]:
        for blk in fn["blocks"]:
            out = []
            for ins in blk.get("instructions", []):
                si = ins.get("sync_info") or {}
                waits = si.get("on_wait") or []
                if len(waits) > max_waits and ins.get("opcode") != "EventSemaphore":
                    extra, keep = waits[:-max_waits], waits[-max_waits:]
                    for i, w in enumerate(extra):
                        out.append({
                            "name": f"{ins['name']}-w{i}",
                            "engine": ins["engine"],
                            "opcode": "EventSemaphore",
                            "ins": [], "outs": [],
                            "sync_info": {"on_wait": [w], "on_update": []},
                        })
                    si["on_wait"] = keep
                    ins["sync_info"] = si
                    n_fix += 1
                out.append(ins)
            blk["instructions"] = out
    nc.m = mybir.module_from_json_bytes(_json.dumps(m).encode())
    return n_fix


def kernel(hidden: np.ndarray, W: np.ndarray, b: np.ndarray) -> np.ndarray:
    if "nc" not in _cached:
        _cached["nc"] = _build_program()
    nc = _cached["nc"]

    hidden = np.ascontiguousarray(hidden, dtype=np.float32)
    W = np.ascontiguousarray(W, dtype=np.float32)
    b = np.ascontiguousarray(b, dtype=np.float32)

    in_maps = []
    for ci in range(NCORES):
        in_maps.append({
            "hidden": hidden[ci * BPC:(ci + 1) * BPC],
            "W": W,
            "bvec": b,
        })
    res = run_bass_kernel_spmd(nc, in_maps, core_ids=list(range(NCORES)))
    _cached["last_results"] = res
    out = np.concatenate([res.results[ci]["out"] for ci in range(NCORES)], axis=0)
    return out.astype(np.float32)


if __name__ == "__main__":
    rng = np.random.default_rng(0)
    h = rng.standard_normal((B, L, H), dtype=np.float32)
    w = rng.standard_normal((H, H), dtype=np.float32) * (1.0 / np.sqrt(H))
    bb = np.zeros((H,), np.float32)
    o = kernel(h, w, bb)
    print(o.shape, o.dtype)


# revision 11
# speedup vs baseline: 1.5719x; 1.0255x over previous
"""DeltaRule memory scan kernel for Trainium2, 8 NeuronCores, data-parallel over batch.

Reference semantics (per batch element, H=512, L=2048):
    M_0 = 0  [H,H]
    for t in 0..L-2:   k = hidden[t]
        d = k.k + eps; delta = k - (M k)/d; M += outer(delta, k)
    out = (M @ hidden[L-1]) @ W.T + b

Implementation: chunked delta rule (UT transform), chunk C=128, float16.
Per chunk with keys K [C,H], r = 1/(rowsum(K^2)+eps):
    A  = strict_tril(diag(r) K K^T)            [C,C]
    T  = (I+A)^{-1} ~= (I-A)(I+A^2)(I+A^4)    (A nilpotent; exact through A^7)
    U  = K - diag(r) (K M^T + X Dl_prev)       [C,H]  (X-term: intra-pair cross)
    Dl = T U                                    [C,H]
    M^T += K_c0^T Dl_c0 + K_c1^T Dl_c1          (pair-accumulated in PSUM)
All matmuls f16 (1 cy/row); M^T state kept in f16 only (validated: relerr ~1.5e-3).
K^T and A^T produced by XBAR DMA transposes (no PE transposes in steady state).
Elementwise work spread across DVE / Act / Pool engines; 4 batch elements per
core with chunk phases interleaved for PE-queue continuity.
"""
import sys
import numpy as np
from contextlib import ExitStack

sys.path.insert(0, "/opt/trn_rl_repo")

import concourse.bass as bass
import concourse.mybir as mybir
from concourse import tile
from concourse.bass_utils import run_bass_kernel_spmd
from concourse.masks import make_identity, make_lower_triangular

B, L, H = 32, 2048, 512
NCORES = 8
BPC = B // NCORES          # 4 batch elements per core
C = 128                    # chunk size
T = L - 1                  # 2047 scan steps
NCHUNK = (T + C - 1) // C  # 16 (last chunk has 127 valid rows)
NLEV = 2                   # Neumann levels -> exact through A^7 (validated)
EPS = 1e-6
HB = H // 128              # 4 h-blocks

f32 = mybir.dt.float32
f16 = mybir.dt.float16

_cached = {}

# engine split for the 16 pair-end M updates (V=vector/DVE, P=pool)
_MUPD_ENG = "PVPPVPVPPVPPVPVP"


def _build_program(legalize=True):
    nc = bass.Bass(target_bir_lowering=False, debug=False)

    hidden_d = nc.declare_dram_parameter("hidden", [BPC, L, H], f32, isOutput=False)
    w_d = nc.declare_dram_parameter("W", [H, H], f32, isOutput=False)
    b_d = nc.declare_dram_parameter("bvec", [H], f32, isOutput=False)
    out_d = nc.declare_dram_parameter("out", [BPC, H], f32, isOutput=True)

    with tile.TileContext(nc) as tc, ExitStack() as ctx:
        consts = ctx.enter_context(tc.tile_pool(name="consts", bufs=1))
        wbuild = ctx.enter_context(tc.tile_pool(name="wbuild", bufs=2))
        mtpool = ctx.enter_context(tc.tile_pool(name="mt", bufs=1))
        kpool = ctx.enter_context(tc.tile_pool(name="k", bufs=16))
        k16pool = ctx.enter_context(tc.tile_pool(name="k16", bufs=5))
        chain = ctx.enter_context(tc.tile_pool(name="chain", bufs=6))
        upool = ctx.enter_context(tc.tile_pool(name="u", bufs=6))
        dlpool = ctx.enter_context(tc.tile_pool(name="dl", bufs=10))
        xpool = ctx.enter_context(tc.tile_pool(name="x", bufs=2))
        small = ctx.enter_context(tc.tile_pool(name="small", bufs=4))
        pslo = ctx.enter_context(tc.tile_pool(name="pslo", bufs=1, space="PSUM"))
        pshi = ctx.enter_context(tc.tile_pool(name="pshi", bufs=6, space="PSUM"))

        # ---- constants needed by the main loop ----
        ident_f = consts.tile([128, 128], f32, tag="identf")
        make_identity(nc, ident_f[:])
        ident_h = consts.tile([128, 128], f16, tag="identh")
        make_identity(nc, ident_h[:])
        identp = consts.tile([128, H], f16, tag="identp")
        for bi in range(BPC):
            nc.vector.tensor_copy(identp[:, bi * 128:(bi + 1) * 128], ident_h[:])
        smask = consts.tile([128, 128], f32, tag="smask")
        make_lower_triangular(nc, smask[:], val=1.0, diag=False)

        # persistent state M^T (f16 only), per (bi, jb)
        mt16 = [[mtpool.tile([128, H], f16, tag=f"mt{bi}_{jb}", name=f"mt{bi}_{jb}")
                 for jb in range(HB)] for bi in range(BPC)]

        # pre-zeroed key tiles for the final (127-row) chunk: zeroing them in
        # the prologue keeps the memset out of the busy mid-scan DVE queue
        k15 = [consts.tile([128, H], f32, tag=f"k15_{bi}", name=f"k15_{bi}")
               for bi in range(BPC)]
        for bi in range(BPC):
            nc.vector.memset(k15[bi][:], 0.0)

        G = {}

        def prep_load(c):
            t0 = c * C
            nrows = min(C, T - t0)
            st = {"k": []}
            for bi in range(BPC):
                if nrows < C:
                    k_t = k15[bi]
                    nc.sync.dma_start(k_t[:nrows, :], hidden_d[bi, t0:t0 + nrows, :])
                else:
                    k_t = kpool.tile([128, H], f32, tag="K", name=f"k{c}_{bi}")
                    nc.sync.dma_start(k_t[:], hidden_d[bi, t0:t0 + C, :])
                st["k"].append(k_t)
            # packed row-major f16 keys [128, (bi h)] (Pool: SBUF-only casts)
            k16a = k16pool.tile([128, BPC * H], f16, tag="k16", name=f"k16_{c}")
            for bi in range(BPC):
                nc.gpsimd.tensor_copy(k16a[:, bi * H:(bi + 1) * H], st["k"][bi][:])
            st["k16"] = k16a
            G[c] = st

        def prep_tr(c):
            # K^T blocks via f16 PE transposes of the packed keys (XBAR DMA on
            # the single SP queue head-of-line-blocks the whole pipeline on hw)
            st = G[c]
            kt16a = k16pool.tile([128, BPC * H], f16, tag="kt16", name=f"kt16_{c}")
            for bi in range(BPC):
                ktps = pshi.tile([128, H], f16, tag="big")
                for hb in range(HB):
                    hsl = slice(bi * H + hb * 128, bi * H + (hb + 1) * 128)
                    nc.tensor.transpose(ktps[:, hb * 128:(hb + 1) * 128],
                                        st["k16"][:, hsl], ident_h[:])
                if bi < 2:
                    nc.scalar.copy(kt16a[:, bi * H:(bi + 1) * H], ktps[:])
                else:
                    nc.vector.tensor_copy(kt16a[:, bi * H:(bi + 1) * H], ktps[:])
            st["kt16"] = kt16a

        def prep_r(c):
            st = G[c]
            dall = small.tile([128, BPC], f32, tag="dall")
            for bi in range(BPC):
                scr = small.tile([128, H], f16, tag="scr")
                nc.scalar.activation(scr[:], st["k"][bi][:],
                                     mybir.ActivationFunctionType.Square,
                                     accum_out=dall[:, bi:bi + 1])
            rall = small.tile([128, BPC], f32, tag="rall")
            if c == NCHUNK - 1:
                # only the zero-padded final chunk needs the EPS guard
                nc.vector.tensor_scalar_add(dall[:], dall[:], EPS)
            nc.vector.reciprocal(rall[:], dall[:])
            st["r"] = rall

        def aform(c):
            st = G[c]
            kt = st["kt16"]
            a_ps = pshi.tile([128, H], f32, tag="big")
            for bi in range(BPC):
                sl = slice(bi * 128, (bi + 1) * 128)
                for hb in range(HB):
                    hsl = slice(bi * H + hb * 128, bi * H + (hb + 1) * 128)
                    nc.tensor.matmul(a_ps[:, sl], kt[:, hsl], kt[:, hsl],
                                     start=(hb == 0), stop=(hb == HB - 1))
            a_all = chain.tile([128, H], f16, tag="ak")
            for bi in range(BPC):
                sl = slice(bi * 128, (bi + 1) * 128)
                nc.vector.scalar_tensor_tensor(a_all[:, sl], a_ps[:, sl],
                                               st["r"][:, bi:bi + 1], smask[:],
                                               mybir.AluOpType.mult,
                                               mybir.AluOpType.mult)
            at_ps = pshi.tile([128, H], f16, tag="big")
            for bi in range(BPC):
                sl = slice(bi * 128, (bi + 1) * 128)
                nc.tensor.transpose(at_ps[:, sl], a_all[:, sl], ident_h[:])
            at_all = chain.tile([128, H], f16, tag="atk")
            nc.scalar.copy(at_all[:], at_ps[:])
            g0 = chain.tile([128, H], f16, tag="g")
            nc.vector.tensor_sub(g0[:], identp[:], at_all[:])
            st["ak"], st["atk"], st["g"] = a_all, at_all, g0

        def chain_sq(c, lev):
            # squaring half: A^(2^lev); its transpose via XBAR DMA (free-ish)
            st = G[c]
            ak, atk = st["ak"], st["atk"]
            sq1 = pshi.tile([128, H], f32, tag="big")
            for bi in range(BPC):
                sl = slice(bi * 128, (bi + 1) * 128)
                nc.tensor.matmul(sq1[:, sl], atk[:, sl], ak[:, sl], start=True, stop=True)
            ak2 = chain.tile([128, H], f16, tag="ak")
            nc.scalar.copy(ak2[:], sq1[:])
            if lev < NLEV:
                sq2 = pshi.tile([128, H], f32, tag="big")
                for bi in range(BPC):
                    sl = slice(bi * 128, (bi + 1) * 128)
                    nc.tensor.matmul(sq2[:, sl], ak[:, sl], atk[:, sl], start=True, stop=True)
                atk2 = chain.tile([128, H], f16, tag="atk")
                nc.scalar.copy(atk2[:], sq2[:])
            else:
                atk2 = None
            st["ak_n"], st["atk_n"] = ak2, atk2

        def chain_gps(c, lev):
            # accumulation half: g += (A^(2^lev))^T g
            st = G[c]
            ak2 = st["ak_n"]
            gps = pshi.tile([128, H], f32, tag="big")
            for bi in range(BPC):
                sl = slice(bi * 128, (bi + 1) * 128)
                nc.tensor.matmul(gps[:, sl], ak2[:, sl], st["g"][:, sl], start=True, stop=True)
            g_nxt = chain.tile([128, H], f16, tag="g")
            nc.vector.tensor_add(g_nxt[:], gps[:], st["g"][:])
            st["ak"], st["atk"], st["g"] = st["ak_n"], st["atk_n"], g_nxt

        def chain_level(c, lev):
            chain_sq(c, lev)
            chain_gps(c, lev)

        def xform(c):
            # X^T for pair (c-1, c): xts[:, bi-slice][a, b] = K_{c-1}[a]·K_c[b]
            st, stp = G[c], G[c - 1]
            xps = pshi.tile([128, H], f32, tag="big")
            for bi in range(BPC):
                sl = slice(bi * 128, (bi + 1) * 128)
                for hb in range(HB):
                    hsl = slice(bi * H + hb * 128, bi * H + (hb + 1) * 128)
                    nc.tensor.matmul(xps[:, sl], stp["kt16"][:, hsl],
                                     st["kt16"][:, hsl],
                                     start=(hb == 0), stop=(hb == HB - 1))
            # negated so the (negated-dl) cross product lands with + sign
            xts = xpool.tile([128, H], f16, tag="x")
            nc.scalar.activation(xts[:], xps[:], mybir.ActivationFunctionType.Copy,
                                 scale=-1.0)
            st["x"] = xts

        def state_u(c):
            # chunks 0/1 update M individually (startup); pairs start at (2,3)
            st = G[c]
            cross = (c % 2 == 1 and c >= 3)
            st["u"] = []
            for bi in range(BPC):
                if c == 0:
                    st["u"].append(st["k16"][:, bi * H:(bi + 1) * H])
                    continue
                ups = pshi.tile([128, H], f32, tag="big")
                for hb in range(HB):
                    hsl = slice(bi * H + hb * 128, bi * H + (hb + 1) * 128)
                    nc.tensor.matmul(ups[:], st["kt16"][:, hsl],
                                     mt16[bi][hb][:],
                                     start=(hb == 0),
                                     stop=(hb == HB - 1 and not cross))
                if cross:
                    sl = slice(bi * 128, (bi + 1) * 128)
                    nc.tensor.matmul(ups[:], st["x"][:, sl], G[c - 1]["dl"][bi][:],
                                     start=False, stop=True)
                # u_neg = r*ups - k (negated U; saves materializing -r)
                u_sb = upool.tile([128, H], f16, tag="u")
                nc.vector.scalar_tensor_tensor(u_sb[:], ups[:], st["r"][:, bi:bi + 1],
                                               st["k"][bi][:], mybir.AluOpType.mult,
                                               mybir.AluOpType.subtract)
                st["u"].append(u_sb[:])

        def state_delta(c):
            st = G[c]
            st["dl"] = []
            for bi in range(BPC):
                sl = slice(bi * 128, (bi + 1) * 128)
                dps = pshi.tile([128, H], f32, tag="big")
                nc.tensor.matmul(dps[:], st["g"][:, sl], st["u"][bi], start=True, stop=True)
                dl = dlpool.tile([128, H], f16, tag="dl")
                nc.scalar.copy(dl[:], dps[:])
                st["dl"].append(dl)

        def mupd(c, bis):
            # pair-end update: M^T += K_{c-1}^T Dl_{c-1} + K_c^T Dl_c
            # (bi 2,3 run at the start of the NEXT iteration to even PE load;
            #  some adds bounce PSUM->SBUF via DMA so Pool can do the add)
            st, stp = G[c], G[c - 1]
            for bi in bis:
                for jb in range(HB):
                    jsl = slice(bi * H + jb * 128, bi * H + (jb + 1) * 128)
                    mps = pshi.tile([128, H], f32, tag="big")
                    nc.tensor.matmul(mps[:], stp["k16"][:, jsl], stp["dl"][bi][:],
                                     start=True, stop=False)
                    nc.tensor.matmul(mps[:], st["k16"][:, jsl], st["dl"][bi][:],
                                     start=False, stop=True)
                    nc.vector.tensor_sub(mt16[bi][jb][:], mt16[bi][jb][:], mps[:])

        def mupd_single(c):
            # startup chunks 0 and 1: per-chunk M update (keeps PE busy early)
            st = G[c]
            for bi in range(BPC):
                for jb in range(HB):
                    jsl = slice(bi * H + jb * 128, bi * H + (jb + 1) * 128)
                    mps = pshi.tile([128, H], f32, tag="big")
                    nc.tensor.matmul(mps[:], st["k16"][:, jsl], st["dl"][bi][:],
                                     start=True, stop=True)
                    if c == 0:
                        nc.scalar.copy(mt16[bi][jb][:], mps[:])
                    else:
                        nc.vector.tensor_sub(mt16[bi][jb][:], mt16[bi][jb][:], mps[:])

        # ---- early prologue: read_proj weights + query + bias ----
        # (issued first: one of these DMAs expands to a long fine-grained
        # descriptor burst; at program start it overlaps the pipeline fill)
        # wtALL[:, ib*512 + op*128 + o] = W^T[ib*128 + i', op*128 + o]
        wtall = consts.tile([128, HB * H], f16, tag="wtall")
        for op in range(HB):
            wsb = wbuild.tile([128, H], f32, tag="wsb")
            nc.sync.dma_start(wsb[:], w_d[op * 128:(op + 1) * 128, :])
            wps = pshi.tile([128, H], f32, tag="big")
            for ib in range(HB):
                nc.tensor.transpose(wps[:, ib * 128:(ib + 1) * 128],
                                    wsb[:, ib * 128:(ib + 1) * 128], ident_f[:])
            nc.scalar.copy(
                wtall[:].rearrange("p (f o2 q) -> p f o2 q", f=HB, q=128)[:, :, op, :],
                wps[:])
        bias_all = consts.tile([BPC, H], f32, tag="biasall")
        for bi in range(BPC):
            nc.sync.dma_start(bias_all[bi:bi + 1, :], b_d[None, :])
        qs = []
        for bi in range(BPC):
            v4 = wbuild.tile([HB, 128], f32, tag="v4")
            nc.sync.dma_start(v4[:], hidden_d[bi, L - 1, :].rearrange("(f p) -> f p", p=128))
            tps = pslo.tile([128, HB], f32, tag="sm")
            nc.tensor.transpose(tps[:], v4[:], ident_f[:HB, :HB])
            q_t = consts.tile([128, HB], f16, tag=f"q{bi}", name=f"q{bi}")
            nc.scalar.copy(q_t[:], tps[:])
            qs.append(q_t)

        # ---- software-pipelined main loop ----
        # Issue order per iteration is tuned so each engine's in-order queue
        # services consumers before producers-for-later: PE never waits behind
        # unready work, Act chain copies aren't stuck behind squares, and the
        # k16->kt16 DMA for c+2 isn't stuck behind pair-end M adds.
        prep_load(0)
        prep_load(1)
        prep_load(2)
        prep_tr(0)
        prep_r(0)
        prep_tr(1)
        prep_r(1)
        aform(0)
        for lev in range(1, NLEV + 1):
            chain_level(0, lev)
        aform(1)
        for c in range(NCHUNK):
            nxt = c + 1 if c + 1 < NCHUNK else None
            nn = c + 2 if c + 2 < NCHUNK else None
            if c % 2 == 1 and c >= 3:
                xform(c)
            state_u(c)
            if nxt is not None:
                chain_sq(nxt, 1)
            state_delta(c)
            if nxt is not None:
                chain_gps(nxt, 1)
                chain_sq(nxt, 2)
            if nn is not None:
                prep_tr(nn)
            if c + 3 < NCHUNK:
                prep_load(c + 3)
            if c <= 1:
                mupd_single(c)
            elif c % 2 == 1:
                mupd(c, [0, 1, 2, 3])
            if nxt is not None:
                chain_gps(nxt, 2)
            if nn is not None:
                prep_r(nn)
                aform(nn)
            prev = c - 3
            if prev in G:
                del G[prev]

        # ---- finale: ctx = M q (row form); out = ctx W^T + b ----
        # phase-major over bi so PE/Act/DVE overlap across batch elements;
        # out-proj packs all 4 bi into one lhsT per ib block (4 matmuls total)
        cpss, ctx_rows = [], []
        for bi in range(BPC):
            cps = pshi.tile([1, H], f32, tag="big")
            for jb in range(HB):
                nc.tensor.matmul(cps[:], qs[bi][:, jb:jb + 1], mt16[bi][jb][:],
                                 start=(jb == 0), stop=(jb == HB - 1))
            cpss.append(cps)
        for bi in range(BPC):
            ctx_row = small.tile([1, H], f16, tag=f"ctxrow{bi}")
            nc.scalar.copy(ctx_row[:], cpss[bi][:])
            ctx_rows.append(ctx_row)
        ctxT = small.tile([128, HB * BPC], f16, tag="ctxT")
        for bi in range(BPC):
            for ib in range(HB):
                tp2 = pslo.tile([128, 1], f16, tag="sm1")
                nc.tensor.transpose(tp2[:], ctx_rows[bi][:, ib * 128:(ib + 1) * 128],
                                    ident_h[:1, :1])
                nc.vector.tensor_copy(ctxT[:, ib * BPC + bi:ib * BPC + bi + 1], tp2[:])
        ops4 = pshi.tile([BPC, H], f32, tag="big")
        for ib in range(HB):
            nc.tensor.matmul(ops4[:], ctxT[:, ib * BPC:(ib + 1) * BPC],
                             wtall[:, ib * H:(ib + 1) * H],
                             start=(ib == 0), stop=(ib == HB - 1))
        out_all = small.tile([BPC, H], f32, tag="outall")
        nc.vector.tensor_add(out_all[:], ops4[:], bias_all[:])
        nc.sync.dma_start(out_d[:, :], out_all[:])

    if legalize:
        _legalize_waits(nc)
    return nc


def _legalize_waits(nc, max_waits=1):
    """This toolchain's walrus encodes at most one semaphore wait per
    instruction. Hoist extra waits onto standalone EventSemaphore
    instructions on the same engine queue, immediately before the owner."""
    import json as _json
    m = _json.loads(bytes(nc.to_json_bytes()))
    n_fix = 0
    for fn in m["functions"]:
        for blk in fn["blocks"]:
            out = []
            for ins in blk.get("instructions", []):
                si = ins.get("sync_info") or {}
                waits = si.get("on_wait") or []
                if len(waits) > max_waits and ins.get("opcode") != "EventSemaphore":
                    extra, keep = waits[:-max_waits], waits[-max_waits:]
                    for i, w in enumerate(extra):
                        out.append({
                            "name": f"{ins['name']}-w{i}",
                            "engine": ins["engine"],
                            "opcode": "EventSemaphore",
                            "ins": [], "outs": [],
                            "sync_info": {"on_wait": [w], "on_update": []},
                        })
                    si["on_wait"] = keep
                    ins["sync_info"] = si
                    n_fix += 1
                out.append(ins)
            blk["instructions"] = out
    nc.m = mybir.module_from_json_bytes(_json.dumps(m).encode())
    return n_fix


def kernel(hidden: np.ndarray, W: np.ndarray, b: np.ndarray) -> np.ndarray:
    if "nc" not in _cached:
        _cached["nc"] = _build_program()
    nc = _cached["nc"]

    hidden = np.ascontiguousarray(hidden, dtype=np.float32)
    W = np.ascontiguousarray(W, dtype=np.float32)
    b = np.ascontiguousarray(b, dtype=np.float32)

    in_maps = []
    for ci in range(NCORES):
        in_maps.append({
            "hidden": hidden[ci * BPC:(ci + 1) * BPC],
            "W": W,
            "bvec": b,
        })
    res = run_bass_kernel_spmd(nc, in_maps, core_ids=list(range(NCORES)))
    _cached["last_results"] = res
    out = np.concatenate([res.results[ci]["out"] for ci in range(NCORES)], axis=0)
    return out.astype(np.float32)


if __name__ == "__main__":
    rng = np.random.default_rng(0)
    h = rng.standard_normal((B, L, H), dtype=np.float32)
    w = rng.standard_normal((H, H), dtype=np.float32) * (1.0 / np.sqrt(H))
    bb = np.zeros((H,), np.float32)
    o = kernel(h, w, bb)
    print(o.shape, o.dtype)


# revision 12
# speedup vs baseline: 1.5796x; 1.0049x over previous
"""DeltaRule memory scan kernel for Trainium2, 8 NeuronCores, data-parallel over batch.

Reference semantics (per batch element, H=512, L=2048):
    M_0 = 0  [H,H]
    for t in 0..L-2:   k = hidden[t]
        d = k.k + eps; delta = k - (M k)/d; M += outer(delta, k)
    out = (M @ hidden[L-1]) @ W.T + b

Implementation: chunked delta rule (UT transform), chunk C=128, float16.
Per chunk with keys K [C,H], r = 1/rowsum(K^2):
    A  = strict_tril(diag(r) K K^T)            [C,C]
    T  = (I+A)^{-1} ~= (I-A)(I+A^2)(I+A^4)    (A nilpotent; exact through A^7)
    U- = diag(r)(K M^T + X Dl-_prev) - K       (negated U; X: intra-pair cross)
    Dl- = T U-                                  [C,H]
    M^T -= K_c0^T Dl-_c0 + K_c1^T Dl-_c1       (pair-accumulated in PSUM)
All matmuls f16 (1 cy/row); M^T state kept in f16 only (validated: relerr ~7e-3
on hw vs 2e-2 gate). Hardware-calibrated engine budget per chunk: PE ~15.7us
span share (matmuls + all transposes - XBAR DMA transposes head-of-line-block
the single SP DMA queue, so K^T/A^T/W^T use PE transposes), DVE ~11us (M
subtracts, U, A-scale, g-adds), Act ~12us (squares, PSUM->SBUF casts), Pool
~7.5us (SBUF-only f32->f16 key casts; GpSimd has no PSUM port and runs ~3x
slower than nominal). Loads prefetch 3 chunks ahead; chunk phases interleave
across the 4 batch elements per core to keep the PE queue fed.
"""
import sys
import numpy as np
from contextlib import ExitStack

sys.path.insert(0, "/opt/trn_rl_repo")

import concourse.bass as bass
import concourse.mybir as mybir
from concourse import tile
from concourse.bass_utils import run_bass_kernel_spmd
from concourse.masks import make_identity, make_lower_triangular

B, L, H = 32, 2048, 512
NCORES = 8
BPC = B // NCORES          # 4 batch elements per core
C = 128                    # chunk size
T = L - 1                  # 2047 scan steps
NCHUNK = (T + C - 1) // C  # 16 (last chunk has 127 valid rows)
NLEV = 2                   # Neumann levels -> exact through A^7 (validated)
EPS = 1e-6
HB = H // 128              # 4 h-blocks

f32 = mybir.dt.float32
f16 = mybir.dt.float16

_cached = {}

# engine split for the 16 pair-end M updates (V=vector/DVE, P=pool)
_MUPD_ENG = "PVPPVPVPPVPPVPVP"


def _build_program(legalize=True):
    nc = bass.Bass(target_bir_lowering=False, debug=False)

    hidden_d = nc.declare_dram_parameter("hidden", [BPC, L, H], f32, isOutput=False)
    w_d = nc.declare_dram_parameter("W", [H, H], f32, isOutput=False)
    b_d = nc.declare_dram_parameter("bvec", [H], f32, isOutput=False)
    out_d = nc.declare_dram_parameter("out", [BPC, H], f32, isOutput=True)

    with tile.TileContext(nc) as tc, ExitStack() as ctx:
        consts = ctx.enter_context(tc.tile_pool(name="consts", bufs=1))
        wbuild = ctx.enter_context(tc.tile_pool(name="wbuild", bufs=2))
        mtpool = ctx.enter_context(tc.tile_pool(name="mt", bufs=1))
        kpool = ctx.enter_context(tc.tile_pool(name="k", bufs=16))
        k16pool = ctx.enter_context(tc.tile_pool(name="k16", bufs=5))
        chain = ctx.enter_context(tc.tile_pool(name="chain", bufs=6))
        upool = ctx.enter_context(tc.tile_pool(name="u", bufs=6))
        dlpool = ctx.enter_context(tc.tile_pool(name="dl", bufs=10))
        xpool = ctx.enter_context(tc.tile_pool(name="x", bufs=2))
        small = ctx.enter_context(tc.tile_pool(name="small", bufs=4))
        pslo = ctx.enter_context(tc.tile_pool(name="pslo", bufs=1, space="PSUM"))
        pshi = ctx.enter_context(tc.tile_pool(name="pshi", bufs=6, space="PSUM"))

        # ---- constants needed by the main loop ----
        ident_f = consts.tile([128, 128], f32, tag="identf")
        make_identity(nc, ident_f[:])
        ident_h = consts.tile([128, 128], f16, tag="identh")
        make_identity(nc, ident_h[:])
        identp = consts.tile([128, H], f16, tag="identp")
        for bi in range(BPC):
            nc.vector.tensor_copy(identp[:, bi * 128:(bi + 1) * 128], ident_h[:])
        smask = consts.tile([128, 128], f32, tag="smask")
        make_lower_triangular(nc, smask[:], val=1.0, diag=False)

        # persistent state M^T (f16 only), per (bi, jb)
        mt16 = [[mtpool.tile([128, H], f16, tag=f"mt{bi}_{jb}", name=f"mt{bi}_{jb}")
                 for jb in range(HB)] for bi in range(BPC)]

        # pre-zeroed key tiles for the final (127-row) chunk: zeroing them in
        # the prologue keeps the memset out of the busy mid-scan DVE queue
        k15 = [consts.tile([128, H], f32, tag=f"k15_{bi}", name=f"k15_{bi}")
               for bi in range(BPC)]
        for bi in range(BPC):
            nc.vector.memset(k15[bi][:], 0.0)

        G = {}

        def prep_load(c):
            t0 = c * C
            nrows = min(C, T - t0)
            st = {"k": []}
            for bi in range(BPC):
                if nrows < C:
                    k_t = k15[bi]
                    nc.sync.dma_start(k_t[:nrows, :], hidden_d[bi, t0:t0 + nrows, :])
                else:
                    k_t = kpool.tile([128, H], f32, tag="K", name=f"k{c}_{bi}")
                    nc.sync.dma_start(k_t[:], hidden_d[bi, t0:t0 + C, :])
                st["k"].append(k_t)
            # packed row-major f16 keys [128, (bi h)] (Pool: SBUF-only casts)
            k16a = k16pool.tile([128, BPC * H], f16, tag="k16", name=f"k16_{c}")
            for bi in range(BPC):
                nc.gpsimd.tensor_copy(k16a[:, bi * H:(bi + 1) * H], st["k"][bi][:])
            st["k16"] = k16a
            G[c] = st

        def prep_tr(c):
            # K^T blocks via f16 PE transposes of the packed keys (XBAR DMA on
            # the single SP queue head-of-line-blocks the whole pipeline on hw)
            st = G[c]
            kt16a = k16pool.tile([128, BPC * H], f16, tag="kt16", name=f"kt16_{c}")
            for bi in range(BPC):
                ktps = pshi.tile([128, H], f16, tag="big")
                for hb in range(HB):
                    hsl = slice(bi * H + hb * 128, bi * H + (hb + 1) * 128)
                    nc.tensor.transpose(ktps[:, hb * 128:(hb + 1) * 128],
                                        st["k16"][:, hsl], ident_h[:])
                if bi < 2:
                    nc.scalar.copy(kt16a[:, bi * H:(bi + 1) * H], ktps[:])
                else:
                    nc.vector.tensor_copy(kt16a[:, bi * H:(bi + 1) * H], ktps[:])
            st["kt16"] = kt16a

        def prep_r(c):
            st = G[c]
            dall = small.tile([128, BPC], f32, tag="dall")
            for bi in range(BPC):
                scr = small.tile([128, H], f16, tag="scr")
                nc.scalar.activation(scr[:], st["k"][bi][:],
                                     mybir.ActivationFunctionType.Square,
                                     accum_out=dall[:, bi:bi + 1])
            rall = small.tile([128, BPC], f32, tag="rall")
            if c == NCHUNK - 1:
                # only the zero-padded final chunk needs the EPS guard
                nc.vector.tensor_scalar_add(dall[:], dall[:], EPS)
            nc.vector.reciprocal(rall[:], dall[:])
            st["r"] = rall

        def aform(c):
            st = G[c]
            kt = st["kt16"]
            a_ps = pshi.tile([128, H], f32, tag="big")
            for bi in range(BPC):
                sl = slice(bi * 128, (bi + 1) * 128)
                for hb in range(HB):
                    hsl = slice(bi * H + hb * 128, bi * H + (hb + 1) * 128)
                    nc.tensor.matmul(a_ps[:, sl], kt[:, hsl], kt[:, hsl],
                                     start=(hb == 0), stop=(hb == HB - 1))
            a_all = chain.tile([128, H], f16, tag="ak")
            for bi in range(BPC):
                sl = slice(bi * 128, (bi + 1) * 128)
                nc.vector.scalar_tensor_tensor(a_all[:, sl], a_ps[:, sl],
                                               st["r"][:, bi:bi + 1], smask[:],
                                               mybir.AluOpType.mult,
                                               mybir.AluOpType.mult)
            at_ps = pshi.tile([128, H], f16, tag="big")
            for bi in range(BPC):
                sl = slice(bi * 128, (bi + 1) * 128)
                nc.tensor.transpose(at_ps[:, sl], a_all[:, sl], ident_h[:])
            at_all = chain.tile([128, H], f16, tag="atk")
            nc.scalar.copy(at_all[:], at_ps[:])
            g0 = chain.tile([128, H], f16, tag="g")
            nc.vector.tensor_sub(g0[:], identp[:], at_all[:])
            st["ak"], st["atk"], st["g"] = a_all, at_all, g0

        def chain_sq(c, lev):
            # squaring half: A^(2^lev); its transpose via XBAR DMA (free-ish)
            st = G[c]
            ak, atk = st["ak"], st["atk"]
            sq1 = pshi.tile([128, H], f32, tag="big")
            for bi in range(BPC):
                sl = slice(bi * 128, (bi + 1) * 128)
                nc.tensor.matmul(sq1[:, sl], atk[:, sl], ak[:, sl], start=True, stop=True)
            ak2 = chain.tile([128, H], f16, tag="ak")
            nc.scalar.copy(ak2[:], sq1[:])
            if lev < NLEV:
                sq2 = pshi.tile([128, H], f32, tag="big")
                for bi in range(BPC):
                    sl = slice(bi * 128, (bi + 1) * 128)
                    nc.tensor.matmul(sq2[:, sl], ak[:, sl], atk[:, sl], start=True, stop=True)
                atk2 = chain.tile([128, H], f16, tag="atk")
                nc.scalar.copy(atk2[:], sq2[:])
            else:
                atk2 = None
            st["ak_n"], st["atk_n"] = ak2, atk2

        def chain_gps(c, lev):
            # accumulation half: g += (A^(2^lev))^T g
            st = G[c]
            ak2 = st["ak_n"]
            gps = pshi.tile([128, H], f32, tag="big")
            for bi in range(BPC):
                sl = slice(bi * 128, (bi + 1) * 128)
                nc.tensor.matmul(gps[:, sl], ak2[:, sl], st["g"][:, sl], start=True, stop=True)
            g_nxt = chain.tile([128, H], f16, tag="g")
            nc.vector.tensor_add(g_nxt[:], gps[:], st["g"][:])
            st["ak"], st["atk"], st["g"] = st["ak_n"], st["atk_n"], g_nxt

        def chain_level(c, lev):
            chain_sq(c, lev)
            chain_gps(c, lev)

        def xform(c):
            # X^T for pair (c-1, c): xts[:, bi-slice][a, b] = K_{c-1}[a]·K_c[b]
            st, stp = G[c], G[c - 1]
            xps = pshi.tile([128, H], f32, tag="big")
            for bi in range(BPC):
                sl = slice(bi * 128, (bi + 1) * 128)
                for hb in range(HB):
                    hsl = slice(bi * H + hb * 128, bi * H + (hb + 1) * 128)
                    nc.tensor.matmul(xps[:, sl], stp["kt16"][:, hsl],
                                     st["kt16"][:, hsl],
                                     start=(hb == 0), stop=(hb == HB - 1))
            # negated so the (negated-dl) cross product lands with + sign
            xts = xpool.tile([128, H], f16, tag="x")
            nc.scalar.activation(xts[:], xps[:], mybir.ActivationFunctionType.Copy,
                                 scale=-1.0)
            st["x"] = xts

        def state_u(c):
            # chunks 0/1 update M individually (startup); pairs start at (2,3)
            st = G[c]
            cross = (c % 2 == 1 and c >= 3)
            st["u"] = []
            for bi in range(BPC):
                if c == 0:
                    st["u"].append(st["k16"][:, bi * H:(bi + 1) * H])
                    continue
                ups = pshi.tile([128, H], f32, tag="big")
                for hb in range(HB):
                    hsl = slice(bi * H + hb * 128, bi * H + (hb + 1) * 128)
                    nc.tensor.matmul(ups[:], st["kt16"][:, hsl],
                                     mt16[bi][hb][:],
                                     start=(hb == 0),
                                     stop=(hb == HB - 1 and not cross))
                if cross:
                    sl = slice(bi * 128, (bi + 1) * 128)
                    nc.tensor.matmul(ups[:], st["x"][:, sl], G[c - 1]["dl"][bi][:],
                                     start=False, stop=True)
                # u_neg = r*ups - k (negated U; saves materializing -r)
                u_sb = upool.tile([128, H], f16, tag="u")
                nc.vector.scalar_tensor_tensor(u_sb[:], ups[:], st["r"][:, bi:bi + 1],
                                               st["k"][bi][:], mybir.AluOpType.mult,
                                               mybir.AluOpType.subtract)
                st["u"].append(u_sb[:])

        def state_delta(c):
            st = G[c]
            st["dl"] = []
            for bi in range(BPC):
                sl = slice(bi * 128, (bi + 1) * 128)
                dps = pshi.tile([128, H], f32, tag="big")
                nc.tensor.matmul(dps[:], st["g"][:, sl], st["u"][bi], start=True, stop=True)
                dl = dlpool.tile([128, H], f16, tag="dl")
                nc.scalar.copy(dl[:], dps[:])
                st["dl"].append(dl)

        def mupd(c, bis):
            # pair-end update: M^T += K_{c-1}^T Dl_{c-1} + K_c^T Dl_c
            # (bi 2,3 run at the start of the NEXT iteration to even PE load;
            #  some adds bounce PSUM->SBUF via DMA so Pool can do the add)
            st, stp = G[c], G[c - 1]
            for bi in bis:
                for jb in range(HB):
                    jsl = slice(bi * H + jb * 128, bi * H + (jb + 1) * 128)
                    mps = pshi.tile([128, H], f32, tag="big")
                    nc.tensor.matmul(mps[:], stp["k16"][:, jsl], stp["dl"][bi][:],
                                     start=True, stop=False)
                    nc.tensor.matmul(mps[:], st["k16"][:, jsl], st["dl"][bi][:],
                                     start=False, stop=True)
                    nc.vector.tensor_sub(mt16[bi][jb][:], mt16[bi][jb][:], mps[:])

        def mupd_single(c):
            # startup chunks 0 and 1: per-chunk M update (keeps PE busy early)
            st = G[c]
            for bi in range(BPC):
                for jb in range(HB):
                    jsl = slice(bi * H + jb * 128, bi * H + (jb + 1) * 128)
                    mps = pshi.tile([128, H], f32, tag="big")
                    nc.tensor.matmul(mps[:], st["k16"][:, jsl], st["dl"][bi][:],
                                     start=True, stop=True)
                    if c == 0:
                        nc.scalar.copy(mt16[bi][jb][:], mps[:])
                    else:
                        nc.vector.tensor_sub(mt16[bi][jb][:], mt16[bi][jb][:], mps[:])

        # ---- early prologue: read_proj weights + query + bias ----
        # (issued first: one of these DMAs expands to a long fine-grained
        # descriptor burst; at program start it overlaps the pipeline fill)
        # wtALL[:, ib*512 + op*128 + o] = W^T[ib*128 + i', op*128 + o]
        wtall = consts.tile([128, HB * H], f16, tag="wtall")
        for op in range(HB):
            wsb = wbuild.tile([128, H], f32, tag="wsb")
            nc.sync.dma_start(wsb[:], w_d[op * 128:(op + 1) * 128, :])
            wps = pshi.tile([128, H], f32, tag="big")
            for ib in range(HB):
                nc.tensor.transpose(wps[:, ib * 128:(ib + 1) * 128],
                                    wsb[:, ib * 128:(ib + 1) * 128], ident_f[:])
            nc.scalar.copy(
                wtall[:].rearrange("p (f o2 q) -> p f o2 q", f=HB, q=128)[:, :, op, :],
                wps[:])
        bias_all = consts.tile([BPC, H], f32, tag="biasall")
        for bi in range(BPC):
            nc.sync.dma_start(bias_all[bi:bi + 1, :], b_d[None, :])
        qs = []
        for bi in range(BPC):
            v4 = wbuild.tile([HB, 128], f32, tag="v4")
            nc.sync.dma_start(v4[:], hidden_d[bi, L - 1, :].rearrange("(f p) -> f p", p=128))
            tps = pslo.tile([128, HB], f32, tag="sm")
            nc.tensor.transpose(tps[:], v4[:], ident_f[:HB, :HB])
            q_t = consts.tile([128, HB], f16, tag=f"q{bi}", name=f"q{bi}")
            nc.scalar.copy(q_t[:], tps[:])
            qs.append(q_t)

        # ---- software-pipelined main loop ----
        # Issue order per iteration is tuned so each engine's in-order queue
        # services consumers before producers-for-later: PE never waits behind
        # unready work, Act chain copies aren't stuck behind squares, and the
        # k16->kt16 DMA for c+2 isn't stuck behind pair-end M adds.
        prep_load(0)
        prep_load(1)
        prep_load(2)
        prep_tr(0)
        prep_r(0)
        prep_tr(1)
        prep_r(1)
        aform(0)
        for lev in range(1, NLEV + 1):
            chain_level(0, lev)
        aform(1)
        for c in range(NCHUNK):
            nxt = c + 1 if c + 1 < NCHUNK else None
            nn = c + 2 if c + 2 < NCHUNK else None
            if c % 2 == 1 and c >= 3:
                xform(c)
            state_u(c)
            if nxt is not None:
                chain_sq(nxt, 1)
            state_delta(c)
            if nxt is not None:
                chain_gps(nxt, 1)
                chain_sq(nxt, 2)
            if nn is not None:
                prep_tr(nn)
            if c + 3 < NCHUNK:
                prep_load(c + 3)
            if c <= 1:
                mupd_single(c)
            elif c % 2 == 1:
                mupd(c, [0, 1, 2, 3])
            if nxt is not None:
                chain_gps(nxt, 2)
            if nn is not None:
                prep_r(nn)
                aform(nn)
            prev = c - 3
            if prev in G:
                del G[prev]

        # ---- finale: ctx = M q (row form); out = ctx W^T + b ----
        # phase-major over bi so PE/Act/DVE overlap across batch elements;
        # out-proj packs all 4 bi into one lhsT per ib block (4 matmuls total)
        cpss, ctx_rows = [], []
        for bi in range(BPC):
            cps = pshi.tile([1, H], f32, tag="big")
            for jb in range(HB):
                nc.tensor.matmul(cps[:], qs[bi][:, jb:jb + 1], mt16[bi][jb][:],
                                 start=(jb == 0), stop=(jb == HB - 1))
            cpss.append(cps)
        for bi in range(BPC):
            ctx_row = small.tile([1, H], f16, tag=f"ctxrow{bi}")
            nc.scalar.copy(ctx_row[:], cpss[bi][:])
            ctx_rows.append(ctx_row)
        ctxT = small.tile([128, HB * BPC], f16, tag="ctxT")
        for bi in range(BPC):
            for ib in range(HB):
                tp2 = pslo.tile([128, 1], f16, tag="sm1")
                nc.tensor.transpose(tp2[:], ctx_rows[bi][:, ib * 128:(ib + 1) * 128],
                                    ident_h[:1, :1])
                nc.vector.tensor_copy(ctxT[:, ib * BPC + bi:ib * BPC + bi + 1], tp2[:])
        ops4 = pshi.tile([BPC, H], f32, tag="big")
        for ib in range(HB):
            nc.tensor.matmul(ops4[:], ctxT[:, ib * BPC:(ib + 1) * BPC],
                             wtall[:, ib * H:(ib + 1) * H],
                             start=(ib == 0), stop=(ib == HB - 1))
        out_all = small.tile([BPC, H], f32, tag="outall")
        nc.vector.tensor_add(out_all[:], ops4[:], bias_all[:])
        nc.sync.dma_start(out_d[:, :], out_all[:])

    if legalize:
        _legalize_waits(nc)
    return nc


def _legalize_waits(nc, max_waits=1):
    """This toolchain's walrus encodes at most one semaphore wait per
    instruction. Hoist extra waits onto standalone EventSemaphore
    instructions on the same engine queue, immediately before the owner."""
    import json as _json
    m = _json.loads(bytes(nc.to_json_bytes()))
    n_fix = 0
    for fn in m["functions"]:
        for blk in fn["blocks"]:
            out = []
            for ins in blk.get("instructions", []):
                si = ins.get("sync_info") or {}
                waits = si.get("on_wait") or []
                if len(waits) > max_waits and ins.get("opcode") != "EventSemaphore":
                    extra, keep = waits[:-max_waits], waits[-max_waits:]
                    for i, w in enumerate(extra):
                        out.append({
                            "name": f"{ins['name']}-w{i}",
                            "engine": ins["engine"],
                            "opcode": "EventSemaphore",
                            "ins": [], "outs": [],
                            "sync_info": {"on_wait": [w], "on_update": []},
                        })
                    si["on_wait"] = keep
                    ins["sync_info"] = si
                    n_fix += 1
                out.append(ins)
            blk["instructions"] = out
    nc.m = mybir.module_from_json_bytes(_json.dumps(m).encode())
    return n_fix


def kernel(hidden: np.ndarray, W: np.ndarray, b: np.ndarray) -> np.ndarray:
    if "nc" not in _cached:
        _cached["nc"] = _build_program()
    nc = _cached["nc"]

    hidden = np.ascontiguousarray(hidden, dtype=np.float32)
    W = np.ascontiguousarray(W, dtype=np.float32)
    b = np.ascontiguousarray(b, dtype=np.float32)

    in_maps = []
    for ci in range(NCORES):
        in_maps.append({
            "hidden": hidden[ci * BPC:(ci + 1) * BPC],
            "W": W,
            "bvec": b,
        })
    res = run_bass_kernel_spmd(nc, in_maps, core_ids=list(range(NCORES)))
    _cached["last_results"] = res
    out = np.concatenate([res.results[ci]["out"] for ci in range(NCORES)], axis=0)
    return out.astype(np.float32)


if __name__ == "__main__":
    rng = np.random.default_rng(0)
    h = rng.standard_normal((B, L, H), dtype=np.float32)
    w = rng.standard_normal((H, H), dtype=np.float32) * (1.0 / np.sqrt(H))
    bb = np.zeros((H,), np.float32)
    o = kernel(h, w, bb)
    print(o.shape, o.dtype)


# revision 13
# speedup vs baseline: 1.6398x; 1.0381x over previous
"""DeltaRule memory scan kernel for Trainium2, 8 NeuronCores, data-parallel over batch.

Reference semantics (per batch element, H=512, L=2048):
    M_0 = 0  [H,H]
    for t in 0..L-2:   k = hidden[t]
        d = k.k + eps; delta = k - (M k)/d; M += outer(delta, k)
    out = (M @ hidden[L-1]) @ W.T + b

Implementation: chunked delta rule (UT transform), chunk C=128, float16.
Per chunk with keys K [C,H], r = 1/rowsum(K^2):
    A  = strict_tril(diag(r) K K^T)            [C,C]
    T  = (I+A)^{-1} ~= (I-A)(I+A^2)(I+A^4)    (A nilpotent; exact through A^7)
    U- = diag(r)(K M^T + X Dl-_prev) - K       (negated U; X: intra-pair cross)
    Dl- = T U-                                  [C,H]
    M^T -= K_c0^T Dl-_c0 + K_c1^T Dl-_c1       (pair-accumulated in PSUM)
All matmuls f16 (1 cy/row); M^T state kept in f16 only (validated: relerr ~7e-3
on hw vs 2e-2 gate). Hardware-calibrated engine budget per chunk: PE ~15.7us
span share (matmuls + all transposes - XBAR DMA transposes head-of-line-block
the single SP DMA queue, so K^T/A^T/W^T use PE transposes), DVE ~11us (M
subtracts, U, A-scale, g-adds), Act ~12us (squares, PSUM->SBUF casts), Pool
~7.5us (SBUF-only f32->f16 key casts; GpSimd has no PSUM port and runs ~3x
slower than nominal). Loads prefetch 3 chunks ahead; chunk phases interleave
across the 4 batch elements per core to keep the PE queue fed.
"""
import sys
import numpy as np
from contextlib import ExitStack

sys.path.insert(0, "/opt/trn_rl_repo")

import concourse.bass as bass
import concourse.mybir as mybir
from concourse import tile
from concourse.bass_utils import run_bass_kernel_spmd
from concourse.masks import make_identity, make_lower_triangular

B, L, H = 32, 2048, 512
NCORES = 8
BPC = B // NCORES          # 4 batch elements per core
C = 128                    # chunk size
T = L - 1                  # 2047 scan steps
NCHUNK = (T + C - 1) // C  # 16 (last chunk has 127 valid rows)
NLEV = 2                   # Neumann levels -> exact through A^7 (validated)
EPS = 1e-6
HB = H // 128              # 4 h-blocks

f32 = mybir.dt.float32
f16 = mybir.dt.float16

_cached = {}

# engine split for the 16 pair-end M updates (V=vector/DVE, P=pool)
_MUPD_ENG = "PVPPVPVPPVPPVPVP"


def _build_program(legalize=True):
    nc = bass.Bass(target_bir_lowering=False, debug=False)

    hidden_d = nc.declare_dram_parameter("hidden", [BPC, L, H], f32, isOutput=False)
    w_d = nc.declare_dram_parameter("W", [H, H], f32, isOutput=False)
    b_d = nc.declare_dram_parameter("bvec", [H], f32, isOutput=False)
    out_d = nc.declare_dram_parameter("out", [BPC, H], f32, isOutput=True)

    with tile.TileContext(nc) as tc, ExitStack() as ctx:
        consts = ctx.enter_context(tc.tile_pool(name="consts", bufs=1))
        wbuild = ctx.enter_context(tc.tile_pool(name="wbuild", bufs=2))
        mtpool = ctx.enter_context(tc.tile_pool(name="mt", bufs=1))
        kpool = ctx.enter_context(tc.tile_pool(name="k", bufs=16))
        k16pool = ctx.enter_context(tc.tile_pool(name="k16", bufs=5))
        chain = ctx.enter_context(tc.tile_pool(name="chain", bufs=6))
        upool = ctx.enter_context(tc.tile_pool(name="u", bufs=6))
        dlpool = ctx.enter_context(tc.tile_pool(name="dl", bufs=10))
        xpool = ctx.enter_context(tc.tile_pool(name="x", bufs=2))
        small = ctx.enter_context(tc.tile_pool(name="small", bufs=4))
        pslo = ctx.enter_context(tc.tile_pool(name="pslo", bufs=1, space="PSUM"))
        pshi = ctx.enter_context(tc.tile_pool(name="pshi", bufs=6, space="PSUM"))

        # ---- constants needed by the main loop ----
        ident_f = consts.tile([128, 128], f32, tag="identf")
        make_identity(nc, ident_f[:])
        ident_h = consts.tile([128, 128], f16, tag="identh")
        make_identity(nc, ident_h[:])
        identp = consts.tile([128, H], f16, tag="identp")
        for bi in range(BPC):
            nc.vector.tensor_copy(identp[:, bi * 128:(bi + 1) * 128], ident_h[:])
        smask = consts.tile([128, 128], f32, tag="smask")
        make_lower_triangular(nc, smask[:], val=1.0, diag=False)

        # persistent state M^T (f16 only), per (bi, jb)
        mt16 = [[mtpool.tile([128, H], f16, tag=f"mt{bi}_{jb}", name=f"mt{bi}_{jb}")
                 for jb in range(HB)] for bi in range(BPC)]

        # pre-zeroed key tiles for the final (127-row) chunk: zeroing them in
        # the prologue keeps the memset out of the busy mid-scan DVE queue
        k15 = [consts.tile([128, H], f32, tag=f"k15_{bi}", name=f"k15_{bi}")
               for bi in range(BPC)]
        for bi in range(BPC):
            nc.vector.memset(k15[bi][:], 0.0)

        G = {}

        def prep_load(c):
            t0 = c * C
            nrows = min(C, T - t0)
            st = {"k": []}
            for bi in range(BPC):
                if nrows < C:
                    k_t = k15[bi]
                    nc.sync.dma_start(k_t[:nrows, :], hidden_d[bi, t0:t0 + nrows, :])
                else:
                    k_t = kpool.tile([128, H], f32, tag="K", name=f"k{c}_{bi}")
                    nc.sync.dma_start(k_t[:], hidden_d[bi, t0:t0 + C, :])
                st["k"].append(k_t)
            # packed row-major f16 keys [128, (bi h)] (Pool: SBUF-only casts)
            k16a = k16pool.tile([128, BPC * H], f16, tag="k16", name=f"k16_{c}")
            for bi in range(BPC):
                nc.gpsimd.tensor_copy(k16a[:, bi * H:(bi + 1) * H], st["k"][bi][:])
            st["k16"] = k16a
            G[c] = st

        def prep_tr(c):
            # K^T blocks via f16 PE transposes of the packed keys (XBAR DMA on
            # the single SP queue head-of-line-blocks the whole pipeline on hw)
            st = G[c]
            kt16a = k16pool.tile([128, BPC * H], f16, tag="kt16", name=f"kt16_{c}")
            for bi in range(BPC):
                ktps = pshi.tile([128, H], f16, tag="big")
                for hb in range(HB):
                    hsl = slice(bi * H + hb * 128, bi * H + (hb + 1) * 128)
                    nc.tensor.transpose(ktps[:, hb * 128:(hb + 1) * 128],
                                        st["k16"][:, hsl], ident_h[:])
                if bi < 2:
                    nc.scalar.copy(kt16a[:, bi * H:(bi + 1) * H], ktps[:])
                else:
                    nc.vector.tensor_copy(kt16a[:, bi * H:(bi + 1) * H], ktps[:])
            st["kt16"] = kt16a

        def prep_r(c):
            st = G[c]
            dall = small.tile([128, BPC], f32, tag="dall")
            for bi in range(BPC):
                scr = small.tile([128, H], f16, tag="scr")
                nc.scalar.activation(scr[:], st["k"][bi][:],
                                     mybir.ActivationFunctionType.Square,
                                     accum_out=dall[:, bi:bi + 1])
            rall = small.tile([128, BPC], f32, tag="rall")
            if c == NCHUNK - 1:
                # only the zero-padded final chunk needs the EPS guard
                nc.vector.tensor_scalar_add(dall[:], dall[:], EPS)
            nc.vector.reciprocal(rall[:], dall[:])
            st["r"] = rall

        def aform(c):
            st = G[c]
            kt = st["kt16"]
            a_ps = pshi.tile([128, H], f32, tag="big")
            for bi in range(BPC):
                sl = slice(bi * 128, (bi + 1) * 128)
                for hb in range(HB):
                    hsl = slice(bi * H + hb * 128, bi * H + (hb + 1) * 128)
                    nc.tensor.matmul(a_ps[:, sl], kt[:, hsl], kt[:, hsl],
                                     start=(hb == 0), stop=(hb == HB - 1))
            a_all = chain.tile([128, H], f16, tag="ak")
            for bi in range(BPC):
                sl = slice(bi * 128, (bi + 1) * 128)
                nc.vector.scalar_tensor_tensor(a_all[:, sl], a_ps[:, sl],
                                               st["r"][:, bi:bi + 1], smask[:],
                                               mybir.AluOpType.mult,
                                               mybir.AluOpType.mult)
            at_ps = pshi.tile([128, H], f16, tag="big")
            for bi in range(BPC):
                sl = slice(bi * 128, (bi + 1) * 128)
                nc.tensor.transpose(at_ps[:, sl], a_all[:, sl], ident_h[:])
            at_all = chain.tile([128, H], f16, tag="atk")
            nc.scalar.copy(at_all[:], at_ps[:])
            g0 = chain.tile([128, H], f16, tag="g")
            nc.vector.tensor_sub(g0[:], identp[:], at_all[:])
            st["ak"], st["atk"], st["g"] = a_all, at_all, g0

        def chain_sq(c, lev):
            # squaring half: A^(2^lev); its transpose via XBAR DMA (free-ish)
            st = G[c]
            ak, atk = st["ak"], st["atk"]
            sq1 = pshi.tile([128, H], f32, tag="big")
            for bi in range(BPC):
                sl = slice(bi * 128, (bi + 1) * 128)
                nc.tensor.matmul(sq1[:, sl], atk[:, sl], ak[:, sl], start=True, stop=True)
            ak2 = chain.tile([128, H], f16, tag="ak")
            nc.scalar.copy(ak2[:], sq1[:])
            if lev < NLEV:
                sq2 = pshi.tile([128, H], f32, tag="big")
                for bi in range(BPC):
                    sl = slice(bi * 128, (bi + 1) * 128)
                    nc.tensor.matmul(sq2[:, sl], ak[:, sl], atk[:, sl], start=True, stop=True)
                atk2 = chain.tile([128, H], f16, tag="atk")
                nc.scalar.copy(atk2[:], sq2[:])
            else:
                atk2 = None
            st["ak_n"], st["atk_n"] = ak2, atk2

        def chain_gps(c, lev):
            # accumulation half: g += (A^(2^lev))^T g
            st = G[c]
            ak2 = st["ak_n"]
            gps = pshi.tile([128, H], f32, tag="big")
            for bi in range(BPC):
                sl = slice(bi * 128, (bi + 1) * 128)
                nc.tensor.matmul(gps[:, sl], ak2[:, sl], st["g"][:, sl], start=True, stop=True)
            g_nxt = chain.tile([128, H], f16, tag="g")
            nc.vector.tensor_add(g_nxt[:], gps[:], st["g"][:])
            st["ak"], st["atk"], st["g"] = st["ak_n"], st["atk_n"], g_nxt

        def chain_level(c, lev):
            chain_sq(c, lev)
            chain_gps(c, lev)

        def xform(c):
            # X^T for pair (c-1, c): xts[:, bi-slice][a, b] = K_{c-1}[a]·K_c[b]
            st, stp = G[c], G[c - 1]
            xps = pshi.tile([128, H], f32, tag="big")
            for bi in range(BPC):
                sl = slice(bi * 128, (bi + 1) * 128)
                for hb in range(HB):
                    hsl = slice(bi * H + hb * 128, bi * H + (hb + 1) * 128)
                    nc.tensor.matmul(xps[:, sl], stp["kt16"][:, hsl],
                                     st["kt16"][:, hsl],
                                     start=(hb == 0), stop=(hb == HB - 1))
            # negated so the (negated-dl) cross product lands with + sign
            xts = xpool.tile([128, H], f16, tag="x")
            nc.scalar.activation(xts[:], xps[:], mybir.ActivationFunctionType.Copy,
                                 scale=-1.0)
            st["x"] = xts

        def state_u(c):
            # chunks 0/1 update M individually (startup); pairs start at (2,3)
            st = G[c]
            cross = (c % 2 == 1 and c >= 3)
            st["u"] = []
            for bi in range(BPC):
                if c == 0:
                    st["u"].append(st["k16"][:, bi * H:(bi + 1) * H])
                    continue
                ups = pshi.tile([128, H], f32, tag="big")
                for hb in range(HB):
                    hsl = slice(bi * H + hb * 128, bi * H + (hb + 1) * 128)
                    nc.tensor.matmul(ups[:], st["kt16"][:, hsl],
                                     mt16[bi][hb][:],
                                     start=(hb == 0),
                                     stop=(hb == HB - 1 and not cross))
                if cross:
                    sl = slice(bi * 128, (bi + 1) * 128)
                    nc.tensor.matmul(ups[:], st["x"][:, sl], G[c - 1]["dl"][bi][:],
                                     start=False, stop=True)
                # u_neg = r*ups - k (negated U; saves materializing -r)
                u_sb = upool.tile([128, H], f16, tag="u")
                nc.vector.scalar_tensor_tensor(u_sb[:], ups[:], st["r"][:, bi:bi + 1],
                                               st["k"][bi][:], mybir.AluOpType.mult,
                                               mybir.AluOpType.subtract)
                st["u"].append(u_sb[:])

        def state_delta(c):
            st = G[c]
            st["dl"] = []
            for bi in range(BPC):
                sl = slice(bi * 128, (bi + 1) * 128)
                dps = pshi.tile([128, H], f32, tag="big")
                nc.tensor.matmul(dps[:], st["g"][:, sl], st["u"][bi], start=True, stop=True)
                dl = dlpool.tile([128, H], f16, tag="dl")
                nc.scalar.copy(dl[:], dps[:])
                st["dl"].append(dl)

        def mupd(c, bis):
            # pair-end update: M^T += K_{c-1}^T Dl_{c-1} + K_c^T Dl_c
            # (bi 2,3 run at the start of the NEXT iteration to even PE load;
            #  some adds bounce PSUM->SBUF via DMA so Pool can do the add)
            st, stp = G[c], G[c - 1]
            for bi in bis:
                for jb in range(HB):
                    jsl = slice(bi * H + jb * 128, bi * H + (jb + 1) * 128)
                    mps = pshi.tile([128, H], f32, tag="big")
                    nc.tensor.matmul(mps[:], stp["k16"][:, jsl], stp["dl"][bi][:],
                                     start=True, stop=False)
                    nc.tensor.matmul(mps[:], st["k16"][:, jsl], st["dl"][bi][:],
                                     start=False, stop=True)
                    nc.vector.tensor_sub(mt16[bi][jb][:], mt16[bi][jb][:], mps[:])

        def mupd_single(c):
            # startup chunks 0 and 1: per-chunk M update (keeps PE busy early)
            st = G[c]
            for bi in range(BPC):
                for jb in range(HB):
                    jsl = slice(bi * H + jb * 128, bi * H + (jb + 1) * 128)
                    mps = pshi.tile([128, H], f32, tag="big")
                    nc.tensor.matmul(mps[:], st["k16"][:, jsl], st["dl"][bi][:],
                                     start=True, stop=True)
                    if c == 0:
                        nc.scalar.copy(mt16[bi][jb][:], mps[:])
                    else:
                        nc.vector.tensor_sub(mt16[bi][jb][:], mt16[bi][jb][:], mps[:])

        # ---- early prologue: read_proj weights + query + bias ----
        # (issued first: one of these DMAs expands to a long fine-grained
        # descriptor burst; at program start it overlaps the pipeline fill)
        # wtALL[:, ib*512 + op*128 + o] = W^T[ib*128 + i', op*128 + o]
        wtall = consts.tile([128, HB * H], f16, tag="wtall")
        for op in range(HB):
            wsb = wbuild.tile([128, H], f32, tag="wsb")
            nc.sync.dma_start(wsb[:], w_d[op * 128:(op + 1) * 128, :])
            wps = pshi.tile([128, H], f32, tag="big")
            for ib in range(HB):
                nc.tensor.transpose(wps[:, ib * 128:(ib + 1) * 128],
                                    wsb[:, ib * 128:(ib + 1) * 128], ident_f[:])
            nc.scalar.copy(
                wtall[:].rearrange("p (f o2 q) -> p f o2 q", f=HB, q=128)[:, :, op, :],
                wps[:])
        bias_all = consts.tile([BPC, H], f32, tag="biasall")
        for bi in range(BPC):
            nc.sync.dma_start(bias_all[bi:bi + 1, :], b_d[None, :])
        qs = []
        for bi in range(BPC):
            v4 = wbuild.tile([HB, 128], f32, tag="v4")
            nc.sync.dma_start(v4[:], hidden_d[bi, L - 1, :].rearrange("(f p) -> f p", p=128))
            tps = pslo.tile([128, HB], f32, tag="sm")
            nc.tensor.transpose(tps[:], v4[:], ident_f[:HB, :HB])
            q_t = consts.tile([128, HB], f16, tag=f"q{bi}", name=f"q{bi}")
            nc.scalar.copy(q_t[:], tps[:])
            qs.append(q_t)

        # ---- software-pipelined main loop ----
        # Issue order per iteration is tuned so each engine's in-order queue
        # services consumers before producers-for-later: PE never waits behind
        # unready work, Act chain copies aren't stuck behind squares, and the
        # k16->kt16 DMA for c+2 isn't stuck behind pair-end M adds.
        prep_load(0)
        prep_load(1)
        prep_load(2)
        prep_tr(0)
        prep_r(0)
        prep_tr(1)
        prep_r(1)
        aform(0)
        for lev in range(1, NLEV + 1):
            chain_level(0, lev)
        aform(1)
        for c in range(NCHUNK):
            nxt = c + 1 if c + 1 < NCHUNK else None
            nn = c + 2 if c + 2 < NCHUNK else None
            if c % 2 == 1 and c >= 3:
                xform(c)
            state_u(c)
            if nxt is not None:
                chain_sq(nxt, 1)
            state_delta(c)
            if nxt is not None:
                chain_gps(nxt, 1)
                chain_sq(nxt, 2)
            if c > 1 and c % 2 == 1:
                mupd(c, [0, 1])       # start the DVE subtract drain early
            if nn is not None:
                prep_tr(nn)
            if c + 3 < NCHUNK:
                prep_load(c + 3)
            if c <= 1:
                mupd_single(c)
            elif c % 2 == 1:
                mupd(c, [2, 3])
            if nxt is not None:
                chain_gps(nxt, 2)
            if nn is not None:
                prep_r(nn)
                aform(nn)
            prev = c - 3
            if prev in G:
                del G[prev]

        # ---- finale: ctx = M q (row form); out = ctx W^T + b ----
        # phase-major over bi so PE/Act/DVE overlap across batch elements;
        # out-proj packs all 4 bi into one lhsT per ib block (4 matmuls total)
        cpss, ctx_rows = [], []
        for bi in range(BPC):
            cps = pshi.tile([1, H], f32, tag="big")
            for jb in range(HB):
                nc.tensor.matmul(cps[:], qs[bi][:, jb:jb + 1], mt16[bi][jb][:],
                                 start=(jb == 0), stop=(jb == HB - 1))
            cpss.append(cps)
        for bi in range(BPC):
            ctx_row = small.tile([1, H], f16, tag=f"ctxrow{bi}")
            nc.scalar.copy(ctx_row[:], cpss[bi][:])
            ctx_rows.append(ctx_row)
        ctxT = small.tile([128, HB * BPC], f16, tag="ctxT")
        for bi in range(BPC):
            for ib in range(HB):
                tp2 = pslo.tile([128, 1], f16, tag="sm1")
                nc.tensor.transpose(tp2[:], ctx_rows[bi][:, ib * 128:(ib + 1) * 128],
                                    ident_h[:1, :1])
                nc.vector.tensor_copy(ctxT[:, ib * BPC + bi:ib * BPC + bi + 1], tp2[:])
        ops4 = pshi.tile([BPC, H], f32, tag="big")
        for ib in range(HB):
            nc.tensor.matmul(ops4[:], ctxT[:, ib * BPC:(ib + 1) * BPC],
                             wtall[:, ib * H:(ib + 1) * H],
                             start=(ib == 0), stop=(ib == HB - 1))
        out_all = small.tile([BPC, H], f32, tag="outall")
        nc.vector.tensor_add(out_all[:], ops4[:], bias_all[:])
        nc.sync.dma_start(out_d[:, :], out_all[:])

    if legalize:
        _legalize_waits(nc)
    return nc


def _legalize_waits(nc, max_waits=1):
    """This toolchain's walrus encodes at most one semaphore wait per
    instruction. Hoist extra waits onto standalone EventSemaphore
    instructions on the same engine queue, immediately before the owner."""
    import json as _json
    m = _json.loads(bytes(nc.to_json_bytes()))
    n_fix = 0
    for fn in m["functions"]:
        for blk in fn["blocks"]:
            out = []
            for ins in blk.get("instructions", []):
                si = ins.get("sync_info") or {}
                waits = si.get("on_wait") or []
                if len(waits) > max_waits and ins.get("opcode") != "EventSemaphore":
                    extra, keep = waits[:-max_waits], waits[-max_waits:]
                    for i, w in enumerate(extra):
                        out.append({
                            "name": f"{ins['name']}-w{i}",
                            "engine": ins["engine"],
                            "opcode": "EventSemaphore",
                            "ins": [], "outs": [],
                            "sync_info": {"on_wait": [w], "on_update": []},
                        })
                    si["on_wait"] = keep
                    ins["sync_info"] = si
                    n_fix += 1
                out.append(ins)
            blk["instructions"] = out
    nc.m = mybir.module_from_json_bytes(_json.dumps(m).encode())
    return n_fix


def kernel(hidden: np.ndarray, W: np.ndarray, b: np.ndarray) -> np.ndarray:
    if "nc" not in _cached:
        _cached["nc"] = _build_program()
    nc = _cached["nc"]

    hidden = np.ascontiguousarray(hidden, dtype=np.float32)
    W = np.ascontiguousarray(W, dtype=np.float32)
    b = np.ascontiguousarray(b, dtype=np.float32)

    in_maps = []
    for ci in range(NCORES):
        in_maps.append({
            "hidden": hidden[ci * BPC:(ci + 1) * BPC],
            "W": W,
            "bvec": b,
        })
    res = run_bass_kernel_spmd(nc, in_maps, core_ids=list(range(NCORES)))
    _cached["last_results"] = res
    out = np.concatenate([res.results[ci]["out"] for ci in range(NCORES)], axis=0)
    return out.astype(np.float32)


if __name__ == "__main__":
    rng = np.random.default_rng(0)
    h = rng.standard_normal((B, L, H), dtype=np.float32)
    w = rng.standard_normal((H, H), dtype=np.float32) * (1.0 / np.sqrt(H))
    bb = np.zeros((H,), np.float32)
    o = kernel(h, w, bb)
    print(o.shape, o.dtype)


# revision 14
# speedup vs baseline: 1.7056x; 1.0401x over previous
"""DeltaRule memory scan kernel for Trainium2, 8 NeuronCores, data-parallel over batch.

Reference semantics (per batch element, H=512, L=2048):
    M_0 = 0  [H,H]
    for t in 0..L-2:   k = hidden[t]
        d = k.k + eps; delta = k - (M k)/d; M += outer(delta, k)
    out = (M @ hidden[L-1]) @ W.T + b

Implementation: chunked delta rule (UT transform), chunk C=128, float16.
Per chunk with keys K [C,H], r = 1/rowsum(K^2):
    A  = strict_tril(diag(r) K K^T)            [C,C]
    T  = (I+A)^{-1} ~= (I-A)(I+A^2)(I+A^4)    (A nilpotent; exact through A^7)
    U- = diag(r)(K M^T + X Dl-_prev) - K       (negated U; X: intra-pair cross)
    Dl- = T U-                                  [C,H]
    M^T -= K_c0^T Dl-_c0 + K_c1^T Dl-_c1       (pair-accumulated in PSUM)
All matmuls f16 (1 cy/row); M^T state kept in f16 only (validated: relerr ~7e-3
on hw vs 2e-2 gate). Hardware-calibrated engine budget per chunk: PE ~15.7us
span share (matmuls + all transposes - XBAR DMA transposes head-of-line-block
the single SP DMA queue, so K^T/A^T/W^T use PE transposes), DVE ~11us (M
subtracts, U, A-scale, g-adds), Act ~12us (squares, PSUM->SBUF casts), Pool
~7.5us (SBUF-only f32->f16 key casts; GpSimd has no PSUM port and runs ~3x
slower than nominal). Loads prefetch 3 chunks ahead; chunk phases interleave
across the 4 batch elements per core to keep the PE queue fed.
"""
import sys
import numpy as np
from contextlib import ExitStack

sys.path.insert(0, "/opt/trn_rl_repo")

import concourse.bass as bass
import concourse.mybir as mybir
from concourse import tile
from concourse.bass_utils import run_bass_kernel_spmd
from concourse.masks import make_identity, make_lower_triangular

B, L, H = 32, 2048, 512
NCORES = 8
BPC = B // NCORES          # 4 batch elements per core
C = 128                    # chunk size
T = L - 1                  # 2047 scan steps
NCHUNK = (T + C - 1) // C  # 16 (last chunk has 127 valid rows)
NLEV = 2                   # Neumann levels -> exact through A^7 (validated)
EPS = 1e-6
HB = H // 128              # 4 h-blocks

f32 = mybir.dt.float32
f16 = mybir.dt.float16

_cached = {}

# engine split for the 16 pair-end M updates (V=vector/DVE, P=pool)
_MUPD_ENG = "PVPPVPVPPVPPVPVP"


def _build_program(legalize=True):
    nc = bass.Bass(target_bir_lowering=False, debug=False)

    hidden_d = nc.declare_dram_parameter("hidden", [BPC, L, H], f32, isOutput=False)
    w_d = nc.declare_dram_parameter("W", [H, H], f32, isOutput=False)
    b_d = nc.declare_dram_parameter("bvec", [H], f32, isOutput=False)
    out_d = nc.declare_dram_parameter("out", [BPC, H], f32, isOutput=True)

    with tile.TileContext(nc) as tc, ExitStack() as ctx:
        consts = ctx.enter_context(tc.tile_pool(name="consts", bufs=1))
        wbuild = ctx.enter_context(tc.tile_pool(name="wbuild", bufs=2))
        mtpool = ctx.enter_context(tc.tile_pool(name="mt", bufs=1))
        kpool = ctx.enter_context(tc.tile_pool(name="k", bufs=16))
        k16pool = ctx.enter_context(tc.tile_pool(name="k16", bufs=5))
        chain = ctx.enter_context(tc.tile_pool(name="chain", bufs=6))
        upool = ctx.enter_context(tc.tile_pool(name="u", bufs=6))
        dlpool = ctx.enter_context(tc.tile_pool(name="dl", bufs=10))
        xpool = ctx.enter_context(tc.tile_pool(name="x", bufs=2))
        small = ctx.enter_context(tc.tile_pool(name="small", bufs=4))
        pslo = ctx.enter_context(tc.tile_pool(name="pslo", bufs=1, space="PSUM"))
        pshi = ctx.enter_context(tc.tile_pool(name="pshi", bufs=6, space="PSUM"))

        # ---- constants needed by the main loop ----
        ident_f = consts.tile([128, 128], f32, tag="identf")
        make_identity(nc, ident_f[:])
        ident_h = consts.tile([128, 128], f16, tag="identh")
        make_identity(nc, ident_h[:])
        identp = consts.tile([128, H], f16, tag="identp")
        for bi in range(BPC):
            nc.vector.tensor_copy(identp[:, bi * 128:(bi + 1) * 128], ident_h[:])
        smask = consts.tile([128, 128], f32, tag="smask")
        make_lower_triangular(nc, smask[:], val=1.0, diag=False)

        # persistent state M^T (f16 only), per (bi, jb)
        mt16 = [[mtpool.tile([128, H], f16, tag=f"mt{bi}_{jb}", name=f"mt{bi}_{jb}")
                 for jb in range(HB)] for bi in range(BPC)]

        # pre-zeroed key tiles for the final (127-row) chunk: zeroing them in
        # the prologue keeps the memset out of the busy mid-scan DVE queue
        k15 = [consts.tile([128, H], f32, tag=f"k15_{bi}", name=f"k15_{bi}")
               for bi in range(BPC)]
        for bi in range(BPC):
            nc.vector.memset(k15[bi][:], 0.0)

        G = {}

        def prep_load(c):
            t0 = c * C
            nrows = min(C, T - t0)
            st = {"k": []}
            for bi in range(BPC):
                if nrows < C:
                    k_t = k15[bi]
                    nc.sync.dma_start(k_t[:nrows, :], hidden_d[bi, t0:t0 + nrows, :])
                else:
                    k_t = kpool.tile([128, H], f32, tag="K", name=f"k{c}_{bi}")
                    nc.sync.dma_start(k_t[:], hidden_d[bi, t0:t0 + C, :])
                st["k"].append(k_t)
            # packed row-major f16 keys [128, (bi h)] (Pool: SBUF-only casts)
            k16a = k16pool.tile([128, BPC * H], f16, tag="k16", name=f"k16_{c}")
            for bi in range(BPC):
                nc.gpsimd.tensor_copy(k16a[:, bi * H:(bi + 1) * H], st["k"][bi][:])
            st["k16"] = k16a
            G[c] = st

        def prep_tr(c):
            # K^T blocks via f16 PE transposes of the packed keys (XBAR DMA on
            # the single SP queue head-of-line-blocks the whole pipeline on hw)
            st = G[c]
            kt16a = k16pool.tile([128, BPC * H], f16, tag="kt16", name=f"kt16_{c}")
            for bi in range(BPC):
                ktps = pshi.tile([128, H], f16, tag="big")
                for hb in range(HB):
                    hsl = slice(bi * H + hb * 128, bi * H + (hb + 1) * 128)
                    nc.tensor.transpose(ktps[:, hb * 128:(hb + 1) * 128],
                                        st["k16"][:, hsl], ident_h[:])
                if bi < 2:
                    nc.scalar.copy(kt16a[:, bi * H:(bi + 1) * H], ktps[:])
                else:
                    nc.vector.tensor_copy(kt16a[:, bi * H:(bi + 1) * H], ktps[:])
            st["kt16"] = kt16a

        def prep_r(c):
            st = G[c]
            dall = small.tile([128, BPC], f32, tag="dall")
            for bi in range(BPC):
                scr = small.tile([128, H], f16, tag="scr")
                nc.scalar.activation(scr[:], st["k"][bi][:],
                                     mybir.ActivationFunctionType.Square,
                                     accum_out=dall[:, bi:bi + 1])
            rall = small.tile([128, BPC], f32, tag="rall")
            if c == NCHUNK - 1:
                # only the zero-padded final chunk needs the EPS guard
                nc.vector.tensor_scalar_add(dall[:], dall[:], EPS)
            nc.vector.reciprocal(rall[:], dall[:])
            st["r"] = rall

        def aform_a(c):
            st = G[c]
            kt = st["kt16"]
            a_ps = pshi.tile([128, H], f32, tag="big")
            for bi in range(BPC):
                sl = slice(bi * 128, (bi + 1) * 128)
                for hb in range(HB):
                    hsl = slice(bi * H + hb * 128, bi * H + (hb + 1) * 128)
                    nc.tensor.matmul(a_ps[:, sl], kt[:, hsl], kt[:, hsl],
                                     start=(hb == 0), stop=(hb == HB - 1))
            a_all = chain.tile([128, H], f16, tag="ak")
            for bi in range(BPC):
                sl = slice(bi * 128, (bi + 1) * 128)
                nc.vector.scalar_tensor_tensor(a_all[:, sl], a_ps[:, sl],
                                               st["r"][:, bi:bi + 1], smask[:],
                                               mybir.AluOpType.mult,
                                               mybir.AluOpType.mult)
            st["ak"] = a_all

        def aform_b(c):
            st = G[c]
            a_all = st["ak"]
            at_ps = pshi.tile([128, H], f16, tag="big")
            for bi in range(BPC):
                sl = slice(bi * 128, (bi + 1) * 128)
                nc.tensor.transpose(at_ps[:, sl], a_all[:, sl], ident_h[:])
            at_all = chain.tile([128, H], f16, tag="atk")
            nc.scalar.copy(at_all[:], at_ps[:])
            g0 = chain.tile([128, H], f16, tag="g")
            nc.vector.tensor_sub(g0[:], identp[:], at_all[:])
            st["atk"], st["g"] = at_all, g0

        def aform(c):
            aform_a(c)
            aform_b(c)

        def chain_sq(c, lev):
            # squaring half: A^(2^lev); its transpose via XBAR DMA (free-ish)
            st = G[c]
            ak, atk = st["ak"], st["atk"]
            sq1 = pshi.tile([128, H], f32, tag="big")
            for bi in range(BPC):
                sl = slice(bi * 128, (bi + 1) * 128)
                nc.tensor.matmul(sq1[:, sl], atk[:, sl], ak[:, sl], start=True, stop=True)
            ak2 = chain.tile([128, H], f16, tag="ak")
            nc.scalar.copy(ak2[:], sq1[:])
            if lev < NLEV:
                sq2 = pshi.tile([128, H], f32, tag="big")
                for bi in range(BPC):
                    sl = slice(bi * 128, (bi + 1) * 128)
                    nc.tensor.matmul(sq2[:, sl], ak[:, sl], atk[:, sl], start=True, stop=True)
                atk2 = chain.tile([128, H], f16, tag="atk")
                nc.scalar.copy(atk2[:], sq2[:])
            else:
                atk2 = None
            st["ak_n"], st["atk_n"] = ak2, atk2

        def chain_gps(c, lev):
            # accumulation half: g += (A^(2^lev))^T g
            st = G[c]
            ak2 = st["ak_n"]
            gps = pshi.tile([128, H], f32, tag="big")
            for bi in range(BPC):
                sl = slice(bi * 128, (bi + 1) * 128)
                nc.tensor.matmul(gps[:, sl], ak2[:, sl], st["g"][:, sl], start=True, stop=True)
            g_nxt = chain.tile([128, H], f16, tag="g")
            nc.vector.tensor_add(g_nxt[:], gps[:], st["g"][:])
            st["ak"], st["atk"], st["g"] = st["ak_n"], st["atk_n"], g_nxt

        def chain_level(c, lev):
            chain_sq(c, lev)
            chain_gps(c, lev)

        def xform(c):
            # X^T for pair (c-1, c): xts[:, bi-slice][a, b] = K_{c-1}[a]·K_c[b]
            st, stp = G[c], G[c - 1]
            xps = pshi.tile([128, H], f32, tag="big")
            for bi in range(BPC):
                sl = slice(bi * 128, (bi + 1) * 128)
                for hb in range(HB):
                    hsl = slice(bi * H + hb * 128, bi * H + (hb + 1) * 128)
                    nc.tensor.matmul(xps[:, sl], stp["kt16"][:, hsl],
                                     st["kt16"][:, hsl],
                                     start=(hb == 0), stop=(hb == HB - 1))
            # negated so the (negated-dl) cross product lands with + sign
            xts = xpool.tile([128, H], f16, tag="x")
            nc.scalar.activation(xts[:], xps[:], mybir.ActivationFunctionType.Copy,
                                 scale=-1.0)
            st["x"] = xts

        def state_u(c):
            # chunks 0/1 update M individually (startup); pairs start at (2,3)
            st = G[c]
            cross = (c % 2 == 1 and c >= 3)
            st["u"] = []
            for bi in range(BPC):
                if c == 0:
                    st["u"].append(st["k16"][:, bi * H:(bi + 1) * H])
                    continue
                ups = pshi.tile([128, H], f32, tag="big")
                for hb in range(HB):
                    hsl = slice(bi * H + hb * 128, bi * H + (hb + 1) * 128)
                    nc.tensor.matmul(ups[:], st["kt16"][:, hsl],
                                     mt16[bi][hb][:],
                                     start=(hb == 0),
                                     stop=(hb == HB - 1 and not cross))
                if cross:
                    sl = slice(bi * 128, (bi + 1) * 128)
                    nc.tensor.matmul(ups[:], st["x"][:, sl], G[c - 1]["dl"][bi][:],
                                     start=False, stop=True)
                # u_neg = r*ups - k (negated U; saves materializing -r)
                u_sb = upool.tile([128, H], f16, tag="u")
                nc.vector.scalar_tensor_tensor(u_sb[:], ups[:], st["r"][:, bi:bi + 1],
                                               st["k"][bi][:], mybir.AluOpType.mult,
                                               mybir.AluOpType.subtract)
                st["u"].append(u_sb[:])

        def state_delta(c):
            st = G[c]
            st["dl"] = []
            for bi in range(BPC):
                sl = slice(bi * 128, (bi + 1) * 128)
                dps = pshi.tile([128, H], f32, tag="big")
                nc.tensor.matmul(dps[:], st["g"][:, sl], st["u"][bi], start=True, stop=True)
                dl = dlpool.tile([128, H], f16, tag="dl")
                nc.scalar.copy(dl[:], dps[:])
                st["dl"].append(dl)

        def mupd(c, bis):
            # pair-end update: M^T += K_{c-1}^T Dl_{c-1} + K_c^T Dl_c
            # (bi 2,3 run at the start of the NEXT iteration to even PE load;
            #  some adds bounce PSUM->SBUF via DMA so Pool can do the add)
            st, stp = G[c], G[c - 1]
            for bi in bis:
                for jb in range(HB):
                    jsl = slice(bi * H + jb * 128, bi * H + (jb + 1) * 128)
                    mps = pshi.tile([128, H], f32, tag="big")
                    nc.tensor.matmul(mps[:], stp["k16"][:, jsl], stp["dl"][bi][:],
                                     start=True, stop=False)
                    nc.tensor.matmul(mps[:], st["k16"][:, jsl], st["dl"][bi][:],
                                     start=False, stop=True)
                    nc.vector.tensor_sub(mt16[bi][jb][:], mt16[bi][jb][:], mps[:])

        def mupd_single(c):
            # startup chunks 0 and 1: per-chunk M update (keeps PE busy early)
            st = G[c]
            for bi in range(BPC):
                for jb in range(HB):
                    jsl = slice(bi * H + jb * 128, bi * H + (jb + 1) * 128)
                    mps = pshi.tile([128, H], f32, tag="big")
                    nc.tensor.matmul(mps[:], st["k16"][:, jsl], st["dl"][bi][:],
                                     start=True, stop=True)
                    if c == 0:
                        nc.scalar.copy(mt16[bi][jb][:], mps[:])
                    else:
                        nc.vector.tensor_sub(mt16[bi][jb][:], mt16[bi][jb][:], mps[:])

        # ---- early prologue: read_proj weights + query + bias ----
        # (issued first: one of these DMAs expands to a long fine-grained
        # descriptor burst; at program start it overlaps the pipeline fill)
        # wtALL[:, ib*512 + op*128 + o] = W^T[ib*128 + i', op*128 + o]
        wtall = consts.tile([128, HB * H], f16, tag="wtall")
        for op in range(HB):
            wsb = wbuild.tile([128, H], f32, tag="wsb")
            nc.sync.dma_start(wsb[:], w_d[op * 128:(op + 1) * 128, :])
            wps = pshi.tile([128, H], f32, tag="big")
            for ib in range(HB):
                nc.tensor.transpose(wps[:, ib * 128:(ib + 1) * 128],
                                    wsb[:, ib * 128:(ib + 1) * 128], ident_f[:])
            nc.scalar.copy(
                wtall[:].rearrange("p (f o2 q) -> p f o2 q", f=HB, q=128)[:, :, op, :],
                wps[:])
        bias_all = consts.tile([BPC, H], f32, tag="biasall")
        for bi in range(BPC):
            nc.sync.dma_start(bias_all[bi:bi + 1, :], b_d[None, :])
        qs = []
        for bi in range(BPC):
            v4 = wbuild.tile([HB, 128], f32, tag="v4")
            nc.sync.dma_start(v4[:], hidden_d[bi, L - 1, :].rearrange("(f p) -> f p", p=128))
            tps = pslo.tile([128, HB], f32, tag="sm")
            nc.tensor.transpose(tps[:], v4[:], ident_f[:HB, :HB])
            q_t = consts.tile([128, HB], f16, tag=f"q{bi}", name=f"q{bi}")
            nc.scalar.copy(q_t[:], tps[:])
            qs.append(q_t)

        # ---- software-pipelined main loop ----
        # Issue order per iteration is tuned so each engine's in-order queue
        # services consumers before producers-for-later: PE never waits behind
        # unready work, Act chain copies aren't stuck behind squares, and the
        # k16->kt16 DMA for c+2 isn't stuck behind pair-end M adds.
        prep_load(0)
        prep_load(1)
        prep_load(2)
        prep_tr(0)
        prep_r(0)
        prep_tr(1)
        prep_r(1)
        aform(0)
        for lev in range(1, NLEV + 1):
            chain_level(0, lev)
        aform(1)
        for c in range(NCHUNK):
            nxt = c + 1 if c + 1 < NCHUNK else None
            nn = c + 2 if c + 2 < NCHUNK else None
            if c % 2 == 1 and c >= 3:
                xform(c)
            state_u(c)
            if nxt is not None:
                chain_sq(nxt, 1)
            state_delta(c)
            if nxt is not None:
                chain_gps(nxt, 1)
                chain_sq(nxt, 2)
            if c > 1 and c % 2 == 1:
                mupd(c, [0, 1])       # start the DVE subtract drain early
            if nn is not None:
                prep_tr(nn)
                prep_r(nn)
                aform_a(nn)           # astt lands mid-drain on DVE
            if c + 3 < NCHUNK:
                prep_load(c + 3)
            if c <= 1:
                mupd_single(c)
            elif c % 2 == 1:
                mupd(c, [2, 3])
            if nxt is not None:
                chain_gps(nxt, 2)
            if nn is not None:
                aform_b(nn)           # at-transposes covered by mupd bi2/3
            prev = c - 3
            if prev in G:
                del G[prev]

        # ---- finale: ctx = M q (row form); out = ctx W^T + b ----
        # phase-major over bi so PE/Act/DVE overlap across batch elements;
        # out-proj packs all 4 bi into one lhsT per ib block (4 matmuls total)
        cpss, ctx_rows = [], []
        for bi in range(BPC):
            cps = pshi.tile([1, H], f32, tag="big")
            for jb in range(HB):
                nc.tensor.matmul(cps[:], qs[bi][:, jb:jb + 1], mt16[bi][jb][:],
                                 start=(jb == 0), stop=(jb == HB - 1))
            cpss.append(cps)
        for bi in range(BPC):
            ctx_row = small.tile([1, H], f16, tag=f"ctxrow{bi}")
            nc.scalar.copy(ctx_row[:], cpss[bi][:])
            ctx_rows.append(ctx_row)
        ctxT = small.tile([128, HB * BPC], f16, tag="ctxT")
        for bi in range(BPC):
            for ib in range(HB):
                tp2 = pslo.tile([128, 1], f16, tag="sm1")
                nc.tensor.transpose(tp2[:], ctx_rows[bi][:, ib * 128:(ib + 1) * 128],
                                    ident_h[:1, :1])
                nc.vector.tensor_copy(ctxT[:, ib * BPC + bi:ib * BPC + bi + 1], tp2[:])
        ops4 = pshi.tile([BPC, H], f32, tag="big")
        for ib in range(HB):
            nc.tensor.matmul(ops4[:], ctxT[:, ib * BPC:(ib + 1) * BPC],
                             wtall[:, ib * H:(ib + 1) * H],
                             start=(ib == 0), stop=(ib == HB - 1))
        out_all = small.tile([BPC, H], f32, tag="outall")
        nc.vector.tensor_add(out_all[:], ops4[:], bias_all[:])
        nc.sync.dma_start(out_d[:, :], out_all[:])

    if legalize:
        _legalize_waits(nc)
    return nc


def _legalize_waits(nc, max_waits=1):
    """This toolchain's walrus encodes at most one semaphore wait per
    instruction. Hoist extra waits onto standalone EventSemaphore
    instructions on the same engine queue, immediately before the owner."""
    import json as _json
    m = _json.loads(bytes(nc.to_json_bytes()))
    n_fix = 0
    for fn in m["functions"]:
        for blk in fn["blocks"]:
            out = []
            for ins in blk.get("instructions", []):
                si = ins.get("sync_info") or {}
                waits = si.get("on_wait") or []
                if len(waits) > max_waits and ins.get("opcode") != "EventSemaphore":
                    extra, keep = waits[:-max_waits], waits[-max_waits:]
                    for i, w in enumerate(extra):
                        out.append({
                            "name": f"{ins['name']}-w{i}",
                            "engine": ins["engine"],
                            "opcode": "EventSemaphore",
                            "ins": [], "outs": [],
                            "sync_info": {"on_wait": [w], "on_update": []},
                        })
                    si["on_wait"] = keep
                    ins["sync_info"] = si
                    n_fix += 1
                out.append(ins)
            blk["instructions"] = out
    nc.m = mybir.module_from_json_bytes(_json.dumps(m).encode())
    return n_fix


def kernel(hidden: np.ndarray, W: np.ndarray, b: np.ndarray) -> np.ndarray:
    if "nc" not in _cached:
        _cached["nc"] = _build_program()
    nc = _cached["nc"]

    hidden = np.ascontiguousarray(hidden, dtype=np.float32)
    W = np.ascontiguousarray(W, dtype=np.float32)
    b = np.ascontiguousarray(b, dtype=np.float32)

    in_maps = []
    for ci in range(NCORES):
        in_maps.append({
            "hidden": hidden[ci * BPC:(ci + 1) * BPC],
            "W": W,
            "bvec": b,
        })
    res = run_bass_kernel_spmd(nc, in_maps, core_ids=list(range(NCORES)))
    _cached["last_results"] = res
    out = np.concatenate([res.results[ci]["out"] for ci in range(NCORES)], axis=0)
    return out.astype(np.float32)


if __name__ == "__main__":
    rng = np.random.default_rng(0)
    h = rng.standard_normal((B, L, H), dtype=np.float32)
    w = rng.standard_normal((H, H), dtype=np.float32) * (1.0 / np.sqrt(H))
    bb = np.zeros((H,), np.float32)
    o = kernel(h, w, bb)
    print(o.shape, o.dtype)
